# revision 10
# baseline (speedup 1.0000x reference)
"""GCN (EnhancedTaintFlowGNN) on 8 Trainium2 NeuronCores.

Sharding: 32 graphs/core (batch sorted -> contiguous node ranges). Per GCN
layer: M = h @ W locally (PE), AllGather M into a pair-Shared HBM table,
then pull-side aggregation WITHOUT seg matrices: dma_gather source rows
(per-owner int16 tables), PE per-block run-reduction (R matrices with the
GCN norm coefs folded in, runs never cross block boundaries so output
slots have unique dst per owner region), dma_scatter_add of the reduced
f32 rows into a local agg slab (bias folded into the self-loop init
write; scatter calls are WAW-serialized by tile so cross-call duplicate
dsts cannot race the non-atomic CCE add). Layer 0 gathers rows of
T0 = emb @ w0 (replicated vocab table) so no exchange is needed.
Pooling: masked DVE mean/max on transposed features; classifier MLP on
PE.
"""
import numpy as np
import ml_dtypes

import concourse.bass as bass
import concourse.bacc as bacc
import concourse.tile as tile
from concourse import mybir
from concourse.bass_utils import run_bass_kernel_spmd
from concourse.masks import make_identity

P = 128
N, E, G, V, D, H = 150000, 300000, 256, 1000, 128, 256
CORES = 8
GPC = G // CORES
CH = 8         # 128-slot blocks per gather/scatter call (1024-desc ring cap)
BF = ml_dtypes.bfloat16
F32 = np.float32


def _idx16_layout(idx):
    n = idx.shape[0]
    s = n // 16
    out = np.zeros((P, s), np.int16)
    blk = idx.reshape(s, 16).T.astype(np.int16)
    for g in range(8):
        out[g * 16:(g + 1) * 16, :] = blk
    return out


def _pad128(x):
    return (int(x) + P - 1) // P * P


def _region_layout(dst_loc):
    """Positions for dst-sorted slots such that no dst run crosses a
    128 boundary. Returns (order, slot_pos, run_id, run_j, run_dst,
    padded_len). run_j = out-slot index of each run within its block."""
    order = np.argsort(dst_loc, kind="stable")
    ds = dst_loc[order]
    n = len(ds)
    if n == 0:
        return order, np.zeros(0, np.int64), None, None, None, 0
    starts = np.flatnonzero(np.r_[True, ds[1:] != ds[:-1]])
    runlen = np.diff(np.r_[starts, n])
    run_start = np.zeros(len(starts), np.int64)
    pos = 0
    for i, L in enumerate(runlen):
        if pos % P + L > P:
            pos = _pad128(pos)
        run_start[i] = pos
        pos += L
    run_id = np.repeat(np.arange(len(starts)), runlen)
    slot_pos = run_start[run_id] + (np.arange(n) - starts[run_id])
    # out-slot index per run within its block = rank - first_rank_of_block
    rb = run_start // P
    _, first = np.unique(rb, return_index=True)
    first_of_block = np.zeros(rb.max() + 1, np.int64)
    first_of_block[rb[first]] = np.arange(len(starts))[first]
    run_j = np.arange(len(starts)) - first_of_block[rb]
    run_dst = ds[starts]
    return order, slot_pos, run_id, (rb, run_j), run_dst, _pad128(pos)


def _fill_region(gidx, sidx, Rf, base, src_loc, dst_loc, coef):
    """Fill gidx/sidx/R for one owner region starting at slot `base`."""
    order, slot_pos, run_id, rj, run_dst, plen = _region_layout(dst_loc)
    if plen == 0:
        return 0
    rb, run_j = rj
    sp = base + slot_pos
    gidx[sp] = src_loc[order]
    blk = sp // P
    Rf[blk, sp % P, run_j[run_id]] = coef[order]
    base_blk = base // P
    sidx[(base_blk + rb) * P + run_j] = run_dst
    return plen


def _preprocess(node_ids, edge_index, batch):
    node_ids = np.asarray(node_ids).astype(np.int64)
    src_g = np.asarray(edge_index)[0].astype(np.int64)
    dst_g = np.asarray(edge_index)[1].astype(np.int64)
    batch = np.asarray(batch).astype(np.int64)

    deg = 1.0 + np.bincount(dst_g, minlength=N).astype(np.float64)
    dinv = (1.0 / np.sqrt(deg)).astype(F32)
    selfnorm = (dinv * dinv).astype(F32)

    gcnt = np.bincount(batch, minlength=G)
    goff = np.zeros(G + 1, np.int64)
    goff[1:] = np.cumsum(gcnt)
    node_start = np.array([int(goff[c * GPC]) for c in range(CORES + 1)])
    n_c = node_start[1:] - node_start[:-1]
    TILES = int(np.ceil(n_c.max() / P))
    SLAB = TILES * P
    assert SLAB <= 32767, SLAB

    e_oc = np.searchsorted(node_start[1:], dst_g, side="right")
    e_os = np.searchsorted(node_start[1:], src_g, side="right")

    # ---- measure padded region lengths (uniform across cores) ----
    len1 = np.zeros((CORES, CORES), np.int64)
    len0 = np.zeros(CORES, np.int64)
    per_edge = []
    for c in range(CORES):
        m = e_oc == c
        srcs, dsts, owns = src_g[m], dst_g[m], e_os[m]
        dstl = dsts - node_start[c]
        coefs = dinv[srcs] * dinv[dsts]
        per_edge.append((srcs, dstl, owns, coefs))
        for o in range(CORES):
            mo = owns == o
            _, _, _, _, _, plen = _region_layout(dstl[mo])
            len1[c, o] = plen
        _, _, _, _, _, plen = _region_layout(dstl)
        len0[c] = plen
    K1 = [_pad128(len1[:, o].max()) for o in range(CORES)]
    off1 = np.zeros(CORES + 1, np.int64)
    off1[1:] = np.cumsum(K1)
    S1 = int(off1[-1])
    S0 = _pad128(len0.max())
    NB0, NB1 = S0 // P, S1 // P

    per_core = []
    for c in range(CORES):
        srcs, dstl, owns, coefs = per_edge[c]

        gidx1 = np.zeros(S1, np.int64)
        sidx1 = np.full(S1, SLAB, np.int64)
        R1 = np.zeros((NB1, P, P), F32)
        for o in range(CORES):
            mo = owns == o
            _fill_region(gidx1, sidx1, R1, int(off1[o]),
                         srcs[mo] - node_start[o], dstl[mo], coefs[mo])

        gidx0 = np.zeros(S0, np.int64)
        sidx0 = np.full(S0, SLAB, np.int64)
        R0 = np.zeros((NB0, P, P), F32)
        _fill_region(gidx0, sidx0, R0, 0, node_ids[srcs], dstl, coefs)

        gidxs = np.zeros(SLAB, np.int64)
        gidxs[:n_c[c]] = node_ids[node_start[c]:node_start[c + 1]]
        selfn = np.zeros((P, TILES), F32)
        ar = np.arange(int(n_c[c]))
        selfn[ar % P, ar // P] = selfnorm[node_start[c]:node_start[c + 1]]

        # R stored partition-major for contiguous chunk loads:
        # Rpm[p, blk*128 + j] = R[blk, p, j]
        Rall = np.concatenate([R0, R1], 0)           # [NB0+NB1, 128, 128]
        Rpm = np.ascontiguousarray(
            Rall.transpose(1, 0, 2).reshape(P, -1).astype(BF))

        per_core.append(dict(
            gidx1=_idx16_layout(gidx1.astype(np.int16)),
            sidx1=_idx16_layout(sidx1.astype(np.int16)),
            gidx0=_idx16_layout(gidx0.astype(np.int16)),
            sidx0=_idx16_layout(sidx0.astype(np.int16)),
            gidxs=_idx16_layout(gidxs.astype(np.int16)),
            rmat=Rpm, selfn=selfn,
        ))

    # graph cover ranges for pooling (uniform across cores)
    glo_all = np.stack([goff[c * GPC:(c + 1) * GPC] - node_start[c]
                        for c in range(CORES)])   # [CORES, GPC]
    ghi_all = np.stack([goff[c * GPC + 1:(c + 1) * GPC + 1] - node_start[c]
                        for c in range(CORES)])
    lo_fix = glo_all.min(0)
    hi_fix = ghi_all.max(0)
    MAXCOV = int((hi_fix - lo_fix).max())
    MAXCOV = (MAXCOV + 31) // 32 * 32
    cov_len = np.minimum(MAXCOV, SLAB - lo_fix)

    for c in range(CORES):
        mask = np.zeros((GPC, MAXCOV), F32)
        cinv = np.zeros(GPC, F32)
        for j in range(GPC):
            lo = int(glo_all[c, j]) - int(lo_fix[j])
            hi = int(ghi_all[c, j]) - int(lo_fix[j])
            mask[j, lo:hi] = 1.0
            cnt_ = int(ghi_all[c, j] - glo_all[c, j])
            cinv[j] = 1.0 / max(cnt_, 1)
        maskb = np.broadcast_to(mask[:, None, :], (GPC, P, MAXCOV))
        per_core[c]["mask"] = np.ascontiguousarray(maskb.astype(BF))
        per_core[c]["cinv"] = np.broadcast_to(cinv, (P, GPC)).copy()

    meta = dict(TILES=TILES, SLAB=SLAB, S0=S0, S1=S1,
                K1=[int(k) for k in K1], off1=[int(o) for o in off1],
                lo_fix=lo_fix, cov_len=cov_len, MAXCOV=MAXCOV)
    return per_core, meta


def _chunks(meta, layer):
    """Static (slot_base, nblocks, owner) chunk list; identical on all
    cores. Chunks never span owner regions."""
    out = []
    if layer == 0:
        nb_tot = meta["S0"] // P
        for b0 in range(0, nb_tot, CH):
            out.append((b0 * P, min(CH, nb_tot - b0), 0))
    else:
        for o in range(CORES):
            base, nb_tot = meta["off1"][o], meta["K1"][o] // P
            for b0 in range(0, nb_tot, CH):
                out.append((base + b0 * P, min(CH, nb_tot - b0), o))
    return out


def _rgroups(chunks, cap=16):
    """Group consecutive chunks into runs of <= cap blocks (for one big
    R-matrix load per group; chunk slots are contiguous within a layer)."""
    groups, cur, acc = [], [], 0
    for ch in chunks:
        if acc + ch[1] > cap and cur:
            groups.append(cur)
            cur, acc = [], 0
        cur.append(ch)
        acc += ch[1]
    if cur:
        groups.append(cur)
    return groups


def _build(meta):
    TILES, SLAB = meta["TILES"], meta["SLAB"]
    S0, S1 = meta["S0"], meta["S1"]
    NB0 = S0 // P
    MAXCOV = meta["MAXCOV"]
    f32, bf16, i16 = mybir.dt.float32, mybir.dt.bfloat16, mybir.dt.int16
    AF = mybir.ActivationFunctionType
    nc = bacc.Bacc("TRN2", target_bir_lowering=False, debug=False,
                   num_devices=CORES)

    gidx1_in = nc.dram_tensor("gidx1", [P, S1 // 16], i16, kind="ExternalInput")
    sidx1_in = nc.dram_tensor("sidx1", [P, S1 // 16], i16, kind="ExternalInput")
    gidx0_in = nc.dram_tensor("gidx0", [P, S0 // 16], i16, kind="ExternalInput")
    sidx0_in = nc.dram_tensor("sidx0", [P, S0 // 16], i16, kind="ExternalInput")
    gidxs_in = nc.dram_tensor("gidxs", [P, SLAB // 16], i16,
                              kind="ExternalInput")
    rmat_in = nc.dram_tensor("rmat", [P, (S0 + S1)], bf16,
                             kind="ExternalInput")
    selfn_in = nc.dram_tensor("selfn", [P, TILES], f32, kind="ExternalInput")
    mask_in = nc.dram_tensor("mask", [GPC, P, MAXCOV], bf16,
                             kind="ExternalInput")
    cinv_in = nc.dram_tensor("cinv", [P, GPC], f32, kind="ExternalInput")
    embT_in = nc.dram_tensor("embT", [P, 1024], bf16, kind="ExternalInput")
    w0_in = nc.dram_tensor("w0", [P, H], bf16, kind="ExternalInput")
    wl_in = nc.dram_tensor("wl", [P, 3, 2, H], bf16, kind="ExternalInput")
    b_in = nc.dram_tensor("bias", [P, 4, H], f32, kind="ExternalInput")
    cw0_in = nc.dram_tensor("cw0", [P, 4, H], bf16, kind="ExternalInput")
    cb0_in = nc.dram_tensor("cb0", [GPC, H], f32, kind="ExternalInput")
    cw1_in = nc.dram_tensor("cw1", [P, 2, 2], bf16, kind="ExternalInput")
    cb1_in = nc.dram_tensor("cb1", [GPC, 2], f32, kind="ExternalInput")
    out = nc.dram_tensor("out", [GPC, 2], f32, kind="ExternalOutput")

    T0_d = nc.dram_tensor("T0d", [1024, H], bf16)
    MB = nc.dram_tensor("MBd", [SLAB, H], bf16)
    MFs = [nc.dram_tensor(f"MF{i}", [CORES * SLAB, H], bf16,
                          addr_space="Shared") for i in range(2)]
    AGG = nc.dram_tensor("AGGd", [SLAB + P, H], f32)

    MB_v = None  # set below once tensors exist

    with tile.TileContext(nc) as tc:
        with (
            tc.tile_pool(name="const", bufs=1) as cpool,
            tc.tile_pool(name="gat", bufs=2) as gpool,
            tc.tile_pool(name="rp", bufs=2) as rpool,
            tc.tile_pool(name="sca", bufs=2) as spool,
            tc.tile_pool(name="work", bufs=5) as work,
            tc.tile_pool(name="mst", bufs=2) as mst,
            tc.tile_pool(name="ist", bufs=2) as ist,
            tc.tile_pool(name="rst", bufs=2) as rst,
            tc.tile_pool(name="hmp", bufs=2) as hmp,
            tc.tile_pool(name="pstr", bufs=2, space="PSUM") as pstr,
            tc.tile_pool(name="psmm", bufs=2, space="PSUM") as psmm,
            tc.tile_pool(name="psrd", bufs=2, space="PSUM") as psrd,
        ):
            gidx1_t = cpool.tile([P, S1 // 16], i16)
            nc.sync.dma_start(out=gidx1_t[:], in_=gidx1_in[:, :])
            sidx1_t = cpool.tile([P, S1 // 16], i16)
            nc.sync.dma_start(out=sidx1_t[:], in_=sidx1_in[:, :])
            gidx0_t = cpool.tile([P, S0 // 16], i16)
            nc.sync.dma_start(out=gidx0_t[:], in_=gidx0_in[:, :])
            sidx0_t = cpool.tile([P, S0 // 16], i16)
            nc.sync.dma_start(out=sidx0_t[:], in_=sidx0_in[:, :])
            gidxs_t = cpool.tile([P, SLAB // 16], i16)
            nc.sync.dma_start(out=gidxs_t[:], in_=gidxs_in[:, :])
            selfn_t = cpool.tile([P, TILES], f32)
            nc.sync.dma_start(out=selfn_t[:], in_=selfn_in[:, :])
            cinv_t = cpool.tile([P, GPC], f32)
            nc.sync.dma_start(out=cinv_t[:], in_=cinv_in[:, :])
            embT_t = cpool.tile([P, 1024], bf16)
            nc.sync.dma_start(out=embT_t[:], in_=embT_in[:, :])
            w0_t = cpool.tile([P, H], bf16)
            nc.sync.dma_start(out=w0_t[:], in_=w0_in[:, :])
            wl_t = cpool.tile([P, 3, 2, H], bf16)
            nc.sync.dma_start(out=wl_t[:], in_=wl_in[:, :, :, :])
            bias_t = cpool.tile([P, 4, H], f32)
            nc.sync.dma_start(out=bias_t[:], in_=b_in[:, :, :])

            h_T = nc.alloc_sbuf_tensor("hT", [P, 2, SLAB], bf16)
            ident = cpool.tile([P, P], bf16)
            make_identity(nc, ident[:])

            AGG_v = AGG[:, :].rearrange("(b p) e -> p b e", p=P)
            MB_v = MB[:, :].rearrange("(b p) e -> p b e", p=P)
            T0_v = T0_d[:, :].rearrange("(b p) e -> p b e", p=P)

            # ---- T0 = emb @ w0 ----
            t0st = mst.tile([P, CH, H], bf16, tag="m")
            for vb in range(8):
                pt0 = psmm.tile([P, H], f32, tag="mm")
                nc.tensor.matmul(out=pt0[:], lhsT=embT_t[:, vb * P:(vb + 1) * P],
                                 rhs=w0_t[:], start=True, stop=True)
                nc.scalar.copy(out=t0st[:, vb, :], in_=pt0[:])
            nc.sync.dma_start(out=T0_v[:, 0:8, :], in_=t0st[:])

            def edge_phase(layer, mf):
                gidx_t = gidx0_t if layer == 0 else gidx1_t
                sidx_t = sidx0_t if layer == 0 else sidx1_t
                rbase = 0 if layer == 0 else S0
                for grp in _rgroups(_chunks(meta, layer)):
                    g0 = grp[0][0]            # first slot of group
                    gnb = sum(ch[1] for ch in grp)
                    rt = rpool.tile([P, 16 * P], bf16)
                    nc.sync.dma_start(
                        out=rt[:, 0:gnb * P],
                        in_=rmat_in[:, rbase + g0:rbase + g0 + gnb * P])
                    for (base, nb, o) in grp:
                        gt = gpool.tile([P, CH, H], bf16)
                        tab = (T0_d[0:1024, :] if layer == 0
                               else mf[o * SLAB:(o + 1) * SLAB, :])
                        nc.gpsimd.dma_gather(
                            out_ap=gt[:, 0:nb, :], in_ap=tab,
                            idxs_ap=gidx_t[:, base // 16:(base + nb * P) // 16],
                            num_idxs=nb * P, num_idxs_reg=nb * P,
                            elem_size=H, queue_num=0)
                        sf = spool.tile([P, CH, H], f32)
                        for b in range(nb):
                            rb = (base - g0) // P + b
                            pr = psrd.tile([P, H], f32, tag="rd")
                            nc.tensor.matmul(out=pr[:],
                                             lhsT=rt[:, rb * P:(rb + 1) * P],
                                             rhs=gt[:, b, :],
                                             start=True, stop=True)
                            nc.vector.tensor_copy(out=sf[:, b, :], in_=pr[:])
                        nc.gpsimd.dma_scatter_add(
                            AGG[:, :], sf[:, 0:nb, :],
                            sidx_t[:, base // 16:(base + nb * P) // 16],
                            nb * P, nb * P, H, queue_num=0)

            def readback_phase(layer):
                for t0 in range(0, TILES, CH):
                    nb = min(CH, TILES - t0)
                    at = rst.tile([P, CH, H], f32)
                    nc.sync.dma_start(out=at[:, 0:nb, :],
                                      in_=AGG_v[:, t0:t0 + nb, :])
                    for b in range(nb):
                        t = t0 + b
                        hb = work.tile([P, H], bf16)
                        nc.scalar.activation(out=hb[:], in_=at[:, b, :],
                                             func=AF.Relu)
                        for fh in range(2):
                            ptr_ = pstr.tile([P, P], bf16)
                            nc.tensor.transpose(
                                out=ptr_[:], in_=hb[:, fh * P:(fh + 1) * P],
                                identity=ident[:])
                            nc.vector.tensor_copy(
                                out=h_T[:, fh, t * P:(t + 1) * P], in_=ptr_[:])

            # ---- layer 0: agg init from T0 self rows (+bias), edge scatter --
            nbs = SLAB // P
            for b0 in range(0, nbs, CH):
                nb = min(CH, nbs - b0)
                gt = gpool.tile([P, CH, H], bf16)
                nc.gpsimd.dma_gather(
                    out_ap=gt[:, 0:nb, :], in_ap=T0_d[0:1024, :],
                    idxs_ap=gidxs_t[:, b0 * 8:(b0 + nb) * 8],
                    num_idxs=nb * P, num_idxs_reg=nb * P,
                    elem_size=H, queue_num=0)
                sf = ist.tile([P, CH, H], f32, tag="i")
                for b in range(nb):
                    t = b0 + b
                    nc.scalar.activation(out=sf[:, b, :], in_=gt[:, b, :],
                                         func=AF.Copy,
                                         scale=selfn_t[:, t:t + 1])
                    nc.vector.tensor_add(out=sf[:, b, :], in0=sf[:, b, :],
                                         in1=bias_t[:, 0, :])
                nc.sync.dma_start(out=AGG_v[:, b0:b0 + nb, :],
                                  in_=sf[:, 0:nb, :])
            edge_phase(0, None)
            readback_phase(0)

            for layer in (1, 2, 3):
                mf = MFs[layer % 2]
                # M = h @ W -> MB; agg init = selfnorm*M + bias
                for t0 in range(0, TILES, CH):
                    nb = min(CH, TILES - t0)
                    mtile = mst.tile([P, CH, H], bf16, tag="m")
                    itile = ist.tile([P, CH, H], f32, tag="i")
                    for b in range(nb):
                        t = t0 + b
                        pm = psmm.tile([P, H], f32, tag="mm")
                        for fh in range(2):
                            nc.tensor.matmul(
                                out=pm[:],
                                lhsT=h_T[:, fh, t * P:(t + 1) * P],
                                rhs=wl_t[:, layer - 1, fh, :],
                                start=(fh == 0), stop=(fh == 1))
                        nc.scalar.copy(out=mtile[:, b, :], in_=pm[:])
                        nc.scalar.activation(out=itile[:, b, :], in_=pm[:],
                                             func=AF.Copy,
                                             scale=selfn_t[:, t:t + 1])
                        nc.vector.tensor_add(out=itile[:, b, :],
                                             in0=itile[:, b, :],
                                             in1=bias_t[:, layer, :])
                    nc.sync.dma_start(out=MB_v[:, t0:t0 + nb, :],
                                      in_=mtile[:, 0:nb, :])
                    nc.sync.dma_start(out=AGG_v[:, t0:t0 + nb, :],
                                      in_=itile[:, 0:nb, :])
                nc.gpsimd.collective_compute(
                    "AllGather", mybir.AluOpType.bypass,
                    replica_groups=[list(range(CORES))],
                    ins=[MB[:].opt()], outs=[mf[:].opt()])
                edge_phase(layer, mf)
                readback_phase(layer)

            # ---- pooling (masked mean/max on h_T) ----
            pooled = []
            for fh in range(2):
                mean_t = cpool.tile([P, GPC], f32, tag=f"mean{fh}")
                max_t = cpool.tile([P, GPC], f32, tag=f"max{fh}")
                nc.vector.memset(mean_t[:], 0.0)
                nc.vector.memset(max_t[:], 0.0)
                pooled.append((mean_t, max_t))
            lo_fix, cov_len = meta["lo_fix"], meta["cov_len"]
            for j in range(GPC):
                mk = hmp.tile([P, MAXCOV], bf16, tag="mask")
                ln = int(cov_len[j])
                nc.sync.dma_start(out=mk[:, 0:ln], in_=mask_in[j, :, 0:ln])
                for fh in range(2):
                    hm = hmp.tile([P, MAXCOV], bf16, tag="hm")
                    lo = int(lo_fix[j])
                    nc.vector.tensor_mul(out=hm[:, 0:ln],
                                         in0=h_T[:, fh, lo:lo + ln],
                                         in1=mk[:, 0:ln])
                    nc.vector.tensor_reduce(
                        out=pooled[fh][0][:, j:j + 1], in_=hm[:, 0:ln],
                        axis=mybir.AxisListType.X, op=mybir.AluOpType.add)
                    nc.vector.tensor_reduce(
                        out=pooled[fh][1][:, j:j + 1], in_=hm[:, 0:ln],
                        axis=mybir.AxisListType.X, op=mybir.AluOpType.max)
            # scale means by 1/cnt, cast to bf16 lhsT chunks
            chunks = []
            for fh in range(2):
                mean_t, max_t = pooled[fh]
                nc.vector.tensor_mul(out=mean_t[:], in0=mean_t[:],
                                     in1=cinv_t[:])
            for (kind, fh) in ((0, 0), (0, 1), (1, 0), (1, 1)):
                src = pooled[fh][kind]
                cb = work.tile([P, GPC], bf16, tag=f"ch{kind}{fh}")
                nc.vector.tensor_copy(out=cb[:], in_=src[:])
                chunks.append(cb)

            # ---- classifier MLP ----
            cw0_t = cpool.tile([P, 4, H], bf16)
            nc.sync.dma_start(out=cw0_t[:], in_=cw0_in[:, :, :])
            cb0_t = cpool.tile([GPC, H], f32)
            nc.sync.dma_start(out=cb0_t[:], in_=cb0_in[:, :])
            cw1_t = cpool.tile([P, 2, 2], bf16)
            nc.sync.dma_start(out=cw1_t[:], in_=cw1_in[:, :, :])
            cb1_t = cpool.tile([GPC, 2], f32)
            nc.sync.dma_start(out=cb1_t[:], in_=cb1_in[:, :])

            ph_full = psmm.tile([P, H], f32, tag="mm")
            ph = ph_full[0:GPC, :]
            for k in range(4):
                nc.tensor.matmul(out=ph[:], lhsT=chunks[k][:],
                                 rhs=cw0_t[:, k, :],
                                 start=(k == 0), stop=(k == 3))
            hc1 = work.tile([GPC, H], f32, tag="hc1")
            nc.vector.tensor_add(out=hc1[:], in0=ph[:], in1=cb0_t[:])
            hcb = work.tile([GPC, H], bf16, tag="hcb")
            nc.scalar.activation(out=hcb[:], in_=hc1[:], func=AF.Relu)
            hTt = []
            for k in range(2):
                ptr_ = pstr.tile([P, P], bf16)
                nc.tensor.transpose(out=ptr_[0:P, 0:GPC],
                                    in_=hcb[:, k * P:(k + 1) * P],
                                    identity=ident[0:GPC, 0:GPC])
                ht = work.tile([P, GPC], bf16, tag=f"hTt{k}")
                nc.vector.tensor_copy(out=ht[:], in_=ptr_[0:P, 0:GPC])
                hTt.append(ht)
            pl_full = psmm.tile([P, H], f32, tag="mm")
            pl = pl_full[0:GPC, 0:2]
            for k in range(2):
                nc.tensor.matmul(out=pl[:], lhsT=hTt[k][:],
                                 rhs=cw1_t[:, k, :],
                                 start=(k == 0), stop=(k == 1))
            lg = work.tile([GPC, 2], f32, tag="lg")
            nc.vector.tensor_add(out=lg[:], in0=pl[:], in1=cb1_t[:])
            nc.sync.dma_start(out=out[:, :], in_=lg[:])
    nc.finalize()
    return nc


def kernel(node_ids, edge_index, batch, emb, w0, b0, w1, b1, w2, b2, w3, b3,
           cw0, cb0, cw1, cb1):
    per_core, meta = _preprocess(node_ids, edge_index, batch)
    nc = _build(meta)

    embT = np.zeros((P, 1024), F32)
    embT[:, :V] = np.asarray(emb, F32).T
    wlk = np.transpose(np.stack([np.asarray(w, F32).reshape(2, P, H)
                    for w in (w1, w2, w3)]), (2, 0, 1, 3)).copy()
    biases = np.stack([np.broadcast_to(np.asarray(b, F32), (P, H))
                       for b in (b0, b1, b2, b3)], axis=1).copy()
    ins = []
    for c in range(CORES):
        pc = per_core[c]
        ins.append(dict(
            gidx1=pc["gidx1"], sidx1=pc["sidx1"],
            gidx0=pc["gidx0"], sidx0=pc["sidx0"],
            gidxs=pc["gidxs"], rmat=pc["rmat"], selfn=pc["selfn"],
            mask=pc["mask"], cinv=pc["cinv"],
            embT=embT.astype(BF), w0=np.asarray(w0, F32).astype(BF),
            wl=wlk.astype(BF), bias=biases.astype(F32),
            cw0=np.transpose(np.asarray(cw0, F32).reshape(4, P, H), (1, 0, 2)).astype(BF),
            cb0=np.broadcast_to(np.asarray(cb0, F32), (GPC, H)).copy(),
            cw1=np.transpose(np.asarray(cw1, F32).reshape(2, P, 2), (1, 0, 2)).astype(BF),
            cb1=np.broadcast_to(np.asarray(cb1, F32), (GPC, 2)).copy(),
        ))
    trace = False
    try:  # register NTFF hook so exec_time_ns is measurable (best effort)
        import sys, types
        import antenv
        if "antenv.axon_hooks" not in sys.modules:
            hooks = types.ModuleType("antenv.axon_hooks")
            hooks._h = None
            hooks.set_axon_ntff_profile_hook = lambda h: setattr(hooks, "_h", h)
            hooks.get_axon_ntff_profile_hook = lambda: hooks._h
            sys.modules["antenv.axon_hooks"] = hooks
            antenv.axon_hooks = hooks
            from trn_agent_boot.trn_boot import _ntff_profile_via_ctypes
            hk = _ntff_profile_via_ctypes("/opt/axon/libaxon_pjrt.so")
            if hk is not None:
                hooks.set_axon_ntff_profile_hook(hk)
                trace = True
        else:
            trace = True
    except Exception:
        trace = False
    res = run_bass_kernel_spmd(nc, ins, core_ids=list(range(CORES)),
                               trace=trace)
    logits = np.concatenate([res.results[c]["out"] for c in range(CORES)], 0)
    globals()["last_exec_ns"] = res.exec_time_ns
    return logits.astype(np.float32)


# revision 11
# speedup vs baseline: 1.1231x; 1.1231x over previous
"""GCN (EnhancedTaintFlowGNN) on 8 Trainium2 NeuronCores.

Sharding: 32 graphs/core (batch sorted -> contiguous node ranges). Per GCN
layer: M = h @ W locally (PE), AllGather M into a pair-Shared HBM table,
then pull-side aggregation WITHOUT seg matrices: dma_gather source rows
(per-owner int16 tables), PE per-block run-reduction (R matrices with the
GCN norm coefs folded in, runs never cross block boundaries so output
slots have unique dst per owner region), dma_scatter_add of the reduced
f32 rows into a local agg slab (bias folded into the self-loop init
write; scatter calls are WAW-serialized by tile so cross-call duplicate
dsts cannot race the non-atomic CCE add). Layer 0 gathers rows of
T0 = emb @ w0 (replicated vocab table) so no exchange is needed.
Pooling: masked DVE mean/max on transposed features; classifier MLP on
PE.
"""
import numpy as np
import ml_dtypes

import concourse.bass as bass
import concourse.bacc as bacc
import concourse.tile as tile
from concourse import mybir
from concourse.bass_utils import run_bass_kernel_spmd
from concourse.masks import make_identity

P = 128
N, E, G, V, D, H = 150000, 300000, 256, 1000, 128, 256
CORES = 8
GPC = G // CORES
CH = 8         # 128-slot blocks per gather/scatter call (1024-desc ring cap)
BF = ml_dtypes.bfloat16
F32 = np.float32


def _idx16_layout(idx):
    n = idx.shape[0]
    s = n // 16
    out = np.zeros((P, s), np.int16)
    blk = idx.reshape(s, 16).T.astype(np.int16)
    for g in range(8):
        out[g * 16:(g + 1) * 16, :] = blk
    return out


def _pad128(x):
    return (int(x) + P - 1) // P * P


def _pm(idx, nblk):
    """Partition-major row remap: node d -> (d%128)*nblk + d//128."""
    return (idx % P) * nblk + idx // P


def _region_layout(dst_loc):
    """Positions for dst-sorted slots such that no dst run crosses a
    128 boundary. Returns (order, slot_pos, run_id, run_j, run_dst,
    padded_len). run_j = out-slot index of each run within its block."""
    order = np.argsort(dst_loc, kind="stable")
    ds = dst_loc[order]
    n = len(ds)
    if n == 0:
        return order, np.zeros(0, np.int64), None, None, None, 0
    starts = np.flatnonzero(np.r_[True, ds[1:] != ds[:-1]])
    runlen = np.diff(np.r_[starts, n])
    run_start = np.zeros(len(starts), np.int64)
    pos = 0
    for i, L in enumerate(runlen):
        if pos % P + L > P:
            pos = _pad128(pos)
        run_start[i] = pos
        pos += L
    run_id = np.repeat(np.arange(len(starts)), runlen)
    slot_pos = run_start[run_id] + (np.arange(n) - starts[run_id])
    # out-slot index per run within its block = rank - first_rank_of_block
    rb = run_start // P
    _, first = np.unique(rb, return_index=True)
    first_of_block = np.zeros(rb.max() + 1, np.int64)
    first_of_block[rb[first]] = np.arange(len(starts))[first]
    run_j = np.arange(len(starts)) - first_of_block[rb]
    run_dst = ds[starts]
    return order, slot_pos, run_id, (rb, run_j), run_dst, _pad128(pos)


def _fill_region(gidx, sidx, Rf, base, src_loc, dst_loc, coef):
    """Fill gidx/sidx/R for one owner region starting at slot `base`."""
    order, slot_pos, run_id, rj, run_dst, plen = _region_layout(dst_loc)
    if plen == 0:
        return 0
    rb, run_j = rj
    sp = base + slot_pos
    gidx[sp] = src_loc[order]
    blk = sp // P
    Rf[blk, sp % P, run_j[run_id]] = coef[order]
    base_blk = base // P
    sidx[(base_blk + rb) * P + run_j] = run_dst
    return plen


def _preprocess(node_ids, edge_index, batch):
    node_ids = np.asarray(node_ids).astype(np.int64)
    src_g = np.asarray(edge_index)[0].astype(np.int64)
    dst_g = np.asarray(edge_index)[1].astype(np.int64)
    batch = np.asarray(batch).astype(np.int64)

    deg = 1.0 + np.bincount(dst_g, minlength=N).astype(np.float64)
    dinv = (1.0 / np.sqrt(deg)).astype(F32)
    selfnorm = (dinv * dinv).astype(F32)

    gcnt = np.bincount(batch, minlength=G)
    goff = np.zeros(G + 1, np.int64)
    goff[1:] = np.cumsum(gcnt)
    node_start = np.array([int(goff[c * GPC]) for c in range(CORES + 1)])
    n_c = node_start[1:] - node_start[:-1]
    TILES = int(np.ceil(n_c.max() / P))
    SLAB = TILES * P
    assert SLAB <= 32767, SLAB

    e_oc = np.searchsorted(node_start[1:], dst_g, side="right")
    e_os = np.searchsorted(node_start[1:], src_g, side="right")

    # ---- measure padded region lengths (uniform across cores) ----
    len1 = np.zeros((CORES, CORES), np.int64)
    len0 = np.zeros(CORES, np.int64)
    per_edge = []
    for c in range(CORES):
        m = e_oc == c
        srcs, dsts, owns = src_g[m], dst_g[m], e_os[m]
        dstl = dsts - node_start[c]
        coefs = dinv[srcs] * dinv[dsts]
        per_edge.append((srcs, dstl, owns, coefs))
        for o in range(CORES):
            mo = owns == o
            _, _, _, _, _, plen = _region_layout(dstl[mo])
            len1[c, o] = plen
        _, _, _, _, _, plen = _region_layout(dstl)
        len0[c] = plen
    K1 = [_pad128(len1[:, o].max()) for o in range(CORES)]
    off1 = np.zeros(CORES + 1, np.int64)
    off1[1:] = np.cumsum(K1)
    S1 = int(off1[-1])
    S0 = _pad128(len0.max())
    NB0, NB1 = S0 // P, S1 // P

    per_core = []
    for c in range(CORES):
        srcs, dstl, owns, coefs = per_edge[c]

        TILESN = SLAB // P
        AGB = TILESN + 1
        gidx1 = np.zeros(S1, np.int64)
        sidx1 = np.full(S1, -1, np.int64)
        R1 = np.zeros((NB1, P, P), F32)
        for o in range(CORES):
            mo = owns == o
            _fill_region(gidx1, sidx1, R1, int(off1[o]),
                         _pm(srcs[mo] - node_start[o], TILESN),
                         dstl[mo], coefs[mo])

        gidx0 = np.zeros(S0, np.int64)
        sidx0 = np.full(S0, -1, np.int64)
        R0 = np.zeros((NB0, P, P), F32)
        _fill_region(gidx0, sidx0, R0, 0, _pm(node_ids[srcs], 8),
                     dstl, coefs)
        # scatter idx: partition-major agg rows; dummy pads -> row TILESN
        for sx in (sidx1, sidx0):
            real = sx >= 0
            sx[real] = _pm(sx[real], AGB)
            sx[~real] = TILESN

        gidxs = np.zeros(SLAB, np.int64)
        gidxs[:n_c[c]] = _pm(node_ids[node_start[c]:node_start[c + 1]], 8)
        selfn = np.zeros((P, TILES), F32)
        ar = np.arange(int(n_c[c]))
        selfn[ar % P, ar // P] = selfnorm[node_start[c]:node_start[c + 1]]

        # R stored partition-major for contiguous chunk loads:
        # Rpm[p, blk*128 + j] = R[blk, p, j]
        Rall = np.concatenate([R0, R1], 0)           # [NB0+NB1, 128, 128]
        Rpm = np.ascontiguousarray(
            Rall.transpose(1, 0, 2).reshape(P, -1).astype(BF))

        per_core.append(dict(
            gidx1=_idx16_layout(gidx1.astype(np.int16)),
            sidx1=_idx16_layout(sidx1.astype(np.int16)),
            gidx0=_idx16_layout(gidx0.astype(np.int16)),
            sidx0=_idx16_layout(sidx0.astype(np.int16)),
            gidxs=_idx16_layout(gidxs.astype(np.int16)),
            rmat=Rpm, selfn=selfn,
        ))

    # graph cover ranges for pooling (uniform across cores)
    glo_all = np.stack([goff[c * GPC:(c + 1) * GPC] - node_start[c]
                        for c in range(CORES)])   # [CORES, GPC]
    ghi_all = np.stack([goff[c * GPC + 1:(c + 1) * GPC + 1] - node_start[c]
                        for c in range(CORES)])
    lo_fix = glo_all.min(0)
    hi_fix = ghi_all.max(0)
    MAXCOV = int((hi_fix - lo_fix).max())
    MAXCOV = (MAXCOV + 31) // 32 * 32
    cov_len = np.minimum(MAXCOV, SLAB - lo_fix)

    for c in range(CORES):
        mask = np.zeros((GPC, MAXCOV), F32)
        cinv = np.zeros(GPC, F32)
        for j in range(GPC):
            lo = int(glo_all[c, j]) - int(lo_fix[j])
            hi = int(ghi_all[c, j]) - int(lo_fix[j])
            mask[j, lo:hi] = 1.0
            cnt_ = int(ghi_all[c, j] - glo_all[c, j])
            cinv[j] = 1.0 / max(cnt_, 1)
        maskb = np.broadcast_to(mask[:, None, :], (GPC, P, MAXCOV))
        per_core[c]["mask"] = np.ascontiguousarray(maskb.astype(BF))
        per_core[c]["cinv"] = np.broadcast_to(cinv, (P, GPC)).copy()

    meta = dict(TILES=TILES, SLAB=SLAB, S0=S0, S1=S1,
                K1=[int(k) for k in K1], off1=[int(o) for o in off1],
                lo_fix=lo_fix, cov_len=cov_len, MAXCOV=MAXCOV)
    return per_core, meta


def _chunks(meta, layer):
    """Static (slot_base, nblocks, owner) chunk list; identical on all
    cores. Chunks never span owner regions."""
    out = []
    if layer == 0:
        nb_tot = meta["S0"] // P
        for b0 in range(0, nb_tot, CH):
            out.append((b0 * P, min(CH, nb_tot - b0), 0))
    else:
        for o in range(CORES):
            base, nb_tot = meta["off1"][o], meta["K1"][o] // P
            for b0 in range(0, nb_tot, CH):
                out.append((base + b0 * P, min(CH, nb_tot - b0), o))
    return out


def _rgroups(chunks, cap=32):
    """Group consecutive chunks into runs of <= cap blocks (for one big
    R-matrix load per group; chunk slots are contiguous within a layer)."""
    groups, cur, acc = [], [], 0
    for ch in chunks:
        if acc + ch[1] > cap and cur:
            groups.append(cur)
            cur, acc = [], 0
        cur.append(ch)
        acc += ch[1]
    if cur:
        groups.append(cur)
    return groups


def _build(meta):
    TILES, SLAB = meta["TILES"], meta["SLAB"]
    S0, S1 = meta["S0"], meta["S1"]
    NB0 = S0 // P
    MAXCOV = meta["MAXCOV"]
    f32, bf16, i16 = mybir.dt.float32, mybir.dt.bfloat16, mybir.dt.int16
    f16 = mybir.dt.float16
    AGB = TILES + 1
    AF = mybir.ActivationFunctionType
    nc = bacc.Bacc("TRN2", target_bir_lowering=False, debug=False,
                   num_devices=CORES)

    gidx1_in = nc.dram_tensor("gidx1", [P, S1 // 16], i16, kind="ExternalInput")
    sidx1_in = nc.dram_tensor("sidx1", [P, S1 // 16], i16, kind="ExternalInput")
    gidx0_in = nc.dram_tensor("gidx0", [P, S0 // 16], i16, kind="ExternalInput")
    sidx0_in = nc.dram_tensor("sidx0", [P, S0 // 16], i16, kind="ExternalInput")
    gidxs_in = nc.dram_tensor("gidxs", [P, SLAB // 16], i16,
                              kind="ExternalInput")
    rmat_in = nc.dram_tensor("rmat", [P, (S0 + S1)], bf16,
                             kind="ExternalInput")
    selfn_in = nc.dram_tensor("selfn", [P, TILES], f32, kind="ExternalInput")
    mask_in = nc.dram_tensor("mask", [GPC, P, MAXCOV], bf16,
                             kind="ExternalInput")
    cinv_in = nc.dram_tensor("cinv", [P, GPC], f32, kind="ExternalInput")
    embT_in = nc.dram_tensor("embT", [P, 1024], bf16, kind="ExternalInput")
    w0_in = nc.dram_tensor("w0", [P, H], bf16, kind="ExternalInput")
    wl_in = nc.dram_tensor("wl", [P, 3, 2, H], bf16, kind="ExternalInput")
    b_in = nc.dram_tensor("bias", [P, 4, H], f16, kind="ExternalInput")
    cw0_in = nc.dram_tensor("cw0", [P, 4, H], bf16, kind="ExternalInput")
    cb0_in = nc.dram_tensor("cb0", [GPC, H], f32, kind="ExternalInput")
    cw1_in = nc.dram_tensor("cw1", [P, 2, 2], bf16, kind="ExternalInput")
    cb1_in = nc.dram_tensor("cb1", [GPC, 2], f32, kind="ExternalInput")
    out = nc.dram_tensor("out", [GPC, 2], f32, kind="ExternalOutput")

    T0_d = nc.dram_tensor("T0d", [P, 8 * H], bf16)
    MB = nc.dram_tensor("MBd", [P, TILES * H], bf16)
    MFs = [nc.dram_tensor(f"MF{i}", [CORES * P, TILES * H], bf16,
                          addr_space="Shared") for i in range(2)]
    AGG = nc.dram_tensor("AGGd", [P, AGB * H], f16)

    MB_v = None  # set below once tensors exist

    with tile.TileContext(nc) as tc:
        with (
            tc.tile_pool(name="const", bufs=1) as cpool,
            tc.tile_pool(name="gat", bufs=2) as gpool,
            tc.tile_pool(name="rp", bufs=2) as rpool,
            tc.tile_pool(name="sca", bufs=2) as spool,
            tc.tile_pool(name="work", bufs=5) as work,
            tc.tile_pool(name="mst", bufs=2) as mst,
            tc.tile_pool(name="ist", bufs=2) as ist,
            tc.tile_pool(name="rst", bufs=2) as rst,
            tc.tile_pool(name="hmp", bufs=2) as hmp,
            tc.tile_pool(name="pstr", bufs=2, space="PSUM") as pstr,
            tc.tile_pool(name="psmm", bufs=2, space="PSUM") as psmm,
            tc.tile_pool(name="psrd", bufs=2, space="PSUM") as psrd,
        ):
            gidx1_t = cpool.tile([P, S1 // 16], i16)
            nc.sync.dma_start(out=gidx1_t[:], in_=gidx1_in[:, :])
            sidx1_t = cpool.tile([P, S1 // 16], i16)
            nc.sync.dma_start(out=sidx1_t[:], in_=sidx1_in[:, :])
            gidx0_t = cpool.tile([P, S0 // 16], i16)
            nc.sync.dma_start(out=gidx0_t[:], in_=gidx0_in[:, :])
            sidx0_t = cpool.tile([P, S0 // 16], i16)
            nc.sync.dma_start(out=sidx0_t[:], in_=sidx0_in[:, :])
            gidxs_t = cpool.tile([P, SLAB // 16], i16)
            nc.sync.dma_start(out=gidxs_t[:], in_=gidxs_in[:, :])
            selfn_t = cpool.tile([P, TILES], f32)
            nc.sync.dma_start(out=selfn_t[:], in_=selfn_in[:, :])
            cinv_t = cpool.tile([P, GPC], f32)
            nc.sync.dma_start(out=cinv_t[:], in_=cinv_in[:, :])
            embT_t = cpool.tile([P, 1024], bf16)
            nc.sync.dma_start(out=embT_t[:], in_=embT_in[:, :])
            w0_t = cpool.tile([P, H], bf16)
            nc.sync.dma_start(out=w0_t[:], in_=w0_in[:, :])
            wl_t = cpool.tile([P, 3, 2, H], bf16)
            nc.sync.dma_start(out=wl_t[:], in_=wl_in[:, :, :, :])
            bias_t = cpool.tile([P, 4, H], f16)
            nc.sync.dma_start(out=bias_t[:], in_=b_in[:, :, :])

            h_T = nc.alloc_sbuf_tensor("hT", [P, 2, SLAB], bf16)
            ident = cpool.tile([P, P], bf16)
            make_identity(nc, ident[:])

            AGG_v = AGG[:, :].rearrange("p (b e) -> p b e", e=H)
            AGG_sc = AGG[:, :].rearrange("p (b e) -> (p b) e", e=H)
            MB_v = MB[:, :].rearrange("p (b e) -> p b e", e=H)
            T0_v = T0_d[:, :].rearrange("p (b e) -> p b e", e=H)
            T0_tab = T0_d[:, :].rearrange("p (b e) -> (p b) e", e=H)

            # ---- T0 = emb @ w0 ----
            t0st = mst.tile([P, CH, H], bf16, tag="m")
            for vb in range(8):
                pt0 = psmm.tile([P, H], f32, tag="mm")
                nc.tensor.matmul(out=pt0[:], lhsT=embT_t[:, vb * P:(vb + 1) * P],
                                 rhs=w0_t[:], start=True, stop=True)
                nc.scalar.copy(out=t0st[:, vb, :], in_=pt0[:])
            nc.sync.dma_start(out=T0_v[:, 0:8, :], in_=t0st[:])

            def edge_phase(layer, mf):
                gidx_t = gidx0_t if layer == 0 else gidx1_t
                sidx_t = sidx0_t if layer == 0 else sidx1_t
                rbase = 0 if layer == 0 else S0
                for grp in _rgroups(_chunks(meta, layer)):
                    g0 = grp[0][0]            # first slot of group
                    gnb = sum(ch[1] for ch in grp)
                    rt = rpool.tile([P, 32 * P], bf16)
                    nc.sync.dma_start(
                        out=rt[:, 0:gnb * P],
                        in_=rmat_in[:, rbase + g0:rbase + g0 + gnb * P])
                    for (base, nb, o) in grp:
                        gt = gpool.tile([P, CH, H], bf16)
                        tab = (T0_tab if layer == 0
                               else mf[o * P:(o + 1) * P, :].rearrange(
                                   "p (b e) -> (p b) e", e=H))
                        nc.gpsimd.dma_gather(
                            out_ap=gt[:, 0:nb, :], in_ap=tab,
                            idxs_ap=gidx_t[:, base // 16:(base + nb * P) // 16],
                            num_idxs=nb * P, num_idxs_reg=nb * P,
                            elem_size=H, queue_num=0)
                        sf = spool.tile([P, CH, H], f16)
                        for b in range(nb):
                            rb = (base - g0) // P + b
                            pr = psrd.tile([P, H], f32, tag="rd")
                            nc.tensor.matmul(out=pr[:],
                                             lhsT=rt[:, rb * P:(rb + 1) * P],
                                             rhs=gt[:, b, :],
                                             start=True, stop=True)
                            nc.vector.tensor_copy(out=sf[:, b, :], in_=pr[:])
                        nc.gpsimd.dma_scatter_add(
                            AGG_sc, sf[:, 0:nb, :],
                            sidx_t[:, base // 16:(base + nb * P) // 16],
                            nb * P, nb * P, H, queue_num=0)

            def readback_phase(layer):
                for t0 in range(0, TILES, CH):
                    nb = min(CH, TILES - t0)
                    at = rst.tile([P, CH, H], f16)
                    nc.sync.dma_start(out=at[:, 0:nb, :],
                                      in_=AGG_v[:, t0:t0 + nb, :])
                    for b in range(nb):
                        t = t0 + b
                        hb = work.tile([P, H], bf16)
                        nc.scalar.activation(out=hb[:], in_=at[:, b, :],
                                             func=AF.Relu)
                        for fh in range(2):
                            ptr_ = pstr.tile([P, P], bf16)
                            nc.tensor.transpose(
                                out=ptr_[:], in_=hb[:, fh * P:(fh + 1) * P],
                                identity=ident[:])
                            nc.vector.tensor_copy(
                                out=h_T[:, fh, t * P:(t + 1) * P], in_=ptr_[:])

            # ---- layer 0: agg init from T0 self rows (+bias), edge scatter --
            nbs = SLAB // P
            for b0 in range(0, nbs, CH):
                nb = min(CH, nbs - b0)
                gt = gpool.tile([P, CH, H], bf16)
                nc.gpsimd.dma_gather(
                    out_ap=gt[:, 0:nb, :], in_ap=T0_tab,
                    idxs_ap=gidxs_t[:, b0 * 8:(b0 + nb) * 8],
                    num_idxs=nb * P, num_idxs_reg=nb * P,
                    elem_size=H, queue_num=0)
                sf = ist.tile([P, CH, H], f16, tag="i")
                for b in range(nb):
                    t = b0 + b
                    nc.scalar.activation(out=sf[:, b, :], in_=gt[:, b, :],
                                         func=AF.Copy,
                                         scale=selfn_t[:, t:t + 1])
                    nc.vector.tensor_add(out=sf[:, b, :], in0=sf[:, b, :],
                                         in1=bias_t[:, 0, :])
                nc.sync.dma_start(out=AGG_v[:, b0:b0 + nb, :],
                                  in_=sf[:, 0:nb, :])
            edge_phase(0, None)
            readback_phase(0)

            for layer in (1, 2, 3):
                mf = MFs[layer % 2]
                # M = h @ W -> MB; agg init = selfnorm*M + bias
                for t0 in range(0, TILES, CH):
                    nb = min(CH, TILES - t0)
                    mtile = mst.tile([P, CH, H], bf16, tag="m")
                    itile = ist.tile([P, CH, H], f16, tag="i")
                    for b in range(nb):
                        t = t0 + b
                        pm = psmm.tile([P, H], f32, tag="mm")
                        for fh in range(2):
                            nc.tensor.matmul(
                                out=pm[:],
                                lhsT=h_T[:, fh, t * P:(t + 1) * P],
                                rhs=wl_t[:, layer - 1, fh, :],
                                start=(fh == 0), stop=(fh == 1))
                        nc.scalar.copy(out=mtile[:, b, :], in_=pm[:])
                        nc.scalar.activation(out=itile[:, b, :], in_=pm[:],
                                             func=AF.Copy,
                                             scale=selfn_t[:, t:t + 1])
                        nc.vector.tensor_add(out=itile[:, b, :],
                                             in0=itile[:, b, :],
                                             in1=bias_t[:, layer, :])
                    nc.sync.dma_start(out=MB_v[:, t0:t0 + nb, :],
                                      in_=mtile[:, 0:nb, :])
                    nc.sync.dma_start(out=AGG_v[:, t0:t0 + nb, :],
                                      in_=itile[:, 0:nb, :])
                nc.gpsimd.collective_compute(
                    "AllGather", mybir.AluOpType.bypass,
                    replica_groups=[list(range(CORES))],
                    ins=[MB[:].opt()], outs=[mf[:].opt()])
                edge_phase(layer, mf)
                readback_phase(layer)

            # ---- pooling (masked mean/max on h_T) ----
            pooled = []
            for fh in range(2):
                mean_t = cpool.tile([P, GPC], f32, tag=f"mean{fh}")
                max_t = cpool.tile([P, GPC], f32, tag=f"max{fh}")
                nc.vector.memset(mean_t[:], 0.0)
                nc.vector.memset(max_t[:], 0.0)
                pooled.append((mean_t, max_t))
            lo_fix, cov_len = meta["lo_fix"], meta["cov_len"]
            for j in range(GPC):
                mk = hmp.tile([P, MAXCOV], bf16, tag="mask")
                ln = int(cov_len[j])
                nc.sync.dma_start(out=mk[:, 0:ln], in_=mask_in[j, :, 0:ln])
                for fh in range(2):
                    hm = hmp.tile([P, MAXCOV], bf16, tag="hm")
                    lo = int(lo_fix[j])
                    nc.vector.tensor_mul(out=hm[:, 0:ln],
                                         in0=h_T[:, fh, lo:lo + ln],
                                         in1=mk[:, 0:ln])
                    nc.vector.tensor_reduce(
                        out=pooled[fh][0][:, j:j + 1], in_=hm[:, 0:ln],
                        axis=mybir.AxisListType.X, op=mybir.AluOpType.add)
                    nc.vector.tensor_reduce(
                        out=pooled[fh][1][:, j:j + 1], in_=hm[:, 0:ln],
                        axis=mybir.AxisListType.X, op=mybir.AluOpType.max)
            # scale means by 1/cnt, cast to bf16 lhsT chunks
            chunks = []
            for fh in range(2):
                mean_t, max_t = pooled[fh]
                nc.vector.tensor_mul(out=mean_t[:], in0=mean_t[:],
                                     in1=cinv_t[:])
            for (kind, fh) in ((0, 0), (0, 1), (1, 0), (1, 1)):
                src = pooled[fh][kind]
                cb = work.tile([P, GPC], bf16, tag=f"ch{kind}{fh}")
                nc.vector.tensor_copy(out=cb[:], in_=src[:])
                chunks.append(cb)

            # ---- classifier MLP ----
            cw0_t = cpool.tile([P, 4, H], bf16)
            nc.sync.dma_start(out=cw0_t[:], in_=cw0_in[:, :, :])
            cb0_t = cpool.tile([GPC, H], f32)
            nc.sync.dma_start(out=cb0_t[:], in_=cb0_in[:, :])
            cw1_t = cpool.tile([P, 2, 2], bf16)
            nc.sync.dma_start(out=cw1_t[:], in_=cw1_in[:, :, :])
            cb1_t = cpool.tile([GPC, 2], f32)
            nc.sync.dma_start(out=cb1_t[:], in_=cb1_in[:, :])

            ph_full = psmm.tile([P, H], f32, tag="mm")
            ph = ph_full[0:GPC, :]
            for k in range(4):
                nc.tensor.matmul(out=ph[:], lhsT=chunks[k][:],
                                 rhs=cw0_t[:, k, :],
                                 start=(k == 0), stop=(k == 3))
            hc1 = work.tile([GPC, H], f32, tag="hc1")
            nc.vector.tensor_add(out=hc1[:], in0=ph[:], in1=cb0_t[:])
            hcb = work.tile([GPC, H], bf16, tag="hcb")
            nc.scalar.activation(out=hcb[:], in_=hc1[:], func=AF.Relu)
            hTt = []
            for k in range(2):
                ptr_ = pstr.tile([P, P], bf16)
                nc.tensor.transpose(out=ptr_[0:P, 0:GPC],
                                    in_=hcb[:, k * P:(k + 1) * P],
                                    identity=ident[0:GPC, 0:GPC])
                ht = work.tile([P, GPC], bf16, tag=f"hTt{k}")
                nc.vector.tensor_copy(out=ht[:], in_=ptr_[0:P, 0:GPC])
                hTt.append(ht)
            pl_full = psmm.tile([P, H], f32, tag="mm")
            pl = pl_full[0:GPC, 0:2]
            for k in range(2):
                nc.tensor.matmul(out=pl[:], lhsT=hTt[k][:],
                                 rhs=cw1_t[:, k, :],
                                 start=(k == 0), stop=(k == 1))
            lg = work.tile([GPC, 2], f32, tag="lg")
            nc.vector.tensor_add(out=lg[:], in0=pl[:], in1=cb1_t[:])
            nc.sync.dma_start(out=out[:, :], in_=lg[:])
    nc.finalize()
    return nc


def kernel(node_ids, edge_index, batch, emb, w0, b0, w1, b1, w2, b2, w3, b3,
           cw0, cb0, cw1, cb1):
    per_core, meta = _preprocess(node_ids, edge_index, batch)
    nc = _build(meta)

    embT = np.zeros((P, 1024), F32)
    embT[:, :V] = np.asarray(emb, F32).T
    wlk = np.transpose(np.stack([np.asarray(w, F32).reshape(2, P, H)
                    for w in (w1, w2, w3)]), (2, 0, 1, 3)).copy()
    biases = np.stack([np.broadcast_to(np.asarray(b, F32), (P, H))
                       for b in (b0, b1, b2, b3)], axis=1).astype(np.float16)
    ins = []
    for c in range(CORES):
        pc = per_core[c]
        ins.append(dict(
            gidx1=pc["gidx1"], sidx1=pc["sidx1"],
            gidx0=pc["gidx0"], sidx0=pc["sidx0"],
            gidxs=pc["gidxs"], rmat=pc["rmat"], selfn=pc["selfn"],
            mask=pc["mask"], cinv=pc["cinv"],
            embT=embT.astype(BF), w0=np.asarray(w0, F32).astype(BF),
            wl=wlk.astype(BF), bias=biases,
            cw0=np.transpose(np.asarray(cw0, F32).reshape(4, P, H), (1, 0, 2)).astype(BF),
            cb0=np.broadcast_to(np.asarray(cb0, F32), (GPC, H)).copy(),
            cw1=np.transpose(np.asarray(cw1, F32).reshape(2, P, 2), (1, 0, 2)).astype(BF),
            cb1=np.broadcast_to(np.asarray(cb1, F32), (GPC, 2)).copy(),
        ))
    trace = False
    try:  # register NTFF hook so exec_time_ns is measurable (best effort)
        import sys, types
        import antenv
        if "antenv.axon_hooks" not in sys.modules:
            hooks = types.ModuleType("antenv.axon_hooks")
            hooks._h = None
            hooks.set_axon_ntff_profile_hook = lambda h: setattr(hooks, "_h", h)
            hooks.get_axon_ntff_profile_hook = lambda: hooks._h
            sys.modules["antenv.axon_hooks"] = hooks
            antenv.axon_hooks = hooks
            from trn_agent_boot.trn_boot import _ntff_profile_via_ctypes
            hk = _ntff_profile_via_ctypes("/opt/axon/libaxon_pjrt.so")
            if hk is not None:
                hooks.set_axon_ntff_profile_hook(hk)
                trace = True
        else:
            trace = True
    except Exception:
        trace = False
    res = run_bass_kernel_spmd(nc, ins, core_ids=list(range(CORES)),
                               trace=trace)
    logits = np.concatenate([res.results[c]["out"] for c in range(CORES)], 0)
    globals()["last_exec_ns"] = res.exec_time_ns
    return logits.astype(np.float32)


# revision 15
# speedup vs baseline: 1.5991x; 1.4238x over previous
"""GCN (EnhancedTaintFlowGNN) on 8 Trainium2 NeuronCores.

Sharding: 32 graphs/core (batch sorted -> contiguous node ranges). Per GCN
layer: M = h @ W locally (PE), AllGather M into a pair-Shared HBM table,
then pull-side aggregation WITHOUT seg matrices: dma_gather source rows
(per-owner int16 tables), PE per-block run-reduction (R matrices with the
GCN norm coefs folded in, runs never cross block boundaries so output
slots have unique dst per owner region), dma_scatter_add of the reduced
f32 rows into a local agg slab (bias folded into the self-loop init
write; scatter calls are WAW-serialized by tile so cross-call duplicate
dsts cannot race the non-atomic CCE add). Layer 0 gathers rows of
T0 = emb @ w0 (replicated vocab table) so no exchange is needed.
Pooling: masked DVE mean/max on transposed features; classifier MLP on
PE.
"""
import numpy as np
import ml_dtypes

import concourse.bass as bass
import concourse.bacc as bacc
import concourse.tile as tile
from concourse import mybir
from concourse.bass_utils import run_bass_kernel_spmd
from concourse.masks import make_identity

P = 128
N, E, G, V, D, H = 150000, 300000, 256, 1000, 128, 256
CORES = 8
GPC = G // CORES
CH = 8         # 128-slot blocks per gather/scatter call (1024-desc ring cap)
BF = ml_dtypes.bfloat16
F32 = np.float32


def _idx16_layout(idx):
    n = idx.shape[0]
    s = n // 16
    out = np.zeros((P, s), np.int16)
    blk = idx.reshape(s, 16).T.astype(np.int16)
    for g in range(8):
        out[g * 16:(g + 1) * 16, :] = blk
    return out


def _pad128(x):
    return (int(x) + P - 1) // P * P


def _pm(idx, nblk):
    """Partition-major row remap: node d -> (d%128)*nblk + d//128."""
    return (idx % P) * nblk + idx // P


def _region_layout(dst_loc):
    """Positions for dst-sorted slots such that no dst run crosses a
    128 boundary. Returns (order, slot_pos, run_id, run_j, run_dst,
    padded_len). run_j = out-slot index of each run within its block."""
    order = np.argsort(dst_loc, kind="stable")
    ds = dst_loc[order]
    n = len(ds)
    if n == 0:
        return order, np.zeros(0, np.int64), None, None, None, 0
    starts = np.flatnonzero(np.r_[True, ds[1:] != ds[:-1]])
    runlen = np.diff(np.r_[starts, n])
    run_start = np.zeros(len(starts), np.int64)
    pos = 0
    for i, L in enumerate(runlen):
        if pos % P + L > P:
            pos = _pad128(pos)
        run_start[i] = pos
        pos += L
    run_id = np.repeat(np.arange(len(starts)), runlen)
    slot_pos = run_start[run_id] + (np.arange(n) - starts[run_id])
    # out-slot index per run within its block = rank - first_rank_of_block
    rb = run_start // P
    _, first = np.unique(rb, return_index=True)
    first_of_block = np.zeros(rb.max() + 1, np.int64)
    first_of_block[rb[first]] = np.arange(len(starts))[first]
    run_j = np.arange(len(starts)) - first_of_block[rb]
    run_dst = ds[starts]
    return order, slot_pos, run_id, (rb, run_j), run_dst, _pad128(pos)


def _fill_region(gidx, sidx, Rf, base, src_loc, dst_loc, coef):
    """Fill gidx/sidx/R for one owner region starting at slot `base`."""
    order, slot_pos, run_id, rj, run_dst, plen = _region_layout(dst_loc)
    if plen == 0:
        return 0
    rb, run_j = rj
    sp = base + slot_pos
    gidx[sp] = src_loc[order]
    blk = sp // P
    Rf[blk, sp % P, run_j[run_id]] = coef[order]
    base_blk = base // P
    sidx[(base_blk + rb) * P + run_j] = run_dst
    return plen


def _preprocess(node_ids, edge_index, batch):
    node_ids = np.asarray(node_ids).astype(np.int64)
    src_g = np.asarray(edge_index)[0].astype(np.int64)
    dst_g = np.asarray(edge_index)[1].astype(np.int64)
    batch = np.asarray(batch).astype(np.int64)

    deg = 1.0 + np.bincount(dst_g, minlength=N).astype(np.float64)
    dinv = (1.0 / np.sqrt(deg)).astype(F32)
    selfnorm = (dinv * dinv).astype(F32)

    gcnt = np.bincount(batch, minlength=G)
    goff = np.zeros(G + 1, np.int64)
    goff[1:] = np.cumsum(gcnt)
    node_start = np.array([int(goff[c * GPC]) for c in range(CORES + 1)])
    n_c = node_start[1:] - node_start[:-1]
    TILES = int(np.ceil(n_c.max() / P))
    SLAB = TILES * P
    assert SLAB <= 32767, SLAB

    e_oc = np.searchsorted(node_start[1:], dst_g, side="right")
    e_os = np.searchsorted(node_start[1:], src_g, side="right")

    # ---- measure padded region lengths (uniform across cores) ----
    len1 = np.zeros((CORES, CORES), np.int64)
    per_edge = []
    for c in range(CORES):
        m = e_oc == c
        srcs, dsts, owns = src_g[m], dst_g[m], e_os[m]
        dstl = dsts - node_start[c]
        coefs = dinv[srcs] * dinv[dsts]
        per_edge.append((srcs, dstl, owns, coefs))
        for o in range(CORES):
            mo = owns == o
            _, _, _, _, _, plen = _region_layout(dstl[mo])
            len1[c, o] = plen
    K1 = [_pad128(len1[:, o].max()) for o in range(CORES)]
    off1 = np.zeros(CORES + 1, np.int64)
    off1[1:] = np.cumsum(K1)
    S1 = int(off1[-1])
    S0 = 0
    NB1 = S1 // P

    per_core = []
    for c in range(CORES):
        srcs, dstl, owns, coefs = per_edge[c]

        TILESN = SLAB // P
        AGB = TILESN + 1
        gidx1 = np.zeros(S1, np.int64)
        sidx1 = np.full(S1, -1, np.int64)
        R1 = np.zeros((NB1, P, P), F32)
        for o in range(CORES):
            mo = owns == o
            _fill_region(gidx1, sidx1, R1, int(off1[o]),
                         _pm(srcs[mo] - node_start[o], TILESN),
                         dstl[mo], coefs[mo])
        real = sidx1 >= 0
        sidx1[real] = _pm(sidx1[real], AGB)
        sidx1[~real] = TILESN

        selfn = np.zeros((P, TILES), F32)
        ar = np.arange(int(n_c[c]))
        selfn[ar % P, ar // P] = selfnorm[node_start[c]:node_start[c + 1]]

        # Rpm[p, blk*128 + j] = R1[blk, p, j]
        Rpm = np.ascontiguousarray(
            R1.transpose(1, 0, 2).reshape(P, -1).astype(BF))

        # layer-0 dense coefficient matrix: agg0 = C @ [T0; b0]
        # C[d, v] = sum of coefs of edges (dst=d, vid(src)=v) + self term;
        # column V (=1000) is the bias-ones column.
        vids = node_ids[srcs]
        C = np.zeros((SLAB, 9 * P), F32)
        np.add.at(C, (dstl, vids), coefs)
        arn = np.arange(int(n_c[c]))
        C[arn, node_ids[node_start[c]:node_start[c + 1]]] += \
            selfnorm[node_start[c]:node_start[c + 1]]
        C[:int(n_c[c]), 1024] = 1.0
        ct = np.ascontiguousarray(
            C.reshape(TILES, P, 9, P).transpose(3, 0, 2, 1)
            .reshape(P, -1).astype(BF))

        per_core.append(dict(
            gidx1=_idx16_layout(gidx1.astype(np.int16)),
            sidx1=_idx16_layout(sidx1.astype(np.int16)),
            rmat=Rpm, selfn=selfn, ct=ct,
        ))

    # graph cover ranges for pooling (uniform across cores)
    glo_all = np.stack([goff[c * GPC:(c + 1) * GPC] - node_start[c]
                        for c in range(CORES)])   # [CORES, GPC]
    ghi_all = np.stack([goff[c * GPC + 1:(c + 1) * GPC + 1] - node_start[c]
                        for c in range(CORES)])
    lo_fix = glo_all.min(0)
    hi_fix = ghi_all.max(0)
    MAXCOV = int((hi_fix - lo_fix).max())
    MAXCOV = (MAXCOV + 31) // 32 * 32
    cov_len = np.minimum(MAXCOV, SLAB - lo_fix)

    for c in range(CORES):
        mask = np.zeros((GPC, MAXCOV), F32)
        cinv = np.zeros(GPC, F32)
        for j in range(GPC):
            lo = int(glo_all[c, j]) - int(lo_fix[j])
            hi = int(ghi_all[c, j]) - int(lo_fix[j])
            mask[j, lo:hi] = 1.0
            cnt_ = int(ghi_all[c, j] - glo_all[c, j])
            cinv[j] = 1.0 / max(cnt_, 1)
        maskb = np.broadcast_to(mask[:, None, :], (GPC, P, MAXCOV))
        per_core[c]["mask"] = np.ascontiguousarray(maskb.astype(BF))
        per_core[c]["cinv"] = np.broadcast_to(cinv, (P, GPC)).copy()

    meta = dict(TILES=TILES, SLAB=SLAB, S0=S0, S1=S1,
                K1=[int(k) for k in K1], off1=[int(o) for o in off1],
                lo_fix=lo_fix, cov_len=cov_len, MAXCOV=MAXCOV)
    return per_core, meta


def _chunks(meta, layer):
    """Static (slot_base, nblocks, owner) chunk list; identical on all
    cores. Chunks never span owner regions."""
    out = []
    for o in range(CORES):
        base, nb_tot = meta["off1"][o], meta["K1"][o] // P
        for b0 in range(0, nb_tot, CH):
            out.append((base + b0 * P, min(CH, nb_tot - b0), o))
    return out


def _rgroups(chunks, cap=32):
    """Group consecutive chunks into runs of <= cap blocks (for one big
    R-matrix load per group; chunk slots are contiguous within a layer)."""
    groups, cur, acc = [], [], 0
    for ch in chunks:
        if acc + ch[1] > cap and cur:
            groups.append(cur)
            cur, acc = [], 0
        cur.append(ch)
        acc += ch[1]
    if cur:
        groups.append(cur)
    return groups


def _build(meta):
    TILES, SLAB = meta["TILES"], meta["SLAB"]
    S0, S1 = meta["S0"], meta["S1"]
    NB0 = S0 // P
    MAXCOV = meta["MAXCOV"]
    f32, bf16, i16 = mybir.dt.float32, mybir.dt.bfloat16, mybir.dt.int16
    f16 = mybir.dt.float16
    AGB = TILES + 1
    AF = mybir.ActivationFunctionType
    nc = bacc.Bacc("TRN2", target_bir_lowering=False, debug=False,
                   num_devices=CORES)

    gidx1_in = nc.dram_tensor("gidx1", [P, S1 // 16], i16, kind="ExternalInput")
    sidx1_in = nc.dram_tensor("sidx1", [P, S1 // 16], i16, kind="ExternalInput")
    ct_in = nc.dram_tensor("ct", [P, TILES * 9 * P], bf16,
                           kind="ExternalInput")
    rmat_in = nc.dram_tensor("rmat", [P, S1], bf16,
                             kind="ExternalInput")
    selfn_in = nc.dram_tensor("selfn", [P, TILES], f32, kind="ExternalInput")
    mask_in = nc.dram_tensor("mask", [GPC, P, MAXCOV], bf16,
                             kind="ExternalInput")
    cinv_in = nc.dram_tensor("cinv", [P, GPC], f32, kind="ExternalInput")
    embT_in = nc.dram_tensor("embT", [P, 1024], bf16, kind="ExternalInput")
    w0_in = nc.dram_tensor("w0", [P, H], bf16, kind="ExternalInput")
    wl_in = nc.dram_tensor("wl", [P, 3, 2, H], bf16, kind="ExternalInput")
    b_in = nc.dram_tensor("bias", [P, 4, H], f16, kind="ExternalInput")
    cw0_in = nc.dram_tensor("cw0", [P, 4, H], bf16, kind="ExternalInput")
    cb0_in = nc.dram_tensor("cb0", [GPC, H], f32, kind="ExternalInput")
    cw1_in = nc.dram_tensor("cw1", [P, 2, 2], bf16, kind="ExternalInput")
    cb1_in = nc.dram_tensor("cb1", [GPC, 2], f32, kind="ExternalInput")
    out = nc.dram_tensor("out", [GPC, 2], f32, kind="ExternalOutput")

    MB = nc.dram_tensor("MBd", [P, TILES * H], bf16)
    MFs = [nc.dram_tensor(f"MF{i}", [CORES * P, TILES * H], bf16,
                          addr_space="Shared") for i in range(2)]
    AGG = nc.dram_tensor("AGGd", [P, AGB * H], f16)

    MB_v = None  # set below once tensors exist

    with tile.TileContext(nc) as tc:
        with (
            tc.tile_pool(name="const", bufs=1) as cpool,
            tc.tile_pool(name="gat", bufs=2) as gpool,
            tc.tile_pool(name="rp", bufs=2) as rpool,
            tc.tile_pool(name="sca", bufs=2) as spool,
            tc.tile_pool(name="work", bufs=5) as work,
            tc.tile_pool(name="mst", bufs=2) as mst,
            tc.tile_pool(name="ist", bufs=2) as ist,
            tc.tile_pool(name="rst", bufs=2) as rst,
            tc.tile_pool(name="hmp", bufs=2) as hmp,
            tc.tile_pool(name="pstr", bufs=2, space="PSUM") as pstr,
            tc.tile_pool(name="psmm", bufs=2, space="PSUM") as psmm,
            tc.tile_pool(name="psrd", bufs=2, space="PSUM") as psrd,
        ):
            gidx1_t = cpool.tile([P, S1 // 16], i16)
            nc.sync.dma_start(out=gidx1_t[:], in_=gidx1_in[:, :])
            sidx1_t = cpool.tile([P, S1 // 16], i16)
            nc.sync.dma_start(out=sidx1_t[:], in_=sidx1_in[:, :])
            selfn_t = cpool.tile([P, TILES], f32)
            nc.sync.dma_start(out=selfn_t[:], in_=selfn_in[:, :])
            cinv_t = cpool.tile([P, GPC], f32)
            nc.sync.dma_start(out=cinv_t[:], in_=cinv_in[:, :])
            embT_t = cpool.tile([P, 1024], bf16)
            nc.sync.dma_start(out=embT_t[:], in_=embT_in[:, :])
            w0_t = cpool.tile([P, H], bf16)
            nc.sync.dma_start(out=w0_t[:], in_=w0_in[:, :])
            wl_t = cpool.tile([P, 3, 2, H], bf16)
            nc.sync.dma_start(out=wl_t[:], in_=wl_in[:, :, :, :])
            bias_t = cpool.tile([P, 4, H], f16)
            nc.sync.dma_start(out=bias_t[:], in_=b_in[:, :, :])

            h_T = nc.alloc_sbuf_tensor("hT", [P, 2, SLAB], bf16)
            ident = cpool.tile([P, P], bf16)
            make_identity(nc, ident[:])

            AGG_v = AGG[:, :].rearrange("p (b e) -> p b e", e=H)
            AGG_sc = AGG[:, :].rearrange("p (b e) -> (p b) e", e=H)
            MB_v = MB[:, :].rearrange("p (b e) -> p b e", e=H)

            # ---- T0 = emb @ w0 (kept in SBUF); block 8 = bias broadcast --
            t0st = cpool.tile([P, 9, H], bf16, tag="t0")
            for vb in range(8):
                pt0 = psmm.tile([P, H], f32, tag="mm")
                nc.tensor.matmul(out=pt0[:], lhsT=embT_t[:, vb * P:(vb + 1) * P],
                                 rhs=w0_t[:], start=True, stop=True)
                nc.scalar.copy(out=t0st[:, vb, :], in_=pt0[:])
            nc.vector.tensor_copy(out=t0st[:, 8, :], in_=bias_t[:, 0, :])

            def edge_phase(layer, mf):
                gidx_t, sidx_t, rbase = gidx1_t, sidx1_t, 0
                for grp in _rgroups(_chunks(meta, layer)):
                    g0 = grp[0][0]            # first slot of group
                    gnb = sum(ch[1] for ch in grp)
                    rt = rpool.tile([P, 32 * P], bf16)
                    nc.sync.dma_start(
                        out=rt[:, 0:gnb * P],
                        in_=rmat_in[:, rbase + g0:rbase + g0 + gnb * P])
                    for (base, nb, o) in grp:
                        gt = gpool.tile([P, CH, H], bf16)
                        tab = mf[o * P:(o + 1) * P, :].rearrange(
                            "p (b e) -> (p b) e", e=H)
                        nc.gpsimd.dma_gather(
                            out_ap=gt[:, 0:nb, :], in_ap=tab,
                            idxs_ap=gidx_t[:, base // 16:(base + nb * P) // 16],
                            num_idxs=nb * P, num_idxs_reg=nb * P,
                            elem_size=H, queue_num=0)
                        sf = spool.tile([P, CH, H], f16)
                        for b in range(nb):
                            rb = (base - g0) // P + b
                            pr = psrd.tile([P, H], f32, tag="rd")
                            nc.tensor.matmul(out=pr[:],
                                             lhsT=rt[:, rb * P:(rb + 1) * P],
                                             rhs=gt[:, b, :],
                                             start=True, stop=True)
                            nc.vector.tensor_copy(out=sf[:, b, :], in_=pr[:])
                        nc.gpsimd.dma_scatter_add(
                            AGG_sc, sf[:, 0:nb, :],
                            sidx_t[:, base // 16:(base + nb * P) // 16],
                            nb * P, nb * P, H, queue_num=0)

            def readback_phase(layer):
                for t0 in range(0, TILES, CH):
                    nb = min(CH, TILES - t0)
                    at = rst.tile([P, CH, H], f16)
                    nc.sync.dma_start(out=at[:, 0:nb, :],
                                      in_=AGG_v[:, t0:t0 + nb, :])
                    for b in range(nb):
                        t = t0 + b
                        hb = work.tile([P, H], bf16)
                        nc.scalar.activation(out=hb[:], in_=at[:, b, :],
                                             func=AF.Relu)
                        for fh in range(2):
                            ptr_ = pstr.tile([P, P], bf16)
                            nc.tensor.transpose(
                                out=ptr_[:], in_=hb[:, fh * P:(fh + 1) * P],
                                identity=ident[:])
                            nc.vector.tensor_copy(
                                out=h_T[:, fh, t * P:(t + 1) * P], in_=ptr_[:])

            # ---- layer 0: h = relu(C @ [T0; b0]) entirely on PE ----
            CTG = 4
            TW = 9 * P
            for tg in range(0, TILES, CTG):
                ng = min(CTG, TILES - tg)
                ctt = gpool.tile([P, CTG * TW], bf16, tag="ct")
                nc.sync.dma_start(out=ctt[:, 0:ng * TW],
                                  in_=ct_in[:, tg * TW:(tg + ng) * TW])
                for i in range(ng):
                    t = tg + i
                    pm = psmm.tile([P, H], f32, tag="mm")
                    for vb in range(9):
                        c0 = (i * 9 + vb) * P
                        nc.tensor.matmul(out=pm[:],
                                         lhsT=ctt[:, c0:c0 + P],
                                         rhs=t0st[:, vb, :],
                                         start=(vb == 0), stop=(vb == 8))
                    hb = work.tile([P, H], bf16)
                    nc.scalar.activation(out=hb[:], in_=pm[:], func=AF.Relu)
                    for fh in range(2):
                        ptr_ = pstr.tile([P, P], bf16)
                        nc.tensor.transpose(
                            out=ptr_[:], in_=hb[:, fh * P:(fh + 1) * P],
                            identity=ident[:])
                        nc.vector.tensor_copy(
                            out=h_T[:, fh, t * P:(t + 1) * P], in_=ptr_[:])

            for layer in (1, 2, 3):
                mf = MFs[layer % 2]
                # M = h @ W -> MB; agg init = selfnorm*M + bias
                for t0 in range(0, TILES, CH):
                    nb = min(CH, TILES - t0)
                    mtile = mst.tile([P, CH, H], bf16, tag="m")
                    itile = ist.tile([P, CH, H], f16, tag="i")
                    for b in range(nb):
                        t = t0 + b
                        pm = psmm.tile([P, H], f32, tag="mm")
                        for fh in range(2):
                            nc.tensor.matmul(
                                out=pm[:],
                                lhsT=h_T[:, fh, t * P:(t + 1) * P],
                                rhs=wl_t[:, layer - 1, fh, :],
                                start=(fh == 0), stop=(fh == 1))
                        nc.scalar.copy(out=mtile[:, b, :], in_=pm[:])
                        nc.scalar.activation(out=itile[:, b, :], in_=pm[:],
                                             func=AF.Copy,
                                             scale=selfn_t[:, t:t + 1])
                        nc.vector.tensor_add(out=itile[:, b, :],
                                             in0=itile[:, b, :],
                                             in1=bias_t[:, layer, :])
                    nc.sync.dma_start(out=MB_v[:, t0:t0 + nb, :],
                                      in_=mtile[:, 0:nb, :])
                    nc.sync.dma_start(out=AGG_v[:, t0:t0 + nb, :],
                                      in_=itile[:, 0:nb, :])
                nc.gpsimd.collective_compute(
                    "AllGather", mybir.AluOpType.bypass,
                    replica_groups=[list(range(CORES))],
                    ins=[MB[:].opt()], outs=[mf[:].opt()])
                edge_phase(layer, mf)
                readback_phase(layer)

            # ---- pooling (masked mean/max on h_T) ----
            pooled = []
            for fh in range(2):
                mean_t = cpool.tile([P, GPC], f32, tag=f"mean{fh}")
                max_t = cpool.tile([P, GPC], f32, tag=f"max{fh}")
                nc.vector.memset(mean_t[:], 0.0)
                nc.vector.memset(max_t[:], 0.0)
                pooled.append((mean_t, max_t))
            lo_fix, cov_len = meta["lo_fix"], meta["cov_len"]
            for j in range(GPC):
                mk = hmp.tile([P, MAXCOV], bf16, tag="mask")
                ln = int(cov_len[j])
                nc.sync.dma_start(out=mk[:, 0:ln], in_=mask_in[j, :, 0:ln])
                for fh in range(2):
                    hm = hmp.tile([P, MAXCOV], bf16, tag="hm")
                    lo = int(lo_fix[j])
                    nc.vector.tensor_mul(out=hm[:, 0:ln],
                                         in0=h_T[:, fh, lo:lo + ln],
                                         in1=mk[:, 0:ln])
                    nc.vector.tensor_reduce(
                        out=pooled[fh][0][:, j:j + 1], in_=hm[:, 0:ln],
                        axis=mybir.AxisListType.X, op=mybir.AluOpType.add)
                    nc.vector.tensor_reduce(
                        out=pooled[fh][1][:, j:j + 1], in_=hm[:, 0:ln],
                        axis=mybir.AxisListType.X, op=mybir.AluOpType.max)
            # scale means by 1/cnt, cast to bf16 lhsT chunks
            chunks = []
            for fh in range(2):
                mean_t, max_t = pooled[fh]
                nc.vector.tensor_mul(out=mean_t[:], in0=mean_t[:],
                                     in1=cinv_t[:])
            for (kind, fh) in ((0, 0), (0, 1), (1, 0), (1, 1)):
                src = pooled[fh][kind]
                cb = work.tile([P, GPC], bf16, tag=f"ch{kind}{fh}")
                nc.vector.tensor_copy(out=cb[:], in_=src[:])
                chunks.append(cb)

            # ---- classifier MLP ----
            cw0_t = cpool.tile([P, 4, H], bf16)
            nc.sync.dma_start(out=cw0_t[:], in_=cw0_in[:, :, :])
            cb0_t = cpool.tile([GPC, H], f32)
            nc.sync.dma_start(out=cb0_t[:], in_=cb0_in[:, :])
            cw1_t = cpool.tile([P, 2, 2], bf16)
            nc.sync.dma_start(out=cw1_t[:], in_=cw1_in[:, :, :])
            cb1_t = cpool.tile([GPC, 2], f32)
            nc.sync.dma_start(out=cb1_t[:], in_=cb1_in[:, :])

            ph_full = psmm.tile([P, H], f32, tag="mm")
            ph = ph_full[0:GPC, :]
            for k in range(4):
                nc.tensor.matmul(out=ph[:], lhsT=chunks[k][:],
                                 rhs=cw0_t[:, k, :],
                                 start=(k == 0), stop=(k == 3))
            hc1 = work.tile([GPC, H], f32, tag="hc1")
            nc.vector.tensor_add(out=hc1[:], in0=ph[:], in1=cb0_t[:])
            hcb = work.tile([GPC, H], bf16, tag="hcb")
            nc.scalar.activation(out=hcb[:], in_=hc1[:], func=AF.Relu)
            hTt = []
            for k in range(2):
                ptr_ = pstr.tile([P, P], bf16)
                nc.tensor.transpose(out=ptr_[0:P, 0:GPC],
                                    in_=hcb[:, k * P:(k + 1) * P],
                                    identity=ident[0:GPC, 0:GPC])
                ht = work.tile([P, GPC], bf16, tag=f"hTt{k}")
                nc.vector.tensor_copy(out=ht[:], in_=ptr_[0:P, 0:GPC])
                hTt.append(ht)
            pl_full = psmm.tile([P, H], f32, tag="mm")
            pl = pl_full[0:GPC, 0:2]
            for k in range(2):
                nc.tensor.matmul(out=pl[:], lhsT=hTt[k][:],
                                 rhs=cw1_t[:, k, :],
                                 start=(k == 0), stop=(k == 1))
            lg = work.tile([GPC, 2], f32, tag="lg")
            nc.vector.tensor_add(out=lg[:], in0=pl[:], in1=cb1_t[:])
            nc.sync.dma_start(out=out[:, :], in_=lg[:])
    nc.finalize()
    return nc


def kernel(node_ids, edge_index, batch, emb, w0, b0, w1, b1, w2, b2, w3, b3,
           cw0, cb0, cw1, cb1):
    per_core, meta = _preprocess(node_ids, edge_index, batch)
    nc = _build(meta)

    embT = np.zeros((P, 1024), F32)
    embT[:, :V] = np.asarray(emb, F32).T
    wlk = np.transpose(np.stack([np.asarray(w, F32).reshape(2, P, H)
                    for w in (w1, w2, w3)]), (2, 0, 1, 3)).copy()
    biases = np.stack([np.broadcast_to(np.asarray(b, F32), (P, H))
                       for b in (b0, b1, b2, b3)], axis=1).astype(np.float16)
    ins = []
    for c in range(CORES):
        pc = per_core[c]
        ins.append(dict(
            gidx1=pc["gidx1"], sidx1=pc["sidx1"], ct=pc["ct"],
            rmat=pc["rmat"], selfn=pc["selfn"],
            mask=pc["mask"], cinv=pc["cinv"],
            embT=embT.astype(BF), w0=np.asarray(w0, F32).astype(BF),
            wl=wlk.astype(BF), bias=biases,
            cw0=np.transpose(np.asarray(cw0, F32).reshape(4, P, H), (1, 0, 2)).astype(BF),
            cb0=np.broadcast_to(np.asarray(cb0, F32), (GPC, H)).copy(),
            cw1=np.transpose(np.asarray(cw1, F32).reshape(2, P, 2), (1, 0, 2)).astype(BF),
            cb1=np.broadcast_to(np.asarray(cb1, F32), (GPC, 2)).copy(),
        ))
    trace = False
    try:  # register NTFF hook so exec_time_ns is measurable (best effort)
        import sys, types
        import antenv
        if "antenv.axon_hooks" not in sys.modules:
            hooks = types.ModuleType("antenv.axon_hooks")
            hooks._h = None
            hooks.set_axon_ntff_profile_hook = lambda h: setattr(hooks, "_h", h)
            hooks.get_axon_ntff_profile_hook = lambda: hooks._h
            sys.modules["antenv.axon_hooks"] = hooks
            antenv.axon_hooks = hooks
            from trn_agent_boot.trn_boot import _ntff_profile_via_ctypes
            hk = _ntff_profile_via_ctypes("/opt/axon/libaxon_pjrt.so")
            if hk is not None:
                hooks.set_axon_ntff_profile_hook(hk)
                trace = True
        else:
            trace = True
    except Exception:
        trace = False
    res = run_bass_kernel_spmd(nc, ins, core_ids=list(range(CORES)),
                               trace=trace)
    logits = np.concatenate([res.results[c]["out"] for c in range(CORES)], 0)
    globals()["last_exec_ns"] = res.exec_time_ns
    return logits.astype(np.float32)


# revision 17
# speedup vs baseline: 1.7765x; 1.1109x over previous
"""GCN (EnhancedTaintFlowGNN) on 8 Trainium2 NeuronCores.

Sharding: 32 graphs/core (batch sorted -> contiguous node ranges). Per GCN
layer: M = h @ W locally (PE), AllGather M into a pair-Shared HBM table,
then pull-side aggregation WITHOUT seg matrices: dma_gather source rows
(per-owner int16 tables), PE per-block run-reduction (R matrices with the
GCN norm coefs folded in, runs never cross block boundaries so output
slots have unique dst per owner region), dma_scatter_add of the reduced
f32 rows into a local agg slab (bias folded into the self-loop init
write; scatter calls are WAW-serialized by tile so cross-call duplicate
dsts cannot race the non-atomic CCE add). Layer 0 gathers rows of
T0 = emb @ w0 (replicated vocab table) so no exchange is needed.
Pooling: masked DVE mean/max on transposed features; classifier MLP on
PE.
"""
import numpy as np
import ml_dtypes

import concourse.bass as bass
import concourse.bacc as bacc
import concourse.tile as tile
from concourse import mybir
from concourse.bass_utils import run_bass_kernel_spmd
from concourse.masks import make_identity

P = 128
N, E, G, V, D, H = 150000, 300000, 256, 1000, 128, 256
CORES = 8
GPC = G // CORES
CH = 8         # 128-slot blocks per gather/scatter call (1024-desc ring cap)
BF = ml_dtypes.bfloat16
F32 = np.float32


def _idx16_layout(idx):
    n = idx.shape[0]
    s = n // 16
    out = np.zeros((P, s), np.int16)
    blk = idx.reshape(s, 16).T.astype(np.int16)
    for g in range(8):
        out[g * 16:(g + 1) * 16, :] = blk
    return out


def _pad128(x):
    return (int(x) + P - 1) // P * P


def _pm(idx, nblk):
    """Partition-major row remap: node d -> (d%128)*nblk + d//128."""
    return (idx % P) * nblk + idx // P


def _region_layout(dst_loc):
    """Positions for dst-sorted slots such that no dst run crosses a
    128 boundary. Returns (order, slot_pos, run_id, run_j, run_dst,
    padded_len). run_j = out-slot index of each run within its block."""
    order = np.argsort(dst_loc, kind="stable")
    ds = dst_loc[order]
    n = len(ds)
    if n == 0:
        return order, np.zeros(0, np.int64), None, None, None, 0
    starts = np.flatnonzero(np.r_[True, ds[1:] != ds[:-1]])
    runlen = np.diff(np.r_[starts, n])
    run_start = np.zeros(len(starts), np.int64)
    pos = 0
    for i, L in enumerate(runlen):
        if pos % P + L > P:
            pos = _pad128(pos)
        run_start[i] = pos
        pos += L
    run_id = np.repeat(np.arange(len(starts)), runlen)
    slot_pos = run_start[run_id] + (np.arange(n) - starts[run_id])
    # out-slot index per run within its block = rank - first_rank_of_block
    rb = run_start // P
    _, first = np.unique(rb, return_index=True)
    first_of_block = np.zeros(rb.max() + 1, np.int64)
    first_of_block[rb[first]] = np.arange(len(starts))[first]
    run_j = np.arange(len(starts)) - first_of_block[rb]
    run_dst = ds[starts]
    return order, slot_pos, run_id, (rb, run_j), run_dst, _pad128(pos)


def _fill_region(gidx, sidx, Rf, base, src_loc, dst_loc, coef):
    """Fill gidx/sidx/R for one owner region starting at slot `base`."""
    order, slot_pos, run_id, rj, run_dst, plen = _region_layout(dst_loc)
    if plen == 0:
        return 0
    rb, run_j = rj
    sp = base + slot_pos
    gidx[sp] = src_loc[order]
    blk = sp // P
    Rf[blk, sp % P, run_j[run_id]] = coef[order]
    base_blk = base // P
    sidx[(base_blk + rb) * P + run_j] = run_dst
    return plen


def _preprocess(node_ids, edge_index, batch):
    node_ids = np.asarray(node_ids).astype(np.int64)
    src_g = np.asarray(edge_index)[0].astype(np.int64)
    dst_g = np.asarray(edge_index)[1].astype(np.int64)
    batch = np.asarray(batch).astype(np.int64)

    deg = 1.0 + np.bincount(dst_g, minlength=N).astype(np.float64)
    dinv = (1.0 / np.sqrt(deg)).astype(F32)
    selfnorm = (dinv * dinv).astype(F32)

    gcnt = np.bincount(batch, minlength=G)
    goff = np.zeros(G + 1, np.int64)
    goff[1:] = np.cumsum(gcnt)
    node_start = np.array([int(goff[c * GPC]) for c in range(CORES + 1)])
    n_c = node_start[1:] - node_start[:-1]
    TILES = int(np.ceil(n_c.max() / P))
    SLAB = TILES * P
    assert SLAB <= 32767, SLAB

    e_oc = np.searchsorted(node_start[1:], dst_g, side="right")
    e_os = np.searchsorted(node_start[1:], src_g, side="right")

    # ---- measure padded region lengths (uniform across cores) ----
    len1 = np.zeros((CORES, CORES), np.int64)
    per_edge = []
    for c in range(CORES):
        m = e_oc == c
        srcs, dsts, owns = src_g[m], dst_g[m], e_os[m]
        dstl = dsts - node_start[c]
        coefs = dinv[srcs] * dinv[dsts]
        per_edge.append((srcs, dstl, owns, coefs))
        for o in range(CORES):
            mo = owns == o
            _, _, _, _, _, plen = _region_layout(dstl[mo])
            len1[c, o] = plen
    K1 = [_pad128(len1[:, o].max()) for o in range(CORES)]
    off1 = np.zeros(CORES + 1, np.int64)
    off1[1:] = np.cumsum(K1)
    S1 = int(off1[-1])
    S0 = 0
    NB1 = S1 // P

    per_core = []
    for c in range(CORES):
        srcs, dstl, owns, coefs = per_edge[c]

        TILESN = SLAB // P
        AGB = TILESN + 1
        gidx1 = np.zeros(S1, np.int64)
        sidx1 = np.full(S1, -1, np.int64)
        R1 = np.zeros((NB1, P, P), F32)
        for o in range(CORES):
            mo = owns == o
            _fill_region(gidx1, sidx1, R1, int(off1[o]),
                         _pm(srcs[mo] - node_start[o], TILESN),
                         dstl[mo], coefs[mo])
        real = sidx1 >= 0
        sidx1[real] = _pm(sidx1[real], AGB)
        sidx1[~real] = TILESN

        selfn = np.zeros((P, TILES), F32)
        ar = np.arange(int(n_c[c]))
        selfn[ar % P, ar // P] = selfnorm[node_start[c]:node_start[c + 1]]

        # Rpm[p, blk*128 + j] = R1[blk, p, j]
        Rpm = np.ascontiguousarray(
            R1.transpose(1, 0, 2).reshape(P, -1).astype(BF))

        # layer-0 dense coefficient matrix: agg0 = C @ [T0; b0]
        # C[d, v] = sum of coefs of edges (dst=d, vid(src)=v) + self term;
        # column V (=1000) is the bias-ones column.
        vids = node_ids[srcs]
        C = np.zeros((SLAB, 9 * P), F32)
        np.add.at(C, (dstl, vids), coefs)
        arn = np.arange(int(n_c[c]))
        C[arn, node_ids[node_start[c]:node_start[c + 1]]] += \
            selfnorm[node_start[c]:node_start[c + 1]]
        C[:int(n_c[c]), 1024] = 1.0
        ct = np.ascontiguousarray(
            C.reshape(TILES, P, 9, P).transpose(3, 0, 2, 1)
            .reshape(P, -1).astype(BF))

        per_core.append(dict(
            gidx1=_idx16_layout(gidx1.astype(np.int16)),
            sidx1=_idx16_layout(sidx1.astype(np.int16)),
            rmat=Rpm, selfn=selfn, ct=ct,
        ))

    # graph cover ranges for pooling (uniform across cores)
    glo_all = np.stack([goff[c * GPC:(c + 1) * GPC] - node_start[c]
                        for c in range(CORES)])   # [CORES, GPC]
    ghi_all = np.stack([goff[c * GPC + 1:(c + 1) * GPC + 1] - node_start[c]
                        for c in range(CORES)])
    lo_fix = glo_all.min(0)
    hi_fix = ghi_all.max(0)
    MAXCOV = int((hi_fix - lo_fix).max())
    MAXCOV = (MAXCOV + 31) // 32 * 32
    cov_len = np.minimum(MAXCOV, SLAB - lo_fix)

    for c in range(CORES):
        mask = np.zeros((GPC, MAXCOV), F32)
        cinv = np.zeros(GPC, F32)
        for j in range(GPC):
            lo = int(glo_all[c, j]) - int(lo_fix[j])
            hi = int(ghi_all[c, j]) - int(lo_fix[j])
            mask[j, lo:hi] = 1.0
            cnt_ = int(ghi_all[c, j] - glo_all[c, j])
            cinv[j] = 1.0 / max(cnt_, 1)
        maskb = np.broadcast_to(mask[:, None, :], (GPC, P, MAXCOV))
        per_core[c]["mask"] = np.ascontiguousarray(maskb.astype(BF))
        per_core[c]["cinv"] = np.broadcast_to(cinv, (P, GPC)).copy()

    meta = dict(TILES=TILES, SLAB=SLAB, S0=S0, S1=S1,
                K1=[int(k) for k in K1], off1=[int(o) for o in off1],
                lo_fix=lo_fix, cov_len=cov_len, MAXCOV=MAXCOV)
    return per_core, meta


def _chunks(meta, layer):
    """Static (slot_base, nblocks, owner) chunk list; identical on all
    cores. Chunks never span owner regions."""
    out = []
    for o in range(CORES):
        base, nb_tot = meta["off1"][o], meta["K1"][o] // P
        for b0 in range(0, nb_tot, CH):
            out.append((base + b0 * P, min(CH, nb_tot - b0), o))
    return out


def _rgroups(chunks, cap=16):
    """Group consecutive chunks into runs of <= cap blocks (for one big
    R-matrix load per group; chunk slots are contiguous within a layer)."""
    groups, cur, acc = [], [], 0
    for ch in chunks:
        if acc + ch[1] > cap and cur:
            groups.append(cur)
            cur, acc = [], 0
        cur.append(ch)
        acc += ch[1]
    if cur:
        groups.append(cur)
    return groups


def _build(meta):
    TILES, SLAB = meta["TILES"], meta["SLAB"]
    S0, S1 = meta["S0"], meta["S1"]
    NB0 = S0 // P
    MAXCOV = meta["MAXCOV"]
    f32, bf16, i16 = mybir.dt.float32, mybir.dt.bfloat16, mybir.dt.int16
    f16 = mybir.dt.float16
    AGB = TILES + 1
    AF = mybir.ActivationFunctionType
    nc = bacc.Bacc("TRN2", target_bir_lowering=False, debug=False,
                   num_devices=CORES, dynamic_dma_scratch_size=32768,
                   num_swdge_queues=2)

    gidx1_in = nc.dram_tensor("gidx1", [P, S1 // 16], i16, kind="ExternalInput")
    sidx1_in = nc.dram_tensor("sidx1", [P, S1 // 16], i16, kind="ExternalInput")
    ct_in = nc.dram_tensor("ct", [P, TILES * 9 * P], bf16,
                           kind="ExternalInput")
    rmat_in = nc.dram_tensor("rmat", [P, S1], bf16,
                             kind="ExternalInput")
    selfn_in = nc.dram_tensor("selfn", [P, TILES], f32, kind="ExternalInput")
    mask_in = nc.dram_tensor("mask", [GPC, P, MAXCOV], bf16,
                             kind="ExternalInput")
    cinv_in = nc.dram_tensor("cinv", [P, GPC], f32, kind="ExternalInput")
    embT_in = nc.dram_tensor("embT", [P, 1024], bf16, kind="ExternalInput")
    w0_in = nc.dram_tensor("w0", [P, H], bf16, kind="ExternalInput")
    wl_in = nc.dram_tensor("wl", [P, 3, 2, H], bf16, kind="ExternalInput")
    b_in = nc.dram_tensor("bias", [P, 4, H], f16, kind="ExternalInput")
    cw0_in = nc.dram_tensor("cw0", [P, 4, H], bf16, kind="ExternalInput")
    cb0_in = nc.dram_tensor("cb0", [GPC, H], f32, kind="ExternalInput")
    cw1_in = nc.dram_tensor("cw1", [P, 2, 2], bf16, kind="ExternalInput")
    cb1_in = nc.dram_tensor("cb1", [GPC, 2], f32, kind="ExternalInput")
    out = nc.dram_tensor("out", [GPC, 2], f32, kind="ExternalOutput")

    MB = nc.dram_tensor("MBd", [P, TILES * H], bf16)
    MFs = [nc.dram_tensor(f"MF{i}", [CORES * P, TILES * H], bf16,
                          addr_space="Shared") for i in range(2)]
    AGG = nc.dram_tensor("AGGd", [P, AGB * H], f16)
    AGG2 = nc.dram_tensor("AGG2d", [P, AGB * H], f16)

    MB_v = None  # set below once tensors exist

    with tile.TileContext(nc) as tc:
        with (
            tc.tile_pool(name="const", bufs=1) as cpool,
            tc.tile_pool(name="gat", bufs=2) as gpool,
            tc.tile_pool(name="rp", bufs=2) as rpool,
            tc.tile_pool(name="sca", bufs=2) as spool,
            tc.tile_pool(name="work", bufs=5) as work,
            tc.tile_pool(name="mst", bufs=2) as mst,
            tc.tile_pool(name="ist", bufs=2) as ist,
            tc.tile_pool(name="rst", bufs=2) as rst,
            tc.tile_pool(name="hmp", bufs=1) as hmp,
            tc.tile_pool(name="pstr", bufs=2, space="PSUM") as pstr,
            tc.tile_pool(name="psmm", bufs=2, space="PSUM") as psmm,
            tc.tile_pool(name="psrd", bufs=2, space="PSUM") as psrd,
        ):
            gidx1_t = cpool.tile([P, S1 // 16], i16)
            nc.sync.dma_start(out=gidx1_t[:], in_=gidx1_in[:, :])
            sidx1_t = cpool.tile([P, S1 // 16], i16)
            nc.sync.dma_start(out=sidx1_t[:], in_=sidx1_in[:, :])
            selfn_t = cpool.tile([P, TILES], f32)
            nc.sync.dma_start(out=selfn_t[:], in_=selfn_in[:, :])
            cinv_t = cpool.tile([P, GPC], f32)
            nc.sync.dma_start(out=cinv_t[:], in_=cinv_in[:, :])
            embT_t = cpool.tile([P, 1024], bf16)
            nc.sync.dma_start(out=embT_t[:], in_=embT_in[:, :])
            w0_t = cpool.tile([P, H], bf16)
            nc.sync.dma_start(out=w0_t[:], in_=w0_in[:, :])
            wl_t = cpool.tile([P, 3, 2, H], bf16)
            nc.sync.dma_start(out=wl_t[:], in_=wl_in[:, :, :, :])
            bias_t = cpool.tile([P, 4, H], f16)
            nc.sync.dma_start(out=bias_t[:], in_=b_in[:, :, :])

            h_T = nc.alloc_sbuf_tensor("hT", [P, 2, SLAB], bf16)
            ident = cpool.tile([P, P], bf16)
            make_identity(nc, ident[:])

            AGG_v = AGG[:, :].rearrange("p (b e) -> p b e", e=H)
            AGG_sc = AGG[:, :].rearrange("p (b e) -> (p b) e", e=H)
            AGG2_v = AGG2[:, :].rearrange("p (b e) -> p b e", e=H)
            AGG2_sc = AGG2[:, :].rearrange("p (b e) -> (p b) e", e=H)
            MB_v = MB[:, :].rearrange("p (b e) -> p b e", e=H)

            # ---- T0 = emb @ w0 (kept in SBUF); block 8 = bias broadcast --
            t0st = cpool.tile([P, 9, H], bf16, tag="t0")
            for vb in range(8):
                pt0 = psmm.tile([P, H], f32, tag="mm")
                nc.tensor.matmul(out=pt0[:], lhsT=embT_t[:, vb * P:(vb + 1) * P],
                                 rhs=w0_t[:], start=True, stop=True)
                nc.scalar.copy(out=t0st[:, vb, :], in_=pt0[:])
            nc.vector.tensor_copy(out=t0st[:, 8, :], in_=bias_t[:, 0, :])

            def edge_phase(layer, mf):
                gidx_t, sidx_t, rbase = gidx1_t, sidx1_t, 0
                ci = 0
                for grp in _rgroups(_chunks(meta, layer)):
                    g0 = grp[0][0]            # first slot of group
                    gnb = sum(ch[1] for ch in grp)
                    rt = rpool.tile([P, 16 * P], bf16)
                    nc.sync.dma_start(
                        out=rt[:, 0:gnb * P],
                        in_=rmat_in[:, rbase + g0:rbase + g0 + gnb * P])
                    for (base, nb, o) in grp:
                        gt = gpool.tile([P, CH, H], bf16)
                        tab = mf[o * P:(o + 1) * P, :].rearrange(
                            "p (b e) -> (p b) e", e=H)
                        nc.gpsimd.dma_gather(
                            out_ap=gt[:, 0:nb, :], in_ap=tab,
                            idxs_ap=gidx_t[:, base // 16:(base + nb * P) // 16],
                            num_idxs=nb * P, num_idxs_reg=nb * P,
                            elem_size=H, queue_num=0)
                        sf = spool.tile([P, CH, H], f16)
                        for b in range(nb):
                            rb = (base - g0) // P + b
                            pr = psrd.tile([P, H], f32, tag="rd")
                            nc.tensor.matmul(out=pr[:],
                                             lhsT=rt[:, rb * P:(rb + 1) * P],
                                             rhs=gt[:, b, :],
                                             start=True, stop=True)
                            nc.vector.tensor_copy(out=sf[:, b, :], in_=pr[:])
                        nc.gpsimd.dma_scatter_add(
                            AGG_sc if ci % 2 == 0 else AGG2_sc,
                            sf[:, 0:nb, :],
                            sidx_t[:, base // 16:(base + nb * P) // 16],
                            nb * P, nb * P, H, queue_num=1)
                        ci += 1

            def readback_phase(layer):
                for t0 in range(0, TILES, CH):
                    nb = min(CH, TILES - t0)
                    at = rst.tile([P, CH, H], f16, tag="a")
                    nc.sync.dma_start(out=at[:, 0:nb, :],
                                      in_=AGG_v[:, t0:t0 + nb, :])
                    at2 = rst.tile([P, CH, H], f16, tag="b")
                    nc.sync.dma_start(out=at2[:, 0:nb, :],
                                      in_=AGG2_v[:, t0:t0 + nb, :])
                    nc.vector.tensor_add(out=at[:, 0:nb, :],
                                         in0=at[:, 0:nb, :],
                                         in1=at2[:, 0:nb, :])
                    for b in range(nb):
                        t = t0 + b
                        hb = work.tile([P, H], bf16)
                        nc.scalar.activation(out=hb[:], in_=at[:, b, :],
                                             func=AF.Relu)
                        for fh in range(2):
                            ptr_ = pstr.tile([P, P], bf16)
                            nc.tensor.transpose(
                                out=ptr_[:], in_=hb[:, fh * P:(fh + 1) * P],
                                identity=ident[:])
                            nc.vector.tensor_copy(
                                out=h_T[:, fh, t * P:(t + 1) * P], in_=ptr_[:])

            # ---- layer 0: h = relu(C @ [T0; b0]) entirely on PE ----
            CTG = 2
            TW = 9 * P
            for tg in range(0, TILES, CTG):
                ng = min(CTG, TILES - tg)
                ctt = gpool.tile([P, CTG * TW], bf16, tag="ct")
                nc.sync.dma_start(out=ctt[:, 0:ng * TW],
                                  in_=ct_in[:, tg * TW:(tg + ng) * TW])
                for i in range(ng):
                    t = tg + i
                    pm = psmm.tile([P, H], f32, tag="mm")
                    for vb in range(9):
                        c0 = (i * 9 + vb) * P
                        nc.tensor.matmul(out=pm[:],
                                         lhsT=ctt[:, c0:c0 + P],
                                         rhs=t0st[:, vb, :],
                                         start=(vb == 0), stop=(vb == 8))
                    hb = work.tile([P, H], bf16)
                    nc.scalar.activation(out=hb[:], in_=pm[:], func=AF.Relu)
                    for fh in range(2):
                        ptr_ = pstr.tile([P, P], bf16)
                        nc.tensor.transpose(
                            out=ptr_[:], in_=hb[:, fh * P:(fh + 1) * P],
                            identity=ident[:])
                        nc.vector.tensor_copy(
                            out=h_T[:, fh, t * P:(t + 1) * P], in_=ptr_[:])

            zst = cpool.tile([P, CH, H], f16, tag="z")
            nc.vector.memset(zst[:], 0.0)
            for layer in (1, 2, 3):
                mf = MFs[layer % 2]
                for t0 in range(0, TILES, CH):
                    nb = min(CH, TILES - t0)
                    nc.sync.dma_start(out=AGG2_v[:, t0:t0 + nb, :],
                                      in_=zst[:, 0:nb, :])
                # M = h @ W -> MB; agg init = selfnorm*M + bias
                for t0 in range(0, TILES, CH):
                    nb = min(CH, TILES - t0)
                    mtile = mst.tile([P, CH, H], bf16, tag="m")
                    itile = ist.tile([P, CH, H], f16, tag="i")
                    for b in range(nb):
                        t = t0 + b
                        pm = psmm.tile([P, H], f32, tag="mm")
                        for fh in range(2):
                            nc.tensor.matmul(
                                out=pm[:],
                                lhsT=h_T[:, fh, t * P:(t + 1) * P],
                                rhs=wl_t[:, layer - 1, fh, :],
                                start=(fh == 0), stop=(fh == 1))
                        nc.scalar.copy(out=mtile[:, b, :], in_=pm[:])
                        nc.scalar.activation(out=itile[:, b, :], in_=pm[:],
                                             func=AF.Copy,
                                             scale=selfn_t[:, t:t + 1])
                        nc.vector.tensor_add(out=itile[:, b, :],
                                             in0=itile[:, b, :],
                                             in1=bias_t[:, layer, :])
                    nc.sync.dma_start(out=MB_v[:, t0:t0 + nb, :],
                                      in_=mtile[:, 0:nb, :])
                    nc.sync.dma_start(out=AGG_v[:, t0:t0 + nb, :],
                                      in_=itile[:, 0:nb, :])
                nc.gpsimd.collective_compute(
                    "AllGather", mybir.AluOpType.bypass,
                    replica_groups=[list(range(CORES))],
                    ins=[MB[:].opt()], outs=[mf[:].opt()])
                edge_phase(layer, mf)
                readback_phase(layer)

            # ---- pooling (masked mean/max on h_T) ----
            pooled = []
            for fh in range(2):
                mean_t = cpool.tile([P, GPC], f32, tag=f"mean{fh}")
                max_t = cpool.tile([P, GPC], f32, tag=f"max{fh}")
                nc.vector.memset(mean_t[:], 0.0)
                nc.vector.memset(max_t[:], 0.0)
                pooled.append((mean_t, max_t))
            lo_fix, cov_len = meta["lo_fix"], meta["cov_len"]
            for j in range(GPC):
                mk = hmp.tile([P, MAXCOV], bf16, tag="mask")
                ln = int(cov_len[j])
                nc.sync.dma_start(out=mk[:, 0:ln], in_=mask_in[j, :, 0:ln])
                for fh in range(2):
                    hm = hmp.tile([P, MAXCOV], bf16, tag="hm")
                    lo = int(lo_fix[j])
                    nc.vector.tensor_mul(out=hm[:, 0:ln],
                                         in0=h_T[:, fh, lo:lo + ln],
                                         in1=mk[:, 0:ln])
                    nc.vector.tensor_reduce(
                        out=pooled[fh][0][:, j:j + 1], in_=hm[:, 0:ln],
                        axis=mybir.AxisListType.X, op=mybir.AluOpType.add)
                    nc.vector.tensor_reduce(
                        out=pooled[fh][1][:, j:j + 1], in_=hm[:, 0:ln],
                        axis=mybir.AxisListType.X, op=mybir.AluOpType.max)
            # scale means by 1/cnt, cast to bf16 lhsT chunks
            chunks = []
            for fh in range(2):
                mean_t, max_t = pooled[fh]
                nc.vector.tensor_mul(out=mean_t[:], in0=mean_t[:],
                                     in1=cinv_t[:])
            for (kind, fh) in ((0, 0), (0, 1), (1, 0), (1, 1)):
                src = pooled[fh][kind]
                cb = work.tile([P, GPC], bf16, tag=f"ch{kind}{fh}")
                nc.vector.tensor_copy(out=cb[:], in_=src[:])
                chunks.append(cb)

            # ---- classifier MLP ----
            cw0_t = cpool.tile([P, 4, H], bf16)
            nc.sync.dma_start(out=cw0_t[:], in_=cw0_in[:, :, :])
            cb0_t = cpool.tile([GPC, H], f32)
            nc.sync.dma_start(out=cb0_t[:], in_=cb0_in[:, :])
            cw1_t = cpool.tile([P, 2, 2], bf16)
            nc.sync.dma_start(out=cw1_t[:], in_=cw1_in[:, :, :])
            cb1_t = cpool.tile([GPC, 2], f32)
            nc.sync.dma_start(out=cb1_t[:], in_=cb1_in[:, :])

            ph_full = psmm.tile([P, H], f32, tag="mm")
            ph = ph_full[0:GPC, :]
            for k in range(4):
                nc.tensor.matmul(out=ph[:], lhsT=chunks[k][:],
                                 rhs=cw0_t[:, k, :],
                                 start=(k == 0), stop=(k == 3))
            hc1 = work.tile([GPC, H], f32, tag="hc1")
            nc.vector.tensor_add(out=hc1[:], in0=ph[:], in1=cb0_t[:])
            hcb = work.tile([GPC, H], bf16, tag="hcb")
            nc.scalar.activation(out=hcb[:], in_=hc1[:], func=AF.Relu)
            hTt = []
            for k in range(2):
                ptr_ = pstr.tile([P, P], bf16)
                nc.tensor.transpose(out=ptr_[0:P, 0:GPC],
                                    in_=hcb[:, k * P:(k + 1) * P],
                                    identity=ident[0:GPC, 0:GPC])
                ht = work.tile([P, GPC], bf16, tag=f"hTt{k}")
                nc.vector.tensor_copy(out=ht[:], in_=ptr_[0:P, 0:GPC])
                hTt.append(ht)
            pl_full = psmm.tile([P, H], f32, tag="mm")
            pl = pl_full[0:GPC, 0:2]
            for k in range(2):
                nc.tensor.matmul(out=pl[:], lhsT=hTt[k][:],
                                 rhs=cw1_t[:, k, :],
                                 start=(k == 0), stop=(k == 1))
            lg = work.tile([GPC, 2], f32, tag="lg")
            nc.vector.tensor_add(out=lg[:], in0=pl[:], in1=cb1_t[:])
            nc.sync.dma_start(out=out[:, :], in_=lg[:])
    nc.finalize()
    return nc


def kernel(node_ids, edge_index, batch, emb, w0, b0, w1, b1, w2, b2, w3, b3,
           cw0, cb0, cw1, cb1):
    per_core, meta = _preprocess(node_ids, edge_index, batch)
    nc = _build(meta)

    embT = np.zeros((P, 1024), F32)
    embT[:, :V] = np.asarray(emb, F32).T
    wlk = np.transpose(np.stack([np.asarray(w, F32).reshape(2, P, H)
                    for w in (w1, w2, w3)]), (2, 0, 1, 3)).copy()
    biases = np.stack([np.broadcast_to(np.asarray(b, F32), (P, H))
                       for b in (b0, b1, b2, b3)], axis=1).astype(np.float16)
    ins = []
    for c in range(CORES):
        pc = per_core[c]
        ins.append(dict(
            gidx1=pc["gidx1"], sidx1=pc["sidx1"], ct=pc["ct"],
            rmat=pc["rmat"], selfn=pc["selfn"],
            mask=pc["mask"], cinv=pc["cinv"],
            embT=embT.astype(BF), w0=np.asarray(w0, F32).astype(BF),
            wl=wlk.astype(BF), bias=biases,
            cw0=np.transpose(np.asarray(cw0, F32).reshape(4, P, H), (1, 0, 2)).astype(BF),
            cb0=np.broadcast_to(np.asarray(cb0, F32), (GPC, H)).copy(),
            cw1=np.transpose(np.asarray(cw1, F32).reshape(2, P, 2), (1, 0, 2)).astype(BF),
            cb1=np.broadcast_to(np.asarray(cb1, F32), (GPC, 2)).copy(),
        ))
    trace = False
    try:  # register NTFF hook so exec_time_ns is measurable (best effort)
        import sys, types
        import antenv
        if "antenv.axon_hooks" not in sys.modules:
            hooks = types.ModuleType("antenv.axon_hooks")
            hooks._h = None
            hooks.set_axon_ntff_profile_hook = lambda h: setattr(hooks, "_h", h)
            hooks.get_axon_ntff_profile_hook = lambda: hooks._h
            sys.modules["antenv.axon_hooks"] = hooks
            antenv.axon_hooks = hooks
            from trn_agent_boot.trn_boot import _ntff_profile_via_ctypes
            hk = _ntff_profile_via_ctypes("/opt/axon/libaxon_pjrt.so")
            if hk is not None:
                hooks.set_axon_ntff_profile_hook(hk)
                trace = True
        else:
            trace = True
    except Exception:
        trace = False
    res = run_bass_kernel_spmd(nc, ins, core_ids=list(range(CORES)),
                               trace=trace)
    logits = np.concatenate([res.results[c]["out"] for c in range(CORES)], 0)
    globals()["last_exec_ns"] = res.exec_time_ns
    return logits.astype(np.float32)


# revision 18
# speedup vs baseline: 1.8213x; 1.0253x over previous
"""GCN (EnhancedTaintFlowGNN) on 8 Trainium2 NeuronCores.

Sharding: 32 graphs/core (batch sorted -> contiguous node ranges). Per GCN
layer: M = h @ W locally (PE), AllGather M into a pair-Shared HBM table,
then pull-side aggregation WITHOUT seg matrices: dma_gather source rows
(per-owner int16 tables), PE per-block run-reduction (R matrices with the
GCN norm coefs folded in, runs never cross block boundaries so output
slots have unique dst per owner region), dma_scatter_add of the reduced
f32 rows into a local agg slab (bias folded into the self-loop init
write; scatter calls are WAW-serialized by tile so cross-call duplicate
dsts cannot race the non-atomic CCE add). Layer 0 gathers rows of
T0 = emb @ w0 (replicated vocab table) so no exchange is needed.
Pooling: masked DVE mean/max on transposed features; classifier MLP on
PE.
"""
import numpy as np
import ml_dtypes

import concourse.bass as bass
import concourse.bacc as bacc
import concourse.tile as tile
from concourse import mybir
from concourse.bass_utils import run_bass_kernel_spmd
from concourse.masks import make_identity

P = 128
N, E, G, V, D, H = 150000, 300000, 256, 1000, 128, 256
CORES = 8
GPC = G // CORES
CH = 8         # 128-slot blocks per gather/scatter call (1024-desc ring cap)
BF = ml_dtypes.bfloat16
F32 = np.float32


def _idx16_layout(idx):
    n = idx.shape[0]
    s = n // 16
    out = np.zeros((P, s), np.int16)
    blk = idx.reshape(s, 16).T.astype(np.int16)
    for g in range(8):
        out[g * 16:(g + 1) * 16, :] = blk
    return out


def _pad128(x):
    return (int(x) + P - 1) // P * P


def _pm(idx, nblk):
    """Partition-major row remap: node d -> (d%128)*nblk + d//128."""
    return (idx % P) * nblk + idx // P


def _region_layout(dst_loc):
    """Positions for dst-sorted slots such that no dst run crosses a
    128 boundary. Returns (order, slot_pos, run_id, run_j, run_dst,
    padded_len). run_j = out-slot index of each run within its block."""
    order = np.argsort(dst_loc, kind="stable")
    ds = dst_loc[order]
    n = len(ds)
    if n == 0:
        return order, np.zeros(0, np.int64), None, None, None, 0
    starts = np.flatnonzero(np.r_[True, ds[1:] != ds[:-1]])
    runlen = np.diff(np.r_[starts, n])
    run_start = np.zeros(len(starts), np.int64)
    pos = 0
    for i, L in enumerate(runlen):
        if pos % P + L > P:
            pos = _pad128(pos)
        run_start[i] = pos
        pos += L
    run_id = np.repeat(np.arange(len(starts)), runlen)
    slot_pos = run_start[run_id] + (np.arange(n) - starts[run_id])
    # out-slot index per run within its block = rank - first_rank_of_block
    rb = run_start // P
    _, first = np.unique(rb, return_index=True)
    first_of_block = np.zeros(rb.max() + 1, np.int64)
    first_of_block[rb[first]] = np.arange(len(starts))[first]
    run_j = np.arange(len(starts)) - first_of_block[rb]
    run_dst = ds[starts]
    return order, slot_pos, run_id, (rb, run_j), run_dst, _pad128(pos)


def _fill_region(gidx, sidx, Rf, base, src_loc, dst_loc, coef):
    """Fill gidx/sidx/R for one owner region starting at slot `base`."""
    order, slot_pos, run_id, rj, run_dst, plen = _region_layout(dst_loc)
    if plen == 0:
        return 0
    rb, run_j = rj
    sp = base + slot_pos
    gidx[sp] = src_loc[order]
    blk = sp // P
    Rf[blk, sp % P, run_j[run_id]] = coef[order]
    base_blk = base // P
    sidx[(base_blk + rb) * P + run_j] = run_dst
    return plen


def _preprocess(node_ids, edge_index, batch):
    node_ids = np.asarray(node_ids).astype(np.int64)
    src_g = np.asarray(edge_index)[0].astype(np.int64)
    dst_g = np.asarray(edge_index)[1].astype(np.int64)
    batch = np.asarray(batch).astype(np.int64)

    deg = 1.0 + np.bincount(dst_g, minlength=N).astype(np.float64)
    dinv = (1.0 / np.sqrt(deg)).astype(F32)
    selfnorm = (dinv * dinv).astype(F32)

    gcnt = np.bincount(batch, minlength=G)
    goff = np.zeros(G + 1, np.int64)
    goff[1:] = np.cumsum(gcnt)
    node_start = np.array([int(goff[c * GPC]) for c in range(CORES + 1)])
    n_c = node_start[1:] - node_start[:-1]
    TILES = int(np.ceil(n_c.max() / P))
    SLAB = TILES * P
    assert SLAB <= 32767, SLAB

    e_oc = np.searchsorted(node_start[1:], dst_g, side="right")
    e_os = np.searchsorted(node_start[1:], src_g, side="right")

    # ---- measure padded region lengths (uniform across cores) ----
    len1 = np.zeros((CORES, CORES), np.int64)
    per_edge = []
    for c in range(CORES):
        m = e_oc == c
        srcs, dsts, owns = src_g[m], dst_g[m], e_os[m]
        dstl = dsts - node_start[c]
        coefs = dinv[srcs] * dinv[dsts]
        per_edge.append((srcs, dstl, owns, coefs))
        for o in range(CORES):
            mo = owns == o
            _, _, _, _, _, plen = _region_layout(dstl[mo])
            len1[c, o] = plen
    K1 = [_pad128(len1[:, o].max()) for o in range(CORES)]
    off1 = np.zeros(CORES + 1, np.int64)
    off1[1:] = np.cumsum(K1)
    S1 = int(off1[-1])
    S0 = 0
    NB1 = S1 // P

    per_core = []
    for c in range(CORES):
        srcs, dstl, owns, coefs = per_edge[c]

        TILESN = SLAB // P
        AGB = TILESN + 1
        gidx1 = np.zeros(S1, np.int64)
        sidx1 = np.full(S1, -1, np.int64)
        R1 = np.zeros((NB1, P, P), F32)
        for o in range(CORES):
            mo = owns == o
            _fill_region(gidx1, sidx1, R1, int(off1[o]),
                         _pm(srcs[mo] - node_start[o], TILESN),
                         dstl[mo], coefs[mo])
        real = sidx1 >= 0
        sidx1[real] = _pm(sidx1[real], AGB)
        sidx1[~real] = TILESN

        selfn = np.zeros((P, TILES), F32)
        ar = np.arange(int(n_c[c]))
        selfn[ar % P, ar // P] = selfnorm[node_start[c]:node_start[c + 1]]

        # Rpm[p, blk*128 + j] = R1[blk, p, j]
        Rpm = np.ascontiguousarray(
            R1.transpose(1, 0, 2).reshape(P, -1).astype(BF))

        # layer-0 dense coefficient matrix: agg0 = C @ [T0; b0]
        # C[d, v] = sum of coefs of edges (dst=d, vid(src)=v) + self term;
        # column V (=1000) is the bias-ones column.
        vids = node_ids[srcs]
        C = np.zeros((SLAB, 9 * P), F32)
        np.add.at(C, (dstl, vids), coefs)
        arn = np.arange(int(n_c[c]))
        C[arn, node_ids[node_start[c]:node_start[c + 1]]] += \
            selfnorm[node_start[c]:node_start[c + 1]]
        C[:int(n_c[c]), 1024] = 1.0
        ct = np.ascontiguousarray(
            C.reshape(TILES, P, 9, P).transpose(3, 0, 2, 1)
            .reshape(P, -1).astype(BF))

        per_core.append(dict(
            gidx1=_idx16_layout(gidx1.astype(np.int16)),
            sidx1=_idx16_layout(sidx1.astype(np.int16)),
            rmat=Rpm, selfn=selfn, ct=ct,
        ))

    # graph cover ranges for pooling (uniform across cores)
    glo_all = np.stack([goff[c * GPC:(c + 1) * GPC] - node_start[c]
                        for c in range(CORES)])   # [CORES, GPC]
    ghi_all = np.stack([goff[c * GPC + 1:(c + 1) * GPC + 1] - node_start[c]
                        for c in range(CORES)])
    lo_fix = glo_all.min(0)
    hi_fix = ghi_all.max(0)
    MAXCOV = int((hi_fix - lo_fix).max())
    MAXCOV = (MAXCOV + 31) // 32 * 32
    cov_len = np.minimum(MAXCOV, SLAB - lo_fix)

    for c in range(CORES):
        mask = np.zeros((GPC, MAXCOV), F32)
        cinv = np.zeros(GPC, F32)
        for j in range(GPC):
            lo = int(glo_all[c, j]) - int(lo_fix[j])
            hi = int(ghi_all[c, j]) - int(lo_fix[j])
            mask[j, lo:hi] = 1.0
            cnt_ = int(ghi_all[c, j] - glo_all[c, j])
            cinv[j] = 1.0 / max(cnt_, 1)
        maskb = np.broadcast_to(mask[:, None, :], (GPC, P, MAXCOV))
        per_core[c]["mask"] = np.ascontiguousarray(maskb.astype(BF))
        per_core[c]["cinv"] = np.broadcast_to(cinv, (P, GPC)).copy()

    meta = dict(TILES=TILES, SLAB=SLAB, S0=S0, S1=S1,
                K1=[int(k) for k in K1], off1=[int(o) for o in off1],
                lo_fix=lo_fix, cov_len=cov_len, MAXCOV=MAXCOV)
    return per_core, meta


def _chunks(meta, layer):
    """Static (slot_base, nblocks, owner) chunk list; identical on all
    cores. Chunks never span owner regions."""
    out = []
    for o in range(CORES):
        base, nb_tot = meta["off1"][o], meta["K1"][o] // P
        for b0 in range(0, nb_tot, CH):
            out.append((base + b0 * P, min(CH, nb_tot - b0), o))
    return out


def _rgroups(chunks, cap=8):
    """Group consecutive chunks into runs of <= cap blocks (for one big
    R-matrix load per group; chunk slots are contiguous within a layer)."""
    groups, cur, acc = [], [], 0
    for ch in chunks:
        if acc + ch[1] > cap and cur:
            groups.append(cur)
            cur, acc = [], 0
        cur.append(ch)
        acc += ch[1]
    if cur:
        groups.append(cur)
    return groups


def _build(meta):
    TILES, SLAB = meta["TILES"], meta["SLAB"]
    S0, S1 = meta["S0"], meta["S1"]
    NB0 = S0 // P
    MAXCOV = meta["MAXCOV"]
    f32, bf16, i16 = mybir.dt.float32, mybir.dt.bfloat16, mybir.dt.int16
    f16 = mybir.dt.float16
    AGB = TILES + 1
    AF = mybir.ActivationFunctionType
    nc = bacc.Bacc("TRN2", target_bir_lowering=False, debug=False,
                   num_devices=CORES, dynamic_dma_scratch_size=32768,
                   num_swdge_queues=2)

    gidx1_in = nc.dram_tensor("gidx1", [P, S1 // 16], i16, kind="ExternalInput")
    sidx1_in = nc.dram_tensor("sidx1", [P, S1 // 16], i16, kind="ExternalInput")
    ct_in = nc.dram_tensor("ct", [P, TILES * 9 * P], bf16,
                           kind="ExternalInput")
    rmat_in = nc.dram_tensor("rmat", [P, S1], bf16,
                             kind="ExternalInput")
    selfn_in = nc.dram_tensor("selfn", [P, TILES], f32, kind="ExternalInput")
    mask_in = nc.dram_tensor("mask", [GPC, P, MAXCOV], bf16,
                             kind="ExternalInput")
    cinv_in = nc.dram_tensor("cinv", [P, GPC], f32, kind="ExternalInput")
    embT_in = nc.dram_tensor("embT", [P, 1024], bf16, kind="ExternalInput")
    w0_in = nc.dram_tensor("w0", [P, H], bf16, kind="ExternalInput")
    wl_in = nc.dram_tensor("wl", [P, 3, 2, H], bf16, kind="ExternalInput")
    b_in = nc.dram_tensor("bias", [P, 4, H], f16, kind="ExternalInput")
    cw0_in = nc.dram_tensor("cw0", [P, 4, H], bf16, kind="ExternalInput")
    cb0_in = nc.dram_tensor("cb0", [GPC, H], f32, kind="ExternalInput")
    cw1_in = nc.dram_tensor("cw1", [P, 2, 2], bf16, kind="ExternalInput")
    cb1_in = nc.dram_tensor("cb1", [GPC, 2], f32, kind="ExternalInput")
    out = nc.dram_tensor("out", [GPC, 2], f32, kind="ExternalOutput")

    MB = nc.dram_tensor("MBd", [P, TILES * H], bf16)
    MFs = [nc.dram_tensor(f"MF{i}", [CORES * P, TILES * H], bf16,
                          addr_space="Shared") for i in range(2)]
    AGG = nc.dram_tensor("AGGd", [P, AGB * H], f16)
    AGG2 = nc.dram_tensor("AGG2d", [P, AGB * H], f16)

    MB_v = None  # set below once tensors exist

    with tile.TileContext(nc) as tc:
        with (
            tc.tile_pool(name="const", bufs=1) as cpool,
            tc.tile_pool(name="gat", bufs=3) as gpool,
            tc.tile_pool(name="rp", bufs=2) as rpool,
            tc.tile_pool(name="sca", bufs=2) as spool,
            tc.tile_pool(name="work", bufs=5) as work,
            tc.tile_pool(name="mst", bufs=2) as mst,
            tc.tile_pool(name="ist", bufs=2) as ist,
            tc.tile_pool(name="rst", bufs=2) as rst,
            tc.tile_pool(name="hmp", bufs=1) as hmp,
            tc.tile_pool(name="pstr", bufs=2, space="PSUM") as pstr,
            tc.tile_pool(name="psmm", bufs=2, space="PSUM") as psmm,
            tc.tile_pool(name="psrd", bufs=4, space="PSUM") as psrd,
        ):
            gidx1_t = cpool.tile([P, S1 // 16], i16)
            nc.sync.dma_start(out=gidx1_t[:], in_=gidx1_in[:, :])
            sidx1_t = cpool.tile([P, S1 // 16], i16)
            nc.sync.dma_start(out=sidx1_t[:], in_=sidx1_in[:, :])
            selfn_t = cpool.tile([P, TILES], f32)
            nc.sync.dma_start(out=selfn_t[:], in_=selfn_in[:, :])
            cinv_t = cpool.tile([P, GPC], f32)
            nc.sync.dma_start(out=cinv_t[:], in_=cinv_in[:, :])
            embT_t = cpool.tile([P, 1024], bf16)
            nc.sync.dma_start(out=embT_t[:], in_=embT_in[:, :])
            w0_t = cpool.tile([P, H], bf16)
            nc.sync.dma_start(out=w0_t[:], in_=w0_in[:, :])
            wl_t = cpool.tile([P, 3, 2, H], bf16)
            nc.sync.dma_start(out=wl_t[:], in_=wl_in[:, :, :, :])
            bias_t = cpool.tile([P, 4, H], f16)
            nc.sync.dma_start(out=bias_t[:], in_=b_in[:, :, :])

            h_T = nc.alloc_sbuf_tensor("hT", [P, 2, SLAB], bf16)
            ident = cpool.tile([P, P], bf16)
            make_identity(nc, ident[:])

            AGG_v = AGG[:, :].rearrange("p (b e) -> p b e", e=H)
            AGG_sc = AGG[:, :].rearrange("p (b e) -> (p b) e", e=H)
            AGG2_v = AGG2[:, :].rearrange("p (b e) -> p b e", e=H)
            AGG2_sc = AGG2[:, :].rearrange("p (b e) -> (p b) e", e=H)
            MB_v = MB[:, :].rearrange("p (b e) -> p b e", e=H)

            # ---- T0 = emb @ w0 (kept in SBUF); block 8 = bias broadcast --
            t0st = cpool.tile([P, 9, H], bf16, tag="t0")
            for vb in range(8):
                pt0 = psmm.tile([P, H], f32, tag="mm")
                nc.tensor.matmul(out=pt0[:], lhsT=embT_t[:, vb * P:(vb + 1) * P],
                                 rhs=w0_t[:], start=True, stop=True)
                nc.scalar.copy(out=t0st[:, vb, :], in_=pt0[:])
            nc.vector.tensor_copy(out=t0st[:, 8, :], in_=bias_t[:, 0, :])

            def edge_phase(layer, mf):
                gidx_t, sidx_t, rbase = gidx1_t, sidx1_t, 0
                ci = 0
                for grp in _rgroups(_chunks(meta, layer)):
                    g0 = grp[0][0]            # first slot of group
                    gnb = sum(ch[1] for ch in grp)
                    rt = rpool.tile([P, 8 * P], bf16)
                    nc.sync.dma_start(
                        out=rt[:, 0:gnb * P],
                        in_=rmat_in[:, rbase + g0:rbase + g0 + gnb * P])
                    for (base, nb, o) in grp:
                        gt = gpool.tile([P, CH, H], bf16)
                        tab = mf[o * P:(o + 1) * P, :].rearrange(
                            "p (b e) -> (p b) e", e=H)
                        nc.gpsimd.dma_gather(
                            out_ap=gt[:, 0:nb, :], in_ap=tab,
                            idxs_ap=gidx_t[:, base // 16:(base + nb * P) // 16],
                            num_idxs=nb * P, num_idxs_reg=nb * P,
                            elem_size=H, queue_num=0)
                        sf = spool.tile([P, CH, H], f16)
                        for b in range(nb):
                            rb = (base - g0) // P + b
                            pr = psrd.tile([P, H], f32, tag="rd")
                            nc.tensor.matmul(out=pr[:],
                                             lhsT=rt[:, rb * P:(rb + 1) * P],
                                             rhs=gt[:, b, :],
                                             start=True, stop=True)
                            nc.vector.tensor_copy(out=sf[:, b, :], in_=pr[:])
                        nc.gpsimd.dma_scatter_add(
                            AGG_sc if ci % 2 == 0 else AGG2_sc,
                            sf[:, 0:nb, :],
                            sidx_t[:, base // 16:(base + nb * P) // 16],
                            nb * P, nb * P, H, queue_num=1)
                        ci += 1

            def readback_phase(layer):
                for t0 in range(0, TILES, CH):
                    nb = min(CH, TILES - t0)
                    at = rst.tile([P, CH, H], f16, tag="a")
                    nc.sync.dma_start(out=at[:, 0:nb, :],
                                      in_=AGG_v[:, t0:t0 + nb, :])
                    at2 = rst.tile([P, CH, H], f16, tag="b")
                    nc.sync.dma_start(out=at2[:, 0:nb, :],
                                      in_=AGG2_v[:, t0:t0 + nb, :])
                    nc.vector.tensor_add(out=at[:, 0:nb, :],
                                         in0=at[:, 0:nb, :],
                                         in1=at2[:, 0:nb, :])
                    for b in range(nb):
                        t = t0 + b
                        hb = work.tile([P, H], bf16)
                        nc.scalar.activation(out=hb[:], in_=at[:, b, :],
                                             func=AF.Relu)
                        for fh in range(2):
                            ptr_ = pstr.tile([P, P], bf16)
                            nc.tensor.transpose(
                                out=ptr_[:], in_=hb[:, fh * P:(fh + 1) * P],
                                identity=ident[:])
                            nc.vector.tensor_copy(
                                out=h_T[:, fh, t * P:(t + 1) * P], in_=ptr_[:])

            # ---- layer 0: h = relu(C @ [T0; b0]) entirely on PE ----
            CTG = 2
            TW = 9 * P
            for tg in range(0, TILES, CTG):
                ng = min(CTG, TILES - tg)
                ctt = gpool.tile([P, CTG * TW], bf16, tag="ct")
                nc.sync.dma_start(out=ctt[:, 0:ng * TW],
                                  in_=ct_in[:, tg * TW:(tg + ng) * TW])
                for i in range(ng):
                    t = tg + i
                    pm = psmm.tile([P, H], f32, tag="mm")
                    for vb in range(9):
                        c0 = (i * 9 + vb) * P
                        nc.tensor.matmul(out=pm[:],
                                         lhsT=ctt[:, c0:c0 + P],
                                         rhs=t0st[:, vb, :],
                                         start=(vb == 0), stop=(vb == 8))
                    hb = work.tile([P, H], bf16)
                    nc.scalar.activation(out=hb[:], in_=pm[:], func=AF.Relu)
                    for fh in range(2):
                        ptr_ = pstr.tile([P, P], bf16)
                        nc.tensor.transpose(
                            out=ptr_[:], in_=hb[:, fh * P:(fh + 1) * P],
                            identity=ident[:])
                        nc.vector.tensor_copy(
                            out=h_T[:, fh, t * P:(t + 1) * P], in_=ptr_[:])

            zst = cpool.tile([P, CH, H], f16, tag="z")
            nc.vector.memset(zst[:], 0.0)
            for layer in (1, 2, 3):
                mf = MFs[layer % 2]
                for t0 in range(0, TILES, CH):
                    nb = min(CH, TILES - t0)
                    nc.sync.dma_start(out=AGG2_v[:, t0:t0 + nb, :],
                                      in_=zst[:, 0:nb, :])
                # M = h @ W -> MB; agg init = selfnorm*M + bias
                for t0 in range(0, TILES, CH):
                    nb = min(CH, TILES - t0)
                    mtile = mst.tile([P, CH, H], bf16, tag="m")
                    itile = ist.tile([P, CH, H], f16, tag="i")
                    for b in range(nb):
                        t = t0 + b
                        pm = psmm.tile([P, H], f32, tag="mm")
                        for fh in range(2):
                            nc.tensor.matmul(
                                out=pm[:],
                                lhsT=h_T[:, fh, t * P:(t + 1) * P],
                                rhs=wl_t[:, layer - 1, fh, :],
                                start=(fh == 0), stop=(fh == 1))
                        nc.scalar.copy(out=mtile[:, b, :], in_=pm[:])
                        nc.scalar.activation(out=itile[:, b, :], in_=pm[:],
                                             func=AF.Copy,
                                             scale=selfn_t[:, t:t + 1])
                        nc.vector.tensor_add(out=itile[:, b, :],
                                             in0=itile[:, b, :],
                                             in1=bias_t[:, layer, :])
                    nc.sync.dma_start(out=MB_v[:, t0:t0 + nb, :],
                                      in_=mtile[:, 0:nb, :])
                    nc.sync.dma_start(out=AGG_v[:, t0:t0 + nb, :],
                                      in_=itile[:, 0:nb, :])
                nc.gpsimd.collective_compute(
                    "AllGather", mybir.AluOpType.bypass,
                    replica_groups=[list(range(CORES))],
                    ins=[MB[:].opt()], outs=[mf[:].opt()])
                edge_phase(layer, mf)
                readback_phase(layer)

            # ---- pooling (masked mean/max on h_T) ----
            pooled = []
            for fh in range(2):
                mean_t = cpool.tile([P, GPC], f32, tag=f"mean{fh}")
                max_t = cpool.tile([P, GPC], f32, tag=f"max{fh}")
                nc.vector.memset(mean_t[:], 0.0)
                nc.vector.memset(max_t[:], 0.0)
                pooled.append((mean_t, max_t))
            lo_fix, cov_len = meta["lo_fix"], meta["cov_len"]
            for j in range(GPC):
                mk = hmp.tile([P, MAXCOV], bf16, tag="mask")
                ln = int(cov_len[j])
                nc.sync.dma_start(out=mk[:, 0:ln], in_=mask_in[j, :, 0:ln])
                for fh in range(2):
                    hm = hmp.tile([P, MAXCOV], bf16, tag="hm")
                    lo = int(lo_fix[j])
                    nc.vector.tensor_mul(out=hm[:, 0:ln],
                                         in0=h_T[:, fh, lo:lo + ln],
                                         in1=mk[:, 0:ln])
                    nc.vector.tensor_reduce(
                        out=pooled[fh][0][:, j:j + 1], in_=hm[:, 0:ln],
                        axis=mybir.AxisListType.X, op=mybir.AluOpType.add)
                    nc.vector.tensor_reduce(
                        out=pooled[fh][1][:, j:j + 1], in_=hm[:, 0:ln],
                        axis=mybir.AxisListType.X, op=mybir.AluOpType.max)
            # scale means by 1/cnt, cast to bf16 lhsT chunks
            chunks = []
            for fh in range(2):
                mean_t, max_t = pooled[fh]
                nc.vector.tensor_mul(out=mean_t[:], in0=mean_t[:],
                                     in1=cinv_t[:])
            for (kind, fh) in ((0, 0), (0, 1), (1, 0), (1, 1)):
                src = pooled[fh][kind]
                cb = work.tile([P, GPC], bf16, tag=f"ch{kind}{fh}")
                nc.vector.tensor_copy(out=cb[:], in_=src[:])
                chunks.append(cb)

            # ---- classifier MLP ----
            cw0_t = cpool.tile([P, 4, H], bf16)
            nc.sync.dma_start(out=cw0_t[:], in_=cw0_in[:, :, :])
            cb0_t = cpool.tile([GPC, H], f32)
            nc.sync.dma_start(out=cb0_t[:], in_=cb0_in[:, :])
            cw1_t = cpool.tile([P, 2, 2], bf16)
            nc.sync.dma_start(out=cw1_t[:], in_=cw1_in[:, :, :])
            cb1_t = cpool.tile([GPC, 2], f32)
            nc.sync.dma_start(out=cb1_t[:], in_=cb1_in[:, :])

            ph_full = psmm.tile([P, H], f32, tag="mm")
            ph = ph_full[0:GPC, :]
            for k in range(4):
                nc.tensor.matmul(out=ph[:], lhsT=chunks[k][:],
                                 rhs=cw0_t[:, k, :],
                                 start=(k == 0), stop=(k == 3))
            hc1 = work.tile([GPC, H], f32, tag="hc1")
            nc.vector.tensor_add(out=hc1[:], in0=ph[:], in1=cb0_t[:])
            hcb = work.tile([GPC, H], bf16, tag="hcb")
            nc.scalar.activation(out=hcb[:], in_=hc1[:], func=AF.Relu)
            hTt = []
            for k in range(2):
                ptr_ = pstr.tile([P, P], bf16)
                nc.tensor.transpose(out=ptr_[0:P, 0:GPC],
                                    in_=hcb[:, k * P:(k + 1) * P],
                                    identity=ident[0:GPC, 0:GPC])
                ht = work.tile([P, GPC], bf16, tag=f"hTt{k}")
                nc.vector.tensor_copy(out=ht[:], in_=ptr_[0:P, 0:GPC])
                hTt.append(ht)
            pl_full = psmm.tile([P, H], f32, tag="mm")
            pl = pl_full[0:GPC, 0:2]
            for k in range(2):
                nc.tensor.matmul(out=pl[:], lhsT=hTt[k][:],
                                 rhs=cw1_t[:, k, :],
                                 start=(k == 0), stop=(k == 1))
            lg = work.tile([GPC, 2], f32, tag="lg")
            nc.vector.tensor_add(out=lg[:], in0=pl[:], in1=cb1_t[:])
            nc.sync.dma_start(out=out[:, :], in_=lg[:])
    nc.finalize()
    return nc


def kernel(node_ids, edge_index, batch, emb, w0, b0, w1, b1, w2, b2, w3, b3,
           cw0, cb0, cw1, cb1):
    per_core, meta = _preprocess(node_ids, edge_index, batch)
    nc = _build(meta)

    embT = np.zeros((P, 1024), F32)
    embT[:, :V] = np.asarray(emb, F32).T
    wlk = np.transpose(np.stack([np.asarray(w, F32).reshape(2, P, H)
                    for w in (w1, w2, w3)]), (2, 0, 1, 3)).copy()
    biases = np.stack([np.broadcast_to(np.asarray(b, F32), (P, H))
                       for b in (b0, b1, b2, b3)], axis=1).astype(np.float16)
    ins = []
    for c in range(CORES):
        pc = per_core[c]
        ins.append(dict(
            gidx1=pc["gidx1"], sidx1=pc["sidx1"], ct=pc["ct"],
            rmat=pc["rmat"], selfn=pc["selfn"],
            mask=pc["mask"], cinv=pc["cinv"],
            embT=embT.astype(BF), w0=np.asarray(w0, F32).astype(BF),
            wl=wlk.astype(BF), bias=biases,
            cw0=np.transpose(np.asarray(cw0, F32).reshape(4, P, H), (1, 0, 2)).astype(BF),
            cb0=np.broadcast_to(np.asarray(cb0, F32), (GPC, H)).copy(),
            cw1=np.transpose(np.asarray(cw1, F32).reshape(2, P, 2), (1, 0, 2)).astype(BF),
            cb1=np.broadcast_to(np.asarray(cb1, F32), (GPC, 2)).copy(),
        ))
    trace = False
    try:  # register NTFF hook so exec_time_ns is measurable (best effort)
        import sys, types
        import antenv
        if "antenv.axon_hooks" not in sys.modules:
            hooks = types.ModuleType("antenv.axon_hooks")
            hooks._h = None
            hooks.set_axon_ntff_profile_hook = lambda h: setattr(hooks, "_h", h)
            hooks.get_axon_ntff_profile_hook = lambda: hooks._h
            sys.modules["antenv.axon_hooks"] = hooks
            antenv.axon_hooks = hooks
            from trn_agent_boot.trn_boot import _ntff_profile_via_ctypes
            hk = _ntff_profile_via_ctypes("/opt/axon/libaxon_pjrt.so")
            if hk is not None:
                hooks.set_axon_ntff_profile_hook(hk)
                trace = True
        else:
            trace = True
    except Exception:
        trace = False
    res = run_bass_kernel_spmd(nc, ins, core_ids=list(range(CORES)),
                               trace=trace)
    logits = np.concatenate([res.results[c]["out"] for c in range(CORES)], 0)
    globals()["last_exec_ns"] = res.exec_time_ns
    return logits.astype(np.float32)


# revision 20
# speedup vs baseline: 1.8870x; 1.0360x over previous
"""GCN (EnhancedTaintFlowGNN) on 8 Trainium2 NeuronCores.

Sharding: 32 graphs/core (batch sorted -> contiguous node ranges). Per GCN
layer: M = h @ W locally (PE), AllGather M into a pair-Shared HBM table,
then pull-side aggregation WITHOUT seg matrices: dma_gather source rows
(per-owner int16 tables), PE per-block run-reduction (R matrices with the
GCN norm coefs folded in, runs never cross block boundaries so output
slots have unique dst per owner region), dma_scatter_add of the reduced
f32 rows into a local agg slab (bias folded into the self-loop init
write; scatter calls are WAW-serialized by tile so cross-call duplicate
dsts cannot race the non-atomic CCE add). Layer 0 gathers rows of
T0 = emb @ w0 (replicated vocab table) so no exchange is needed.
Pooling: masked DVE mean/max on transposed features; classifier MLP on
PE.
"""
import numpy as np
import ml_dtypes

import concourse.bass as bass
import concourse.bacc as bacc
import concourse.tile as tile
from concourse import mybir
from concourse.bass_utils import run_bass_kernel_spmd
from concourse.masks import make_identity

P = 128
N, E, G, V, D, H = 150000, 300000, 256, 1000, 128, 256
CORES = 8
GPC = G // CORES
CH = 8         # 128-slot blocks per gather/scatter call (1024-desc ring cap)
BF = ml_dtypes.bfloat16
F32 = np.float32


def _idx16_layout(idx):
    n = idx.shape[0]
    s = n // 16
    out = np.zeros((P, s), np.int16)
    blk = idx.reshape(s, 16).T.astype(np.int16)
    for g in range(8):
        out[g * 16:(g + 1) * 16, :] = blk
    return out


def _pad128(x):
    return (int(x) + P - 1) // P * P


def _pm(idx, nblk):
    """Partition-major row remap: node d -> (d%128)*nblk + d//128."""
    return (idx % P) * nblk + idx // P


def _region_layout(dst_loc):
    """Positions for dst-sorted slots such that no dst run crosses a
    128 boundary. Returns (order, slot_pos, run_id, run_j, run_dst,
    padded_len). run_j = out-slot index of each run within its block."""
    order = np.argsort(dst_loc, kind="stable")
    ds = dst_loc[order]
    n = len(ds)
    if n == 0:
        return order, np.zeros(0, np.int64), None, None, None, 0
    starts = np.flatnonzero(np.r_[True, ds[1:] != ds[:-1]])
    runlen = np.diff(np.r_[starts, n])
    run_start = np.zeros(len(starts), np.int64)
    pos = 0
    for i, L in enumerate(runlen):
        if pos % P + L > P:
            pos = _pad128(pos)
        run_start[i] = pos
        pos += L
    run_id = np.repeat(np.arange(len(starts)), runlen)
    slot_pos = run_start[run_id] + (np.arange(n) - starts[run_id])
    # out-slot index per run within its block = rank - first_rank_of_block
    rb = run_start // P
    _, first = np.unique(rb, return_index=True)
    first_of_block = np.zeros(rb.max() + 1, np.int64)
    first_of_block[rb[first]] = np.arange(len(starts))[first]
    run_j = np.arange(len(starts)) - first_of_block[rb]
    run_dst = ds[starts]
    return order, slot_pos, run_id, (rb, run_j), run_dst, _pad128(pos)


def _fill_region(gidx, sidx, Rf, base, src_loc, dst_loc, coef):
    """Fill gidx/sidx/R for one owner region starting at slot `base`."""
    order, slot_pos, run_id, rj, run_dst, plen = _region_layout(dst_loc)
    if plen == 0:
        return 0
    rb, run_j = rj
    sp = base + slot_pos
    gidx[sp] = src_loc[order]
    blk = sp // P
    Rf[blk, sp % P, run_j[run_id]] = coef[order]
    base_blk = base // P
    sidx[(base_blk + rb) * P + run_j] = run_dst
    return plen


def _preprocess(node_ids, edge_index, batch):
    node_ids = np.asarray(node_ids).astype(np.int64)
    src_g = np.asarray(edge_index)[0].astype(np.int64)
    dst_g = np.asarray(edge_index)[1].astype(np.int64)
    batch = np.asarray(batch).astype(np.int64)

    deg = 1.0 + np.bincount(dst_g, minlength=N).astype(np.float64)
    dinv = (1.0 / np.sqrt(deg)).astype(F32)
    selfnorm = (dinv * dinv).astype(F32)

    gcnt = np.bincount(batch, minlength=G)
    goff = np.zeros(G + 1, np.int64)
    goff[1:] = np.cumsum(gcnt)
    node_start = np.array([int(goff[c * GPC]) for c in range(CORES + 1)])
    n_c = node_start[1:] - node_start[:-1]
    TILES = int(np.ceil(n_c.max() / P))
    SLAB = TILES * P
    assert SLAB <= 32767, SLAB

    e_oc = np.searchsorted(node_start[1:], dst_g, side="right")
    e_os = np.searchsorted(node_start[1:], src_g, side="right")

    # ---- measure padded region lengths (uniform across cores) ----
    # 16 regions: (half h, owner o) with h = src-tile-half of the owner slab
    HA = (TILES + 1) // 2            # tiles in half A
    HB = TILES - HA
    len1 = np.zeros((CORES, 2 * CORES), np.int64)
    per_edge = []
    for c in range(CORES):
        m = e_oc == c
        srcs, dsts, owns = src_g[m], dst_g[m], e_os[m]
        dstl = dsts - node_start[c]
        coefs = dinv[srcs] * dinv[dsts]
        srcl = srcs - node_start[owns]
        halfs = (srcl >= HA * P).astype(np.int64)
        per_edge.append((srcs, dstl, owns, coefs, srcl, halfs))
        for r in range(2 * CORES):
            h, o = r // CORES, r % CORES
            mo = (owns == o) & (halfs == h)
            _, _, _, _, _, plen = _region_layout(dstl[mo])
            len1[c, r] = plen
    K1 = [_pad128(len1[:, r].max()) for r in range(2 * CORES)]
    off1 = np.zeros(2 * CORES + 1, np.int64)
    off1[1:] = np.cumsum(K1)
    S1 = int(off1[-1])
    S0 = 0
    NB1 = S1 // P

    per_core = []
    for c in range(CORES):
        srcs, dstl, owns, coefs, srcl, halfs = per_edge[c]

        TILESN = SLAB // P
        AGB = TILESN + 1
        gidx1 = np.zeros(S1, np.int64)
        sidx1 = np.full(S1, -1, np.int64)
        R1 = np.zeros((NB1, P, P), F32)
        for r in range(2 * CORES):
            h, o = r // CORES, r % CORES
            mo = (owns == o) & (halfs == h)
            rel = srcl[mo] - h * HA * P
            _fill_region(gidx1, sidx1, R1, int(off1[r]),
                         _pm(rel, HA if h == 0 else HB),
                         dstl[mo], coefs[mo])
        real = sidx1 >= 0
        sidx1[real] = _pm(sidx1[real], AGB)
        sidx1[~real] = TILESN

        selfn = np.zeros((P, TILES), F32)
        ar = np.arange(int(n_c[c]))
        selfn[ar % P, ar // P] = selfnorm[node_start[c]:node_start[c + 1]]

        # Rpm[p, blk*128 + j] = R1[blk, p, j]
        Rpm = np.ascontiguousarray(
            R1.transpose(1, 0, 2).reshape(P, -1).astype(BF))

        # layer-0 dense coefficient matrix: agg0 = C @ [T0; b0]
        # C[d, v] = sum of coefs of edges (dst=d, vid(src)=v) + self term;
        # column V (=1000) is the bias-ones column.
        vids = node_ids[srcs]
        C = np.zeros((SLAB, 9 * P), F32)
        np.add.at(C, (dstl, vids), coefs)
        arn = np.arange(int(n_c[c]))
        C[arn, node_ids[node_start[c]:node_start[c + 1]]] += \
            selfnorm[node_start[c]:node_start[c + 1]]
        C[:int(n_c[c]), 1024] = 1.0
        ct = np.ascontiguousarray(
            C.reshape(TILES, P, 9, P).transpose(3, 0, 2, 1)
            .reshape(P, -1).astype(BF))

        per_core.append(dict(
            gidx1=_idx16_layout(gidx1.astype(np.int16)),
            sidx1=_idx16_layout(sidx1.astype(np.int16)),
            rmat=Rpm, selfn=selfn, ct=ct,
        ))

    # graph cover ranges for pooling (uniform across cores)
    glo_all = np.stack([goff[c * GPC:(c + 1) * GPC] - node_start[c]
                        for c in range(CORES)])   # [CORES, GPC]
    ghi_all = np.stack([goff[c * GPC + 1:(c + 1) * GPC + 1] - node_start[c]
                        for c in range(CORES)])
    lo_fix = glo_all.min(0)
    hi_fix = ghi_all.max(0)
    MAXCOV = int((hi_fix - lo_fix).max())
    MAXCOV = (MAXCOV + 31) // 32 * 32
    cov_len = np.minimum(MAXCOV, SLAB - lo_fix)

    for c in range(CORES):
        mask = np.zeros((GPC, MAXCOV), F32)
        cinv = np.zeros(GPC, F32)
        for j in range(GPC):
            lo = int(glo_all[c, j]) - int(lo_fix[j])
            hi = int(ghi_all[c, j]) - int(lo_fix[j])
            mask[j, lo:hi] = 1.0
            cnt_ = int(ghi_all[c, j] - glo_all[c, j])
            cinv[j] = 1.0 / max(cnt_, 1)
        maskb = np.broadcast_to(mask[:, None, :], (GPC, P, MAXCOV))
        per_core[c]["mask"] = np.ascontiguousarray(maskb.astype(BF))
        per_core[c]["cinv"] = np.broadcast_to(cinv, (P, GPC)).copy()

    meta = dict(TILES=TILES, SLAB=SLAB, S0=S0, S1=S1, HA=HA, HB=HB,
                K1=[int(k) for k in K1], off1=[int(o) for o in off1],
                lo_fix=lo_fix, cov_len=cov_len, MAXCOV=MAXCOV)
    return per_core, meta


def _chunks(meta, layer):
    """Static (slot_base, nblocks, owner) chunk list; identical on all
    cores. Chunks never span owner regions."""
    out = []
    for r in range(2 * CORES):
        h, o = r // CORES, r % CORES
        base, nb_tot = meta["off1"][r], meta["K1"][r] // P
        for b0 in range(0, nb_tot, CH):
            out.append((base + b0 * P, min(CH, nb_tot - b0), o, h))
    return out


def _rgroups(chunks, cap=8):
    """Group consecutive chunks into runs of <= cap blocks (for one big
    R-matrix load per group; chunk slots are contiguous within a layer)."""
    groups, cur, acc = [], [], 0
    for ch in chunks:
        if acc + ch[1] > cap and cur:
            groups.append(cur)
            cur, acc = [], 0
        cur.append(ch)
        acc += ch[1]
    if cur:
        groups.append(cur)
    return groups


def _build(meta):
    TILES, SLAB = meta["TILES"], meta["SLAB"]
    S0, S1 = meta["S0"], meta["S1"]
    NB0 = S0 // P
    MAXCOV = meta["MAXCOV"]
    f32, bf16, i16 = mybir.dt.float32, mybir.dt.bfloat16, mybir.dt.int16
    f16 = mybir.dt.float16
    AGB = TILES + 1
    HA, HB = meta["HA"], meta["HB"]
    AF = mybir.ActivationFunctionType
    nc = bacc.Bacc("TRN2", target_bir_lowering=False, debug=False,
                   num_devices=CORES, dynamic_dma_scratch_size=32768,
                   num_swdge_queues=2)

    gidx1_in = nc.dram_tensor("gidx1", [P, S1 // 16], i16, kind="ExternalInput")
    sidx1_in = nc.dram_tensor("sidx1", [P, S1 // 16], i16, kind="ExternalInput")
    ct_in = nc.dram_tensor("ct", [P, TILES * 9 * P], bf16,
                           kind="ExternalInput")
    rmat_in = nc.dram_tensor("rmat", [P, S1], bf16,
                             kind="ExternalInput")
    selfn_in = nc.dram_tensor("selfn", [P, TILES], f32, kind="ExternalInput")
    mask_in = nc.dram_tensor("mask", [GPC, P, MAXCOV], bf16,
                             kind="ExternalInput")
    cinv_in = nc.dram_tensor("cinv", [P, GPC], f32, kind="ExternalInput")
    embT_in = nc.dram_tensor("embT", [P, 1024], bf16, kind="ExternalInput")
    w0_in = nc.dram_tensor("w0", [P, H], bf16, kind="ExternalInput")
    wl_in = nc.dram_tensor("wl", [P, 3, 2, H], bf16, kind="ExternalInput")
    b_in = nc.dram_tensor("bias", [P, 4, H], f16, kind="ExternalInput")
    cw0_in = nc.dram_tensor("cw0", [P, 4, H], bf16, kind="ExternalInput")
    cb0_in = nc.dram_tensor("cb0", [GPC, H], f32, kind="ExternalInput")
    cw1_in = nc.dram_tensor("cw1", [P, 2, 2], bf16, kind="ExternalInput")
    cb1_in = nc.dram_tensor("cb1", [GPC, 2], f32, kind="ExternalInput")
    out = nc.dram_tensor("out", [GPC, 2], f32, kind="ExternalOutput")

    MBA = nc.dram_tensor("MBAd", [P, HA * H], bf16)
    MBB = nc.dram_tensor("MBBd", [P, HB * H], bf16)
    MFAs = [nc.dram_tensor(f"MFA{i}", [CORES * P, HA * H], bf16,
                           addr_space="Shared") for i in range(2)]
    MFBs = [nc.dram_tensor(f"MFB{i}", [CORES * P, HB * H], bf16,
                           addr_space="Shared") for i in range(2)]
    AGG = nc.dram_tensor("AGGd", [P, AGB * H], f16)
    AGG2 = nc.dram_tensor("AGG2d", [P, AGB * H], f16)

    MB_v = None  # set below once tensors exist

    with tile.TileContext(nc) as tc:
        with (
            tc.tile_pool(name="const", bufs=1) as cpool,
            tc.tile_pool(name="gat", bufs=3) as gpool,
            tc.tile_pool(name="rp", bufs=2) as rpool,
            tc.tile_pool(name="sca", bufs=2) as spool,
            tc.tile_pool(name="work", bufs=5) as work,
            tc.tile_pool(name="mst", bufs=2) as mst,
            tc.tile_pool(name="ist", bufs=2) as ist,
            tc.tile_pool(name="rst", bufs=2) as rst,
            tc.tile_pool(name="hmp", bufs=1) as hmp,
            tc.tile_pool(name="pstr", bufs=2, space="PSUM") as pstr,
            tc.tile_pool(name="psmm", bufs=2, space="PSUM") as psmm,
            tc.tile_pool(name="psrd", bufs=4, space="PSUM") as psrd,
        ):
            gidx1_t = cpool.tile([P, S1 // 16], i16)
            nc.sync.dma_start(out=gidx1_t[:], in_=gidx1_in[:, :])
            sidx1_t = cpool.tile([P, S1 // 16], i16)
            nc.sync.dma_start(out=sidx1_t[:], in_=sidx1_in[:, :])
            selfn_t = cpool.tile([P, TILES], f32)
            nc.sync.dma_start(out=selfn_t[:], in_=selfn_in[:, :])
            cinv_t = cpool.tile([P, GPC], f32)
            nc.sync.dma_start(out=cinv_t[:], in_=cinv_in[:, :])
            embT_t = cpool.tile([P, 1024], bf16)
            nc.sync.dma_start(out=embT_t[:], in_=embT_in[:, :])
            w0_t = cpool.tile([P, H], bf16)
            nc.sync.dma_start(out=w0_t[:], in_=w0_in[:, :])
            wl_t = cpool.tile([P, 3, 2, H], bf16)
            nc.sync.dma_start(out=wl_t[:], in_=wl_in[:, :, :, :])
            bias_t = cpool.tile([P, 4, H], f16)
            nc.sync.dma_start(out=bias_t[:], in_=b_in[:, :, :])

            h_T = nc.alloc_sbuf_tensor("hT", [P, 2, SLAB], bf16)
            ident = cpool.tile([P, P], bf16)
            make_identity(nc, ident[:])

            AGG_v = AGG[:, :].rearrange("p (b e) -> p b e", e=H)
            AGG_sc = AGG[:, :].rearrange("p (b e) -> (p b) e", e=H)
            AGG2_v = AGG2[:, :].rearrange("p (b e) -> p b e", e=H)
            AGG2_sc = AGG2[:, :].rearrange("p (b e) -> (p b) e", e=H)
            MBA_v = MBA[:, :].rearrange("p (b e) -> p b e", e=H)
            MBB_v = MBB[:, :].rearrange("p (b e) -> p b e", e=H)

            # ---- T0 = emb @ w0 (kept in SBUF); block 8 = bias broadcast --
            t0st = cpool.tile([P, 9, H], bf16, tag="t0")
            for vb in range(8):
                pt0 = psmm.tile([P, H], f32, tag="mm")
                nc.tensor.matmul(out=pt0[:], lhsT=embT_t[:, vb * P:(vb + 1) * P],
                                 rhs=w0_t[:], start=True, stop=True)
                nc.scalar.copy(out=t0st[:, vb, :], in_=pt0[:])
            nc.vector.tensor_copy(out=t0st[:, 8, :], in_=bias_t[:, 0, :])

            def edge_phase(layer, mfa, mfb):
                gidx_t, sidx_t, rbase = gidx1_t, sidx1_t, 0
                ci = 0
                for grp in _rgroups(_chunks(meta, layer)):
                    g0 = grp[0][0]            # first slot of group
                    gnb = sum(ch[1] for ch in grp)
                    rt = rpool.tile([P, 8 * P], bf16)
                    nc.sync.dma_start(
                        out=rt[:, 0:gnb * P],
                        in_=rmat_in[:, rbase + g0:rbase + g0 + gnb * P])
                    for (base, nb, o, hh) in grp:
                        gt = gpool.tile([P, CH, H], bf16)
                        mfx = mfa if hh == 0 else mfb
                        tab = mfx[o * P:(o + 1) * P, :].rearrange(
                            "p (b e) -> (p b) e", e=H)
                        nc.gpsimd.dma_gather(
                            out_ap=gt[:, 0:nb, :], in_ap=tab,
                            idxs_ap=gidx_t[:, base // 16:(base + nb * P) // 16],
                            num_idxs=nb * P, num_idxs_reg=nb * P,
                            elem_size=H, queue_num=0)
                        sf = spool.tile([P, CH, H], f16)
                        for b in range(nb):
                            rb = (base - g0) // P + b
                            pr = psrd.tile([P, H], f32, tag="rd")
                            nc.tensor.matmul(out=pr[:],
                                             lhsT=rt[:, rb * P:(rb + 1) * P],
                                             rhs=gt[:, b, :],
                                             start=True, stop=True)
                            nc.vector.tensor_copy(out=sf[:, b, :], in_=pr[:])
                        nc.gpsimd.dma_scatter_add(
                            AGG_sc if ci % 2 == 0 else AGG2_sc,
                            sf[:, 0:nb, :],
                            sidx_t[:, base // 16:(base + nb * P) // 16],
                            nb * P, nb * P, H, queue_num=1)
                        ci += 1

            def readback_phase(layer):
                for t0 in range(0, TILES, CH):
                    nb = min(CH, TILES - t0)
                    at = rst.tile([P, CH, H], f16, tag="a")
                    nc.sync.dma_start(out=at[:, 0:nb, :],
                                      in_=AGG_v[:, t0:t0 + nb, :])
                    at2 = rst.tile([P, CH, H], f16, tag="b")
                    nc.sync.dma_start(out=at2[:, 0:nb, :],
                                      in_=AGG2_v[:, t0:t0 + nb, :])
                    nc.vector.tensor_add(out=at[:, 0:nb, :],
                                         in0=at[:, 0:nb, :],
                                         in1=at2[:, 0:nb, :])
                    for b in range(nb):
                        t = t0 + b
                        hb = work.tile([P, H], bf16)
                        nc.scalar.activation(out=hb[:], in_=at[:, b, :],
                                             func=AF.Relu)
                        for fh in range(2):
                            ptr_ = pstr.tile([P, P], bf16)
                            nc.tensor.transpose(
                                out=ptr_[:], in_=hb[:, fh * P:(fh + 1) * P],
                                identity=ident[:])
                            nc.vector.tensor_copy(
                                out=h_T[:, fh, t * P:(t + 1) * P], in_=ptr_[:])

            # ---- layer 0: h = relu(C @ [T0; b0]) entirely on PE ----
            CTG = 2
            TW = 9 * P
            for tg in range(0, TILES, CTG):
                ng = min(CTG, TILES - tg)
                ctt = gpool.tile([P, CTG * TW], bf16, tag="ct")
                nc.sync.dma_start(out=ctt[:, 0:ng * TW],
                                  in_=ct_in[:, tg * TW:(tg + ng) * TW])
                for i in range(ng):
                    t = tg + i
                    pm = psmm.tile([P, H], f32, tag="mm")
                    for vb in range(9):
                        c0 = (i * 9 + vb) * P
                        nc.tensor.matmul(out=pm[:],
                                         lhsT=ctt[:, c0:c0 + P],
                                         rhs=t0st[:, vb, :],
                                         start=(vb == 0), stop=(vb == 8))
                    hb = work.tile([P, H], bf16)
                    nc.scalar.activation(out=hb[:], in_=pm[:], func=AF.Relu)
                    for fh in range(2):
                        ptr_ = pstr.tile([P, P], bf16)
                        nc.tensor.transpose(
                            out=ptr_[:], in_=hb[:, fh * P:(fh + 1) * P],
                            identity=ident[:])
                        nc.vector.tensor_copy(
                            out=h_T[:, fh, t * P:(t + 1) * P], in_=ptr_[:])

            zst = cpool.tile([P, CH, H], f16, tag="z")
            nc.vector.memset(zst[:], 0.0)
            for layer in (1, 2, 3):
                mfa, mfb = MFAs[layer % 2], MFBs[layer % 2]
                for t0 in range(0, TILES, CH):
                    nb = min(CH, TILES - t0)
                    nc.sync.dma_start(out=AGG2_v[:, t0:t0 + nb, :],
                                      in_=zst[:, 0:nb, :])
                # M = h @ W -> MB halves; agg init = selfnorm*M + bias.
                # Half A flushes then its AllGather, so AG-A overlaps the
                # half-B M compute and AG-B overlaps half-A edge work.
                for (lo, hi, mbv, mb_t, mf_t) in ((0, HA, MBA_v, MBA, mfa),
                                                  (HA, TILES, MBB_v, MBB,
                                                   mfb)):
                    for t0 in range(lo, hi, CH):
                        nb = min(CH, hi - t0)
                        mtile = mst.tile([P, CH, H], bf16, tag="m")
                        itile = ist.tile([P, CH, H], f16, tag="i")
                        for b in range(nb):
                            t = t0 + b
                            pm = psmm.tile([P, H], f32, tag="mm")
                            for fh in range(2):
                                nc.tensor.matmul(
                                    out=pm[:],
                                    lhsT=h_T[:, fh, t * P:(t + 1) * P],
                                    rhs=wl_t[:, layer - 1, fh, :],
                                    start=(fh == 0), stop=(fh == 1))
                            nc.scalar.copy(out=mtile[:, b, :], in_=pm[:])
                            nc.scalar.activation(out=itile[:, b, :],
                                                 in_=pm[:], func=AF.Copy,
                                                 scale=selfn_t[:, t:t + 1])
                            nc.vector.tensor_add(out=itile[:, b, :],
                                                 in0=itile[:, b, :],
                                                 in1=bias_t[:, layer, :])
                        nc.sync.dma_start(out=mbv[:, t0 - lo:t0 - lo + nb, :],
                                          in_=mtile[:, 0:nb, :])
                        nc.sync.dma_start(out=AGG_v[:, t0:t0 + nb, :],
                                          in_=itile[:, 0:nb, :])
                    nc.gpsimd.collective_compute(
                        "AllGather", mybir.AluOpType.bypass,
                        replica_groups=[list(range(CORES))],
                        ins=[mb_t[:].opt()], outs=[mf_t[:].opt()])
                edge_phase(layer, mfa, mfb)
                readback_phase(layer)

            # ---- pooling (masked mean/max on h_T) ----
            pooled = []
            for fh in range(2):
                mean_t = cpool.tile([P, GPC], f32, tag=f"mean{fh}")
                max_t = cpool.tile([P, GPC], f32, tag=f"max{fh}")
                nc.vector.memset(mean_t[:], 0.0)
                nc.vector.memset(max_t[:], 0.0)
                pooled.append((mean_t, max_t))
            lo_fix, cov_len = meta["lo_fix"], meta["cov_len"]
            for j in range(GPC):
                mk = hmp.tile([P, MAXCOV], bf16, tag="mask")
                ln = int(cov_len[j])
                nc.sync.dma_start(out=mk[:, 0:ln], in_=mask_in[j, :, 0:ln])
                for fh in range(2):
                    hm = hmp.tile([P, MAXCOV], bf16, tag="hm")
                    lo = int(lo_fix[j])
                    nc.vector.tensor_mul(out=hm[:, 0:ln],
                                         in0=h_T[:, fh, lo:lo + ln],
                                         in1=mk[:, 0:ln])
                    nc.vector.tensor_reduce(
                        out=pooled[fh][0][:, j:j + 1], in_=hm[:, 0:ln],
                        axis=mybir.AxisListType.X, op=mybir.AluOpType.add)
                    nc.vector.tensor_reduce(
                        out=pooled[fh][1][:, j:j + 1], in_=hm[:, 0:ln],
                        axis=mybir.AxisListType.X, op=mybir.AluOpType.max)
            # scale means by 1/cnt, cast to bf16 lhsT chunks
            chunks = []
            for fh in range(2):
                mean_t, max_t = pooled[fh]
                nc.vector.tensor_mul(out=mean_t[:], in0=mean_t[:],
                                     in1=cinv_t[:])
            for (kind, fh) in ((0, 0), (0, 1), (1, 0), (1, 1)):
                src = pooled[fh][kind]
                cb = work.tile([P, GPC], bf16, tag=f"ch{kind}{fh}")
                nc.vector.tensor_copy(out=cb[:], in_=src[:])
                chunks.append(cb)

            # ---- classifier MLP ----
            cw0_t = cpool.tile([P, 4, H], bf16)
            nc.sync.dma_start(out=cw0_t[:], in_=cw0_in[:, :, :])
            cb0_t = cpool.tile([GPC, H], f32)
            nc.sync.dma_start(out=cb0_t[:], in_=cb0_in[:, :])
            cw1_t = cpool.tile([P, 2, 2], bf16)
            nc.sync.dma_start(out=cw1_t[:], in_=cw1_in[:, :, :])
            cb1_t = cpool.tile([GPC, 2], f32)
            nc.sync.dma_start(out=cb1_t[:], in_=cb1_in[:, :])

            ph_full = psmm.tile([P, H], f32, tag="mm")
            ph = ph_full[0:GPC, :]
            for k in range(4):
                nc.tensor.matmul(out=ph[:], lhsT=chunks[k][:],
                                 rhs=cw0_t[:, k, :],
                                 start=(k == 0), stop=(k == 3))
            hc1 = work.tile([GPC, H], f32, tag="hc1")
            nc.vector.tensor_add(out=hc1[:], in0=ph[:], in1=cb0_t[:])
            hcb = work.tile([GPC, H], bf16, tag="hcb")
            nc.scalar.activation(out=hcb[:], in_=hc1[:], func=AF.Relu)
            hTt = []
            for k in range(2):
                ptr_ = pstr.tile([P, P], bf16)
                nc.tensor.transpose(out=ptr_[0:P, 0:GPC],
                                    in_=hcb[:, k * P:(k + 1) * P],
                                    identity=ident[0:GPC, 0:GPC])
                ht = work.tile([P, GPC], bf16, tag=f"hTt{k}")
                nc.vector.tensor_copy(out=ht[:], in_=ptr_[0:P, 0:GPC])
                hTt.append(ht)
            pl_full = psmm.tile([P, H], f32, tag="mm")
            pl = pl_full[0:GPC, 0:2]
            for k in range(2):
                nc.tensor.matmul(out=pl[:], lhsT=hTt[k][:],
                                 rhs=cw1_t[:, k, :],
                                 start=(k == 0), stop=(k == 1))
            lg = work.tile([GPC, 2], f32, tag="lg")
            nc.vector.tensor_add(out=lg[:], in0=pl[:], in1=cb1_t[:])
            nc.sync.dma_start(out=out[:, :], in_=lg[:])
    nc.finalize()
    return nc


def kernel(node_ids, edge_index, batch, emb, w0, b0, w1, b1, w2, b2, w3, b3,
           cw0, cb0, cw1, cb1):
    per_core, meta = _preprocess(node_ids, edge_index, batch)
    nc = _build(meta)

    embT = np.zeros((P, 1024), F32)
    embT[:, :V] = np.asarray(emb, F32).T
    wlk = np.transpose(np.stack([np.asarray(w, F32).reshape(2, P, H)
                    for w in (w1, w2, w3)]), (2, 0, 1, 3)).copy()
    biases = np.stack([np.broadcast_to(np.asarray(b, F32), (P, H))
                       for b in (b0, b1, b2, b3)], axis=1).astype(np.float16)
    ins = []
    for c in range(CORES):
        pc = per_core[c]
        ins.append(dict(
            gidx1=pc["gidx1"], sidx1=pc["sidx1"], ct=pc["ct"],
            rmat=pc["rmat"], selfn=pc["selfn"],
            mask=pc["mask"], cinv=pc["cinv"],
            embT=embT.astype(BF), w0=np.asarray(w0, F32).astype(BF),
            wl=wlk.astype(BF), bias=biases,
            cw0=np.transpose(np.asarray(cw0, F32).reshape(4, P, H), (1, 0, 2)).astype(BF),
            cb0=np.broadcast_to(np.asarray(cb0, F32), (GPC, H)).copy(),
            cw1=np.transpose(np.asarray(cw1, F32).reshape(2, P, 2), (1, 0, 2)).astype(BF),
            cb1=np.broadcast_to(np.asarray(cb1, F32), (GPC, 2)).copy(),
        ))
    trace = False
    try:  # register NTFF hook so exec_time_ns is measurable (best effort)
        import sys, types
        import antenv
        if "antenv.axon_hooks" not in sys.modules:
            hooks = types.ModuleType("antenv.axon_hooks")
            hooks._h = None
            hooks.set_axon_ntff_profile_hook = lambda h: setattr(hooks, "_h", h)
            hooks.get_axon_ntff_profile_hook = lambda: hooks._h
            sys.modules["antenv.axon_hooks"] = hooks
            antenv.axon_hooks = hooks
            from trn_agent_boot.trn_boot import _ntff_profile_via_ctypes
            hk = _ntff_profile_via_ctypes("/opt/axon/libaxon_pjrt.so")
            if hk is not None:
                hooks.set_axon_ntff_profile_hook(hk)
                trace = True
        else:
            trace = True
    except Exception:
        trace = False
    res = run_bass_kernel_spmd(nc, ins, core_ids=list(range(CORES)),
                               trace=trace)
    logits = np.concatenate([res.results[c]["out"] for c in range(CORES)], 0)
    globals()["last_exec_ns"] = res.exec_time_ns
    return logits.astype(np.float32)


# revision 21
# speedup vs baseline: 1.9531x; 1.0351x over previous
"""GCN (EnhancedTaintFlowGNN) on 8 Trainium2 NeuronCores.

Sharding: 32 graphs/core (batch sorted -> contiguous node ranges). Per GCN
layer: M = h @ W locally (PE), AllGather M into a pair-Shared HBM table,
then pull-side aggregation WITHOUT seg matrices: dma_gather source rows
(per-owner int16 tables), PE per-block run-reduction (R matrices with the
GCN norm coefs folded in, runs never cross block boundaries so output
slots have unique dst per owner region), dma_scatter_add of the reduced
f32 rows into a local agg slab (bias folded into the self-loop init
write; scatter calls are WAW-serialized by tile so cross-call duplicate
dsts cannot race the non-atomic CCE add). Layer 0 gathers rows of
T0 = emb @ w0 (replicated vocab table) so no exchange is needed.
Pooling: masked DVE mean/max on transposed features; classifier MLP on
PE.
"""
import numpy as np
import ml_dtypes

import concourse.bass as bass
import concourse.bacc as bacc
import concourse.tile as tile
from concourse import mybir
from concourse.bass_utils import run_bass_kernel_spmd
from concourse.masks import make_identity

P = 128
N, E, G, V, D, H = 150000, 300000, 256, 1000, 128, 256
CORES = 8
GPC = G // CORES
CH = 8         # 128-slot blocks per gather/scatter call (1024-desc ring cap)
BF = ml_dtypes.bfloat16
F32 = np.float32


def _idx16_layout(idx):
    n = idx.shape[0]
    s = n // 16
    out = np.zeros((P, s), np.int16)
    blk = idx.reshape(s, 16).T.astype(np.int16)
    for g in range(8):
        out[g * 16:(g + 1) * 16, :] = blk
    return out


def _pad128(x):
    return (int(x) + P - 1) // P * P


def _pm(idx, nblk):
    """Partition-major row remap: node d -> (d%128)*nblk + d//128."""
    return (idx % P) * nblk + idx // P


def _region_layout(dst_loc):
    """Positions for dst-sorted slots such that no dst run crosses a
    128 boundary. Returns (order, slot_pos, run_id, run_j, run_dst,
    padded_len). run_j = out-slot index of each run within its block."""
    order = np.argsort(dst_loc, kind="stable")
    ds = dst_loc[order]
    n = len(ds)
    if n == 0:
        return order, np.zeros(0, np.int64), None, None, None, 0
    starts = np.flatnonzero(np.r_[True, ds[1:] != ds[:-1]])
    runlen = np.diff(np.r_[starts, n])
    run_start = np.zeros(len(starts), np.int64)
    pos = 0
    for i, L in enumerate(runlen):
        if pos % P + L > P:
            pos = _pad128(pos)
        run_start[i] = pos
        pos += L
    run_id = np.repeat(np.arange(len(starts)), runlen)
    slot_pos = run_start[run_id] + (np.arange(n) - starts[run_id])
    # out-slot index per run within its block = rank - first_rank_of_block
    rb = run_start // P
    _, first = np.unique(rb, return_index=True)
    first_of_block = np.zeros(rb.max() + 1, np.int64)
    first_of_block[rb[first]] = np.arange(len(starts))[first]
    run_j = np.arange(len(starts)) - first_of_block[rb]
    run_dst = ds[starts]
    return order, slot_pos, run_id, (rb, run_j), run_dst, _pad128(pos)


def _fill_region(gidx, sidx, Rf, base, src_loc, dst_loc, coef):
    """Fill gidx/sidx/R for one owner region starting at slot `base`."""
    order, slot_pos, run_id, rj, run_dst, plen = _region_layout(dst_loc)
    if plen == 0:
        return 0
    rb, run_j = rj
    sp = base + slot_pos
    gidx[sp] = src_loc[order]
    blk = sp // P
    Rf[blk, sp % P, run_j[run_id]] = coef[order]
    base_blk = base // P
    sidx[(base_blk + rb) * P + run_j] = run_dst
    return plen


def _preprocess(node_ids, edge_index, batch):
    node_ids = np.asarray(node_ids).astype(np.int64)
    src_g = np.asarray(edge_index)[0].astype(np.int64)
    dst_g = np.asarray(edge_index)[1].astype(np.int64)
    batch = np.asarray(batch).astype(np.int64)

    deg = 1.0 + np.bincount(dst_g, minlength=N).astype(np.float64)
    dinv = (1.0 / np.sqrt(deg)).astype(F32)
    selfnorm = (dinv * dinv).astype(F32)

    gcnt = np.bincount(batch, minlength=G)
    goff = np.zeros(G + 1, np.int64)
    goff[1:] = np.cumsum(gcnt)
    node_start = np.array([int(goff[c * GPC]) for c in range(CORES + 1)])
    n_c = node_start[1:] - node_start[:-1]
    TILES = int(np.ceil(n_c.max() / P))
    SLAB = TILES * P
    assert SLAB <= 32767, SLAB

    e_oc = np.searchsorted(node_start[1:], dst_g, side="right")
    e_os = np.searchsorted(node_start[1:], src_g, side="right")

    # ---- measure padded region lengths (uniform across cores) ----
    # 16 regions: (half h, owner o) with h = src-tile-half of the owner slab
    HA = (TILES + 1) // 2            # tiles in half A
    HB = TILES - HA
    len1 = np.zeros((CORES, 2 * CORES), np.int64)
    per_edge = []
    for c in range(CORES):
        m = e_oc == c
        srcs, dsts, owns = src_g[m], dst_g[m], e_os[m]
        dstl = dsts - node_start[c]
        coefs = dinv[srcs] * dinv[dsts]
        srcl = srcs - node_start[owns]
        halfs = (srcl >= HA * P).astype(np.int64)
        per_edge.append((srcs, dstl, owns, coefs, srcl, halfs))
        for r in range(2 * CORES):
            h, o = r // CORES, r % CORES
            mo = (owns == o) & (halfs == h)
            _, _, _, _, _, plen = _region_layout(dstl[mo])
            len1[c, r] = plen
    K1 = [_pad128(len1[:, r].max()) for r in range(2 * CORES)]
    off1 = np.zeros(2 * CORES + 1, np.int64)
    off1[1:] = np.cumsum(K1)
    S1 = int(off1[-1])
    S0 = 0
    NB1 = S1 // P

    per_core = []
    for c in range(CORES):
        srcs, dstl, owns, coefs, srcl, halfs = per_edge[c]

        TILESN = SLAB // P
        AGB = TILESN + 1
        gidx1 = np.zeros(S1, np.int64)
        sidx1 = np.full(S1, -1, np.int64)
        R1 = np.zeros((NB1, P, P), F32)
        for r in range(2 * CORES):
            h, o = r // CORES, r % CORES
            mo = (owns == o) & (halfs == h)
            rel = srcl[mo] - h * HA * P
            _fill_region(gidx1, sidx1, R1, int(off1[r]),
                         _pm(rel, HA if h == 0 else HB),
                         dstl[mo], coefs[mo])
        real = sidx1 >= 0
        sidx1[real] = _pm(sidx1[real], AGB)
        sidx1[~real] = TILESN

        selfn = np.zeros((P, TILES), F32)
        ar = np.arange(int(n_c[c]))
        selfn[ar % P, ar // P] = selfnorm[node_start[c]:node_start[c + 1]]

        # Rpm[p, blk*128 + j] = R1[blk, p, j]
        Rpm = np.ascontiguousarray(
            R1.transpose(1, 0, 2).reshape(P, -1).astype(BF))

        # layer-0 dense coefficient matrix: agg0 = C @ [T0; b0]
        # C[d, v] = sum of coefs of edges (dst=d, vid(src)=v) + self term;
        # column V (=1000) is the bias-ones column.
        vids = node_ids[srcs]
        C = np.zeros((SLAB, 9 * P), F32)
        np.add.at(C, (dstl, vids), coefs)
        arn = np.arange(int(n_c[c]))
        C[arn, node_ids[node_start[c]:node_start[c + 1]]] += \
            selfnorm[node_start[c]:node_start[c + 1]]
        C[:int(n_c[c]), 1024] = 1.0
        ct = np.ascontiguousarray(
            C.reshape(TILES, P, 9, P).transpose(3, 0, 2, 1)
            .reshape(P, -1).astype(BF))

        per_core.append(dict(
            gidx1=_idx16_layout(gidx1.astype(np.int16)),
            sidx1=_idx16_layout(sidx1.astype(np.int16)),
            rmat=Rpm, selfn=selfn, ct=ct,
        ))

    # graph cover ranges for pooling (uniform across cores)
    glo_all = np.stack([goff[c * GPC:(c + 1) * GPC] - node_start[c]
                        for c in range(CORES)])   # [CORES, GPC]
    ghi_all = np.stack([goff[c * GPC + 1:(c + 1) * GPC + 1] - node_start[c]
                        for c in range(CORES)])
    lo_fix = glo_all.min(0)
    hi_fix = ghi_all.max(0)
    MAXCOV = int((hi_fix - lo_fix).max())
    MAXCOV = (MAXCOV + 31) // 32 * 32
    cov_len = np.minimum(MAXCOV, SLAB - lo_fix)

    for c in range(CORES):
        mask = np.zeros((GPC, MAXCOV), F32)
        cinv = np.zeros(GPC, F32)
        for j in range(GPC):
            lo = int(glo_all[c, j]) - int(lo_fix[j])
            hi = int(ghi_all[c, j]) - int(lo_fix[j])
            mask[j, lo:hi] = 1.0
            cnt_ = int(ghi_all[c, j] - glo_all[c, j])
            cinv[j] = 1.0 / max(cnt_, 1)
        maskb = np.broadcast_to(mask[:, None, :], (GPC, P, MAXCOV))
        per_core[c]["mask"] = np.ascontiguousarray(maskb.astype(BF))
        per_core[c]["cinv"] = np.broadcast_to(cinv, (P, GPC)).copy()

    meta = dict(TILES=TILES, SLAB=SLAB, S0=S0, S1=S1, HA=HA, HB=HB,
                K1=[int(k) for k in K1], off1=[int(o) for o in off1],
                lo_fix=lo_fix, cov_len=cov_len, MAXCOV=MAXCOV)
    return per_core, meta


def _chunks(meta, layer):
    """Static (slot_base, nblocks, owner) chunk list; identical on all
    cores. Chunks never span owner regions."""
    out = []
    for r in range(2 * CORES):
        h, o = r // CORES, r % CORES
        base, nb_tot = meta["off1"][r], meta["K1"][r] // P
        for b0 in range(0, nb_tot, CH):
            out.append((base + b0 * P, min(CH, nb_tot - b0), o, h))
    return out


def _rgroups(chunks, cap=8):
    """Group consecutive chunks into runs of <= cap blocks (for one big
    R-matrix load per group; chunk slots are contiguous within a layer)."""
    groups, cur, acc = [], [], 0
    for ch in chunks:
        if acc + ch[1] > cap and cur:
            groups.append(cur)
            cur, acc = [], 0
        cur.append(ch)
        acc += ch[1]
    if cur:
        groups.append(cur)
    return groups


def _build(meta):
    TILES, SLAB = meta["TILES"], meta["SLAB"]
    S0, S1 = meta["S0"], meta["S1"]
    NB0 = S0 // P
    MAXCOV = meta["MAXCOV"]
    f32, bf16, i16 = mybir.dt.float32, mybir.dt.bfloat16, mybir.dt.int16
    f16 = mybir.dt.float16
    AGB = TILES + 1
    HA, HB = meta["HA"], meta["HB"]
    AF = mybir.ActivationFunctionType
    nc = bacc.Bacc("TRN2", target_bir_lowering=False, debug=False,
                   num_devices=CORES, dynamic_dma_scratch_size=32768,
                   num_swdge_queues=2)

    gidx1_in = nc.dram_tensor("gidx1", [P, S1 // 16], i16, kind="ExternalInput")
    sidx1_in = nc.dram_tensor("sidx1", [P, S1 // 16], i16, kind="ExternalInput")
    ct_in = nc.dram_tensor("ct", [P, TILES * 9 * P], bf16,
                           kind="ExternalInput")
    rmat_in = nc.dram_tensor("rmat", [P, S1], bf16,
                             kind="ExternalInput")
    selfn_in = nc.dram_tensor("selfn", [P, TILES], f32, kind="ExternalInput")
    mask_in = nc.dram_tensor("mask", [GPC, P, MAXCOV], bf16,
                             kind="ExternalInput")
    cinv_in = nc.dram_tensor("cinv", [P, GPC], f32, kind="ExternalInput")
    embT_in = nc.dram_tensor("embT", [P, 1024], bf16, kind="ExternalInput")
    w0_in = nc.dram_tensor("w0", [P, H], bf16, kind="ExternalInput")
    wl_in = nc.dram_tensor("wl", [P, 3, 2, H], bf16, kind="ExternalInput")
    b_in = nc.dram_tensor("bias", [P, 4, H], f16, kind="ExternalInput")
    cw0_in = nc.dram_tensor("cw0", [P, 4, H], bf16, kind="ExternalInput")
    cb0_in = nc.dram_tensor("cb0", [GPC, H], f32, kind="ExternalInput")
    cw1_in = nc.dram_tensor("cw1", [P, 2, 2], bf16, kind="ExternalInput")
    cb1_in = nc.dram_tensor("cb1", [GPC, 2], f32, kind="ExternalInput")
    out = nc.dram_tensor("out", [GPC, 2], f32, kind="ExternalOutput")

    MBA = nc.dram_tensor("MBAd", [P, HA * H], bf16)
    MBB = nc.dram_tensor("MBBd", [P, HB * H], bf16)
    MFAs = [nc.dram_tensor(f"MFA{i}", [CORES * P, HA * H], bf16,
                           addr_space="Shared") for i in range(2)]
    MFBs = [nc.dram_tensor(f"MFB{i}", [CORES * P, HB * H], bf16,
                           addr_space="Shared") for i in range(2)]
    AGG = nc.dram_tensor("AGGd", [P, AGB * H], f16)
    AGG2 = nc.dram_tensor("AGG2d", [P, AGB * H], f16)

    MB_v = None  # set below once tensors exist

    with tile.TileContext(nc) as tc:
        with (
            tc.tile_pool(name="const", bufs=1) as cpool,
            tc.tile_pool(name="gat", bufs=3) as gpool,
            tc.tile_pool(name="rp", bufs=3) as rpool,
            tc.tile_pool(name="sca", bufs=2) as spool,
            tc.tile_pool(name="work", bufs=4) as work,
            tc.tile_pool(name="mst", bufs=2) as mst,
            tc.tile_pool(name="ist", bufs=2) as ist,
            tc.tile_pool(name="rst", bufs=2) as rst,
            tc.tile_pool(name="hmp", bufs=1) as hmp,
            tc.tile_pool(name="pstr", bufs=2, space="PSUM") as pstr,
            tc.tile_pool(name="psmm", bufs=2, space="PSUM") as psmm,
            tc.tile_pool(name="psrd", bufs=4, space="PSUM") as psrd,
        ):
            gidx1_t = cpool.tile([P, S1 // 16], i16)
            nc.sync.dma_start(out=gidx1_t[:], in_=gidx1_in[:, :])
            sidx1_t = cpool.tile([P, S1 // 16], i16)
            nc.sync.dma_start(out=sidx1_t[:], in_=sidx1_in[:, :])
            selfn_t = cpool.tile([P, TILES], f32)
            nc.sync.dma_start(out=selfn_t[:], in_=selfn_in[:, :])
            cinv_t = cpool.tile([P, GPC], f32)
            nc.sync.dma_start(out=cinv_t[:], in_=cinv_in[:, :])
            embT_t = cpool.tile([P, 1024], bf16)
            nc.sync.dma_start(out=embT_t[:], in_=embT_in[:, :])
            w0_t = cpool.tile([P, H], bf16)
            nc.sync.dma_start(out=w0_t[:], in_=w0_in[:, :])
            wl_t = cpool.tile([P, 3, 2, H], bf16)
            nc.sync.dma_start(out=wl_t[:], in_=wl_in[:, :, :, :])
            bias_t = cpool.tile([P, 4, H], f16)
            nc.sync.dma_start(out=bias_t[:], in_=b_in[:, :, :])

            h_T = nc.alloc_sbuf_tensor("hT", [P, 2, SLAB], bf16)
            ident = cpool.tile([P, P], bf16)
            make_identity(nc, ident[:])

            AGG_v = AGG[:, :].rearrange("p (b e) -> p b e", e=H)
            AGG_sc = AGG[:, :].rearrange("p (b e) -> (p b) e", e=H)
            AGG2_v = AGG2[:, :].rearrange("p (b e) -> p b e", e=H)
            AGG2_sc = AGG2[:, :].rearrange("p (b e) -> (p b) e", e=H)
            MBA_v = MBA[:, :].rearrange("p (b e) -> p b e", e=H)
            MBB_v = MBB[:, :].rearrange("p (b e) -> p b e", e=H)

            # ---- T0 = emb @ w0 (kept in SBUF); block 8 = bias broadcast --
            t0st = cpool.tile([P, 9, H], bf16, tag="t0")
            for vb in range(8):
                pt0 = psmm.tile([P, H], f32, tag="mm")
                nc.tensor.matmul(out=pt0[:], lhsT=embT_t[:, vb * P:(vb + 1) * P],
                                 rhs=w0_t[:], start=True, stop=True)
                nc.scalar.copy(out=t0st[:, vb, :], in_=pt0[:])
            nc.vector.tensor_copy(out=t0st[:, 8, :], in_=bias_t[:, 0, :])

            def edge_phase(layer, mfa, mfb):
                gidx_t, sidx_t, rbase = gidx1_t, sidx1_t, 0
                ci = 0
                for grp in _rgroups(_chunks(meta, layer)):
                    g0 = grp[0][0]            # first slot of group
                    gnb = sum(ch[1] for ch in grp)
                    rt = rpool.tile([P, 8 * P], bf16)
                    nc.sync.dma_start(
                        out=rt[:, 0:gnb * P],
                        in_=rmat_in[:, rbase + g0:rbase + g0 + gnb * P])
                    for (base, nb, o, hh) in grp:
                        gt = gpool.tile([P, CH, H], bf16)
                        mfx = mfa if hh == 0 else mfb
                        tab = mfx[o * P:(o + 1) * P, :].rearrange(
                            "p (b e) -> (p b) e", e=H)
                        nc.gpsimd.dma_gather(
                            out_ap=gt[:, 0:nb, :], in_ap=tab,
                            idxs_ap=gidx_t[:, base // 16:(base + nb * P) // 16],
                            num_idxs=nb * P, num_idxs_reg=nb * P,
                            elem_size=H, queue_num=0)
                        sf = spool.tile([P, CH, H], f16)
                        for b in range(nb):
                            rb = (base - g0) // P + b
                            pr = psrd.tile([P, H], f32, tag="rd")
                            nc.tensor.matmul(out=pr[:],
                                             lhsT=rt[:, rb * P:(rb + 1) * P],
                                             rhs=gt[:, b, :],
                                             start=True, stop=True)
                            nc.vector.tensor_copy(out=sf[:, b, :], in_=pr[:])
                        nc.gpsimd.dma_scatter_add(
                            AGG_sc if ci % 2 == 0 else AGG2_sc,
                            sf[:, 0:nb, :],
                            sidx_t[:, base // 16:(base + nb * P) // 16],
                            nb * P, nb * P, H, queue_num=1)
                        ci += 1

            def readback_phase(layer):
                for t0 in range(0, TILES, CH):
                    nb = min(CH, TILES - t0)
                    at = rst.tile([P, CH, H], f16, tag="a")
                    nc.sync.dma_start(out=at[:, 0:nb, :],
                                      in_=AGG_v[:, t0:t0 + nb, :])
                    at2 = rst.tile([P, CH, H], f16, tag="b")
                    nc.sync.dma_start(out=at2[:, 0:nb, :],
                                      in_=AGG2_v[:, t0:t0 + nb, :])
                    nc.vector.tensor_add(out=at[:, 0:nb, :],
                                         in0=at[:, 0:nb, :],
                                         in1=at2[:, 0:nb, :])
                    for b in range(nb):
                        t = t0 + b
                        hb = work.tile([P, H], bf16)
                        nc.scalar.activation(out=hb[:], in_=at[:, b, :],
                                             func=AF.Relu)
                        for fh in range(2):
                            ptr_ = pstr.tile([P, P], bf16)
                            nc.tensor.transpose(
                                out=ptr_[:], in_=hb[:, fh * P:(fh + 1) * P],
                                identity=ident[:])
                            nc.vector.tensor_copy(
                                out=h_T[:, fh, t * P:(t + 1) * P], in_=ptr_[:])

            # ---- layer 0: h = relu(C @ [T0; b0]) entirely on PE ----
            CTG = 2
            TW = 9 * P
            for tg in range(0, TILES, CTG):
                ng = min(CTG, TILES - tg)
                ctt = gpool.tile([P, CTG * TW], bf16, tag="ct")
                nc.sync.dma_start(out=ctt[:, 0:ng * TW],
                                  in_=ct_in[:, tg * TW:(tg + ng) * TW])
                for i in range(ng):
                    t = tg + i
                    pm = psmm.tile([P, H], f32, tag="mm")
                    for vb in range(9):
                        c0 = (i * 9 + vb) * P
                        nc.tensor.matmul(out=pm[:],
                                         lhsT=ctt[:, c0:c0 + P],
                                         rhs=t0st[:, vb, :],
                                         start=(vb == 0), stop=(vb == 8))
                    hb = work.tile([P, H], bf16)
                    nc.scalar.activation(out=hb[:], in_=pm[:], func=AF.Relu)
                    for fh in range(2):
                        ptr_ = pstr.tile([P, P], bf16)
                        nc.tensor.transpose(
                            out=ptr_[:], in_=hb[:, fh * P:(fh + 1) * P],
                            identity=ident[:])
                        nc.vector.tensor_copy(
                            out=h_T[:, fh, t * P:(t + 1) * P], in_=ptr_[:])

            zst = cpool.tile([P, CH, H], f16, tag="z")
            nc.vector.memset(zst[:], 0.0)
            for layer in (1, 2, 3):
                mfa, mfb = MFAs[layer % 2], MFBs[layer % 2]
                for t0 in range(0, TILES, CH):
                    nb = min(CH, TILES - t0)
                    nc.sync.dma_start(out=AGG2_v[:, t0:t0 + nb, :],
                                      in_=zst[:, 0:nb, :])
                # M = h @ W -> MB halves; agg init = selfnorm*M + bias.
                # Half A flushes then its AllGather, so AG-A overlaps the
                # half-B M compute and AG-B overlaps half-A edge work.
                for (lo, hi, mbv, mb_t, mf_t) in ((0, HA, MBA_v, MBA, mfa),
                                                  (HA, TILES, MBB_v, MBB,
                                                   mfb)):
                    for t0 in range(lo, hi, CH):
                        nb = min(CH, hi - t0)
                        mtile = mst.tile([P, CH, H], bf16, tag="m")
                        itile = ist.tile([P, CH, H], f16, tag="i")
                        for b in range(nb):
                            t = t0 + b
                            pm = psmm.tile([P, H], f32, tag="mm")
                            for fh in range(2):
                                nc.tensor.matmul(
                                    out=pm[:],
                                    lhsT=h_T[:, fh, t * P:(t + 1) * P],
                                    rhs=wl_t[:, layer - 1, fh, :],
                                    start=(fh == 0), stop=(fh == 1))
                            nc.scalar.copy(out=mtile[:, b, :], in_=pm[:])
                            nc.scalar.activation(out=itile[:, b, :],
                                                 in_=pm[:], func=AF.Copy,
                                                 scale=selfn_t[:, t:t + 1])
                            nc.vector.tensor_add(out=itile[:, b, :],
                                                 in0=itile[:, b, :],
                                                 in1=bias_t[:, layer, :])
                        nc.sync.dma_start(out=mbv[:, t0 - lo:t0 - lo + nb, :],
                                          in_=mtile[:, 0:nb, :])
                        nc.sync.dma_start(out=AGG_v[:, t0:t0 + nb, :],
                                          in_=itile[:, 0:nb, :])
                    nc.gpsimd.collective_compute(
                        "AllGather", mybir.AluOpType.bypass,
                        replica_groups=[list(range(CORES))],
                        ins=[mb_t[:].opt()], outs=[mf_t[:].opt()])
                edge_phase(layer, mfa, mfb)
                readback_phase(layer)

            # ---- pooling (masked mean/max on h_T) ----
            pooled = []
            for fh in range(2):
                mean_t = cpool.tile([P, GPC], f32, tag=f"mean{fh}")
                max_t = cpool.tile([P, GPC], f32, tag=f"max{fh}")
                nc.vector.memset(mean_t[:], 0.0)
                nc.vector.memset(max_t[:], 0.0)
                pooled.append((mean_t, max_t))
            lo_fix, cov_len = meta["lo_fix"], meta["cov_len"]
            for j in range(GPC):
                mk = hmp.tile([P, MAXCOV], bf16, tag="mask")
                ln = int(cov_len[j])
                nc.sync.dma_start(out=mk[:, 0:ln], in_=mask_in[j, :, 0:ln])
                for fh in range(2):
                    hm = hmp.tile([P, MAXCOV], bf16, tag="hm")
                    lo = int(lo_fix[j])
                    nc.vector.tensor_mul(out=hm[:, 0:ln],
                                         in0=h_T[:, fh, lo:lo + ln],
                                         in1=mk[:, 0:ln])
                    nc.vector.tensor_reduce(
                        out=pooled[fh][0][:, j:j + 1], in_=hm[:, 0:ln],
                        axis=mybir.AxisListType.X, op=mybir.AluOpType.add)
                    nc.vector.tensor_reduce(
                        out=pooled[fh][1][:, j:j + 1], in_=hm[:, 0:ln],
                        axis=mybir.AxisListType.X, op=mybir.AluOpType.max)
            # scale means by 1/cnt, cast to bf16 lhsT chunks
            chunks = []
            for fh in range(2):
                mean_t, max_t = pooled[fh]
                nc.vector.tensor_mul(out=mean_t[:], in0=mean_t[:],
                                     in1=cinv_t[:])
            for (kind, fh) in ((0, 0), (0, 1), (1, 0), (1, 1)):
                src = pooled[fh][kind]
                cb = work.tile([P, GPC], bf16, tag=f"ch{kind}{fh}")
                nc.vector.tensor_copy(out=cb[:], in_=src[:])
                chunks.append(cb)

            # ---- classifier MLP ----
            cw0_t = cpool.tile([P, 4, H], bf16)
            nc.sync.dma_start(out=cw0_t[:], in_=cw0_in[:, :, :])
            cb0_t = cpool.tile([GPC, H], f32)
            nc.sync.dma_start(out=cb0_t[:], in_=cb0_in[:, :])
            cw1_t = cpool.tile([P, 2, 2], bf16)
            nc.sync.dma_start(out=cw1_t[:], in_=cw1_in[:, :, :])
            cb1_t = cpool.tile([GPC, 2], f32)
            nc.sync.dma_start(out=cb1_t[:], in_=cb1_in[:, :])

            ph_full = psmm.tile([P, H], f32, tag="mm")
            ph = ph_full[0:GPC, :]
            for k in range(4):
                nc.tensor.matmul(out=ph[:], lhsT=chunks[k][:],
                                 rhs=cw0_t[:, k, :],
                                 start=(k == 0), stop=(k == 3))
            hc1 = work.tile([GPC, H], f32, tag="hc1")
            nc.vector.tensor_add(out=hc1[:], in0=ph[:], in1=cb0_t[:])
            hcb = work.tile([GPC, H], bf16, tag="hcb")
            nc.scalar.activation(out=hcb[:], in_=hc1[:], func=AF.Relu)
            hTt = []
            for k in range(2):
                ptr_ = pstr.tile([P, P], bf16)
                nc.tensor.transpose(out=ptr_[0:P, 0:GPC],
                                    in_=hcb[:, k * P:(k + 1) * P],
                                    identity=ident[0:GPC, 0:GPC])
                ht = work.tile([P, GPC], bf16, tag=f"hTt{k}")
                nc.vector.tensor_copy(out=ht[:], in_=ptr_[0:P, 0:GPC])
                hTt.append(ht)
            pl_full = psmm.tile([P, H], f32, tag="mm")
            pl = pl_full[0:GPC, 0:2]
            for k in range(2):
                nc.tensor.matmul(out=pl[:], lhsT=hTt[k][:],
                                 rhs=cw1_t[:, k, :],
                                 start=(k == 0), stop=(k == 1))
            lg = work.tile([GPC, 2], f32, tag="lg")
            nc.vector.tensor_add(out=lg[:], in0=pl[:], in1=cb1_t[:])
            nc.sync.dma_start(out=out[:, :], in_=lg[:])
    nc.finalize()
    return nc


def kernel(node_ids, edge_index, batch, emb, w0, b0, w1, b1, w2, b2, w3, b3,
           cw0, cb0, cw1, cb1):
    per_core, meta = _preprocess(node_ids, edge_index, batch)
    nc = _build(meta)

    embT = np.zeros((P, 1024), F32)
    embT[:, :V] = np.asarray(emb, F32).T
    wlk = np.transpose(np.stack([np.asarray(w, F32).reshape(2, P, H)
                    for w in (w1, w2, w3)]), (2, 0, 1, 3)).copy()
    biases = np.stack([np.broadcast_to(np.asarray(b, F32), (P, H))
                       for b in (b0, b1, b2, b3)], axis=1).astype(np.float16)
    ins = []
    for c in range(CORES):
        pc = per_core[c]
        ins.append(dict(
            gidx1=pc["gidx1"], sidx1=pc["sidx1"], ct=pc["ct"],
            rmat=pc["rmat"], selfn=pc["selfn"],
            mask=pc["mask"], cinv=pc["cinv"],
            embT=embT.astype(BF), w0=np.asarray(w0, F32).astype(BF),
            wl=wlk.astype(BF), bias=biases,
            cw0=np.transpose(np.asarray(cw0, F32).reshape(4, P, H), (1, 0, 2)).astype(BF),
            cb0=np.broadcast_to(np.asarray(cb0, F32), (GPC, H)).copy(),
            cw1=np.transpose(np.asarray(cw1, F32).reshape(2, P, 2), (1, 0, 2)).astype(BF),
            cb1=np.broadcast_to(np.asarray(cb1, F32), (GPC, 2)).copy(),
        ))
    trace = False
    try:  # register NTFF hook so exec_time_ns is measurable (best effort)
        import sys, types
        import antenv
        if "antenv.axon_hooks" not in sys.modules:
            hooks = types.ModuleType("antenv.axon_hooks")
            hooks._h = None
            hooks.set_axon_ntff_profile_hook = lambda h: setattr(hooks, "_h", h)
            hooks.get_axon_ntff_profile_hook = lambda: hooks._h
            sys.modules["antenv.axon_hooks"] = hooks
            antenv.axon_hooks = hooks
            from trn_agent_boot.trn_boot import _ntff_profile_via_ctypes
            hk = _ntff_profile_via_ctypes("/opt/axon/libaxon_pjrt.so")
            if hk is not None:
                hooks.set_axon_ntff_profile_hook(hk)
                trace = True
        else:
            trace = True
    except Exception:
        trace = False
    res = run_bass_kernel_spmd(nc, ins, core_ids=list(range(CORES)),
                               trace=trace)
    logits = np.concatenate([res.results[c]["out"] for c in range(CORES)], 0)
    globals()["last_exec_ns"] = res.exec_time_ns
    return logits.astype(np.float32)
